# revision 1
# baseline (speedup 1.0000x reference)
"""Self-contained Trainium2 Bass kernel for nn_GAT_transformer.

kernel(**inputs) -> np.ndarray [8192, 6] (log_softmax output).

Strategy: 8-core SPMD. Nodes (and their incident edges, grouped by dst)
are sharded across cores. GAT message passing uses dma_gather from an
AllGathered per-node table ([h bf16 | asrc f32 | adst f32] packed rows),
segment-softmax without max subtraction, and per-128-edge-block one-hot
selector matmuls on the PE with a fused denominator column. The dense
NxN attention runs in S^T orientation (keys on partitions) with
row-sharded Q, AllGathered K/V, exp on the scalar engine, and the
softmax denominator folded in as a ones-column of V.
"""
import numpy as np
from contextlib import ExitStack

import concourse.bacc as bacc
import concourse.tile as tile
from concourse import mybir
from concourse.bass_utils import run_bass_kernel_spmd

import numpy as np

N = 8192
E = 262144
D_IN = 256
HEADS = 8
HID = 8
D_OUT = 6
S_MAX = 64
NEG_SLOPE = 0.2
LN_EPS = 1e-5
NCORES = 8
P = N // NCORES            # 1024 nodes per core
GROUP = 64                 # dsts per segment-matmul group
NGROUP = P // GROUP        # 16 groups per core
SENT = 0                   # pad edges gather row 0 (killed via dst_off=64)
NEG_BIG = -1.0e5           # (unused)


def wrap_idx(idx):
    """int array [n] (n % 16 == 0) -> int16 [128, n//16], 16-partition wrap
    replicated across the 8 gpsimd core stripes."""
    idx = np.asarray(idx, np.int16)
    w = idx.reshape(-1, 16).T            # [16, n/16]
    return np.tile(w, (8, 1)).copy()


def prep_edges(edge_index, nbt=None):
    """Shard + sort + block the edge list.

    Returns dict with per-core int16 gather indices (by src and by dst),
    per-edge dst offsets within the 64-dst group (f32), and NBT (max
    blocks per group, compile-time constant shared by all cores).
    """
    src = np.asarray(edge_index[0], np.int64)
    dst = np.asarray(edge_index[1], np.int64)
    loops = np.arange(N, dtype=np.int64)
    src = np.concatenate([src, loops])
    dst = np.concatenate([dst, loops])

    per_core = []
    max_blocks = 0
    for c in range(NCORES):
        m = (dst // P) == c
        s, d = src[m], dst[m] - c * P
        order = np.argsort(d, kind="stable")
        s, d = s[order], d[order]
        g = d // GROUP                     # group id of each edge [0, 16)
        cnt = np.bincount(g, minlength=NGROUP)
        nb = (cnt + 127) // 128            # blocks needed per group
        max_blocks = max(max_blocks, int(nb.max()))
        per_core.append((s, d, cnt))

    if nbt is None:
        nbt = max_blocks
    assert nbt >= max_blocks, (nbt, max_blocks)
    nblk = nbt * NGROUP

    src_idx = np.zeros((NCORES, nblk * 128), np.int64)
    dst_idx = np.zeros((NCORES, nblk * 128), np.int64)
    dst_off = np.full((NCORES, nblk * 128), float(GROUP), np.float32)
    for c in range(NCORES):
        s, d, cnt = per_core[c]
        pos = 0
        for grp in range(NGROUP):
            n = int(cnt[grp])
            base = grp * nbt * 128
            src_idx[c, base:base + n] = s[pos:pos + n]
            dst_idx[c, base:base + n] = d[pos:pos + n] + c * P
            dst_off[c, base:base + n] = (d[pos:pos + n] - grp * GROUP)
            pos += n
    # edge i -> partition i%128, block i//128
    src_w = np.stack([wrap_idx(src_idx[c]) for c in range(NCORES)])
    dst_w = np.stack([wrap_idx(dst_idx[c]) for c in range(NCORES)])
    # dst_off laid out [128, nblk]: partition = i%128, col = block
    off = dst_off.reshape(NCORES, nblk, 128).transpose(0, 2, 1).copy()
    return dict(nbt=nbt, nblk=nblk, src_w=src_w, dst_w=dst_w, dst_off=off)


def expand_att(a):
    """a [HEADS, C] -> block matrix [HEADS*C, HEADS] so that
    (h @ A)[n, head] = sum_c h[n, head, c] * a[head, c]."""
    hh, cc = a.shape
    A = np.zeros((hh * cc, hh), np.float32)
    for h in range(hh):
        A[h * cc:(h + 1) * cc, h] = a[h]
    return A


def prep_weights(W1, a_src1, a_dst1, b1, ln1_g, ln1_b, Wq, Wk, Wv,
                 ln2_g, ln2_b, W2, a_src2, a_dst2, b2):
    """Constant-fold the tiny weights into fused matmul operands."""
    W1 = np.asarray(W1, np.float32)
    W2 = np.asarray(W2, np.float32)
    W1aug = np.concatenate(
        [W1, W1 @ expand_att(np.asarray(a_src1, np.float32)),
         W1 @ expand_att(np.asarray(a_dst1, np.float32))], axis=1)  # [256, 80]
    W2aug = np.concatenate(
        [W2, W2 @ expand_att(np.asarray(a_src2, np.float32)),
         W2 @ expand_att(np.asarray(a_dst2, np.float32))], axis=1)  # [70, 64]
    rep = lambda v: np.tile(np.asarray(v, np.float32)[None, :], (128, 1)).copy()
    return dict(
        W1aug=W1aug, W2aug=W2aug,
        Wq=np.asarray(Wq, np.float32), Wk=np.asarray(Wk, np.float32),
        Wv=np.asarray(Wv, np.float32),
        b1_rep=rep(b1), ln1g_rep=rep(ln1_g), ln1b_rep=rep(ln1_b),
        ln2g_rep=rep(ln2_g), ln2b_rep=rep(ln2_b), b2_rep=rep(b2),
        eye=np.eye(128, dtype=np.float32),
    )




f32 = mybir.dt.float32
f32r = mybir.dt.float32r
bf16 = mybir.dt.bfloat16
i16 = mybir.dt.int16
i32 = mybir.dt.int32
AF = mybir.ActivationFunctionType
OP = mybir.AluOpType

TROWS = N


def bc(ap, shape):
    return ap.broadcast_to(tuple(shape))


class StopPhases(Exception):
    pass


def build_kernel(nbt, debug=False, phases=7):
    nblk = nbt * NGROUP
    CB = 2 * nbt              # blocks per dst-tile chunk (2 groups)
    nc = bacc.Bacc("TRN2", target_bir_lowering=False, debug=False,
                   num_devices=NCORES)

    # ---------------- DRAM I/O ----------------
    xin = nc.dram_tensor("xin", [P, 256], f32, kind="ExternalInput")
    src_it_d = nc.dram_tensor("src_it", [128, nblk * 8], i16, kind="ExternalInput")
    dst_it_d = nc.dram_tensor("dst_it", [128, nblk * 8], i16, kind="ExternalInput")
    dst_off_d = nc.dram_tensor("dst_off", [128, nblk], f32, kind="ExternalInput")
    w1aug_d = nc.dram_tensor("W1aug", [256, 80], f32, kind="ExternalInput")
    w2aug_d = nc.dram_tensor("W2aug", [70, 64], f32, kind="ExternalInput")
    wq_d = nc.dram_tensor("Wq", [64, 64], f32, kind="ExternalInput")
    wk_d = nc.dram_tensor("Wk", [64, 64], f32, kind="ExternalInput")
    wv_d = nc.dram_tensor("Wv", [64, 6], f32, kind="ExternalInput")
    b1_d = nc.dram_tensor("b1_rep", [128, 64], f32, kind="ExternalInput")
    l1g_d = nc.dram_tensor("ln1g_rep", [128, 64], f32, kind="ExternalInput")
    l1b_d = nc.dram_tensor("ln1b_rep", [128, 64], f32, kind="ExternalInput")
    l2g_d = nc.dram_tensor("ln2g_rep", [128, 6], f32, kind="ExternalInput")
    l2b_d = nc.dram_tensor("ln2b_rep", [128, 6], f32, kind="ExternalInput")
    b2_d = nc.dram_tensor("b2_rep", [128, 6], f32, kind="ExternalInput")
    eye_d = nc.dram_tensor("eye", [128, 128], f32, kind="ExternalInput")

    out_d = nc.dram_tensor("out", [P, 6], f32, kind="ExternalOutput")

    t1_loc = nc.dram_tensor("t1_loc", [P, 64], f32)
    t1_full = nc.dram_tensor("t1_full", [TROWS, 64], f32, addr_space="Shared")
    t1_gat = nc.dram_tensor("t1_gat", [TROWS, 64], f32)
    t2_loc = nc.dram_tensor("t2_loc", [P, 64], f32)
    t2_full = nc.dram_tensor("t2_full", [TROWS, 64], f32, addr_space="Shared")
    t2_gat = nc.dram_tensor("t2_gat", [TROWS, 64], f32)
    ht_loc = nc.dram_tensor("ht_loc", [64, P], f32)
    ht_ag = nc.dram_tensor("ht_ag", [64 * NCORES, P], f32, addr_space="Shared")

    dbg = {}
    phase_of = {"dbg_h1a": 0, "dbg_gat1": 1, "dbg_hln": 2, "dbg_ht": 4,
                "dbg_t2": 5}
    if debug:
        for name, shape in [("dbg_h1a", [P, 80]), ("dbg_gat1", [P, 72]),
                            ("dbg_hln", [P, 64]), ("dbg_ht", [P, 6]),
                            ("dbg_t2", [P, 64])]:
            if phases >= phase_of[name]:
                dbg[name] = nc.dram_tensor(name, shape, f32,
                                           kind="ExternalOutput")

    with tile.TileContext(nc) as tc, ExitStack() as top:
      try:
        # ---------------- persistent SBUF ----------------
        pers = top.enter_context(tc.tile_pool(name="pers", bufs=1))

        def ptile(name, shape, dtype):
            return pers.tile(shape, dtype, name=name, tag=name)

        eye = ptile("eye", [128, 128], f32)
        nc.sync.dma_start(eye[:], eye_d.ap()[:])
        iotaf = ptile("iotaf", [128, GROUP], f32)
        ioi = ptile("ioi", [128, GROUP], i32)
        nc.gpsimd.iota(ioi[:], pattern=[[1, GROUP]], base=0, channel_multiplier=0)
        nc.vector.tensor_copy(iotaf[:], ioi[:])

        src_it = ptile("src_it", [128, nblk * 8], i16)
        nc.sync.dma_start(src_it[:], src_it_d.ap()[:])
        dst_it = ptile("dst_it", [128, nblk * 8], i16)
        nc.sync.dma_start(dst_it[:], dst_it_d.ap()[:])
        dst_off = ptile("dst_off", [128, nblk], f32)
        nc.sync.dma_start(dst_off[:], dst_off_d.ap()[:])

        w1aug = ptile("w1aug", [128, 2 * 80], f32)
        nc.sync.dma_start(w1aug[:].rearrange("p (k d) -> p k d", k=2),
                          w1aug_d.ap().rearrange("(k p) d -> p k d", p=128))
        w2top = ptile("w2top", [64, 64], f32)
        nc.sync.dma_start(w2top[:], w2aug_d.ap()[0:64, :])
        w2bot = ptile("w2bot", [6, 64], f32)
        nc.sync.dma_start(w2bot[:], w2aug_d.ap()[64:70, :])
        wq = ptile("wq", [64, 64], f32)
        nc.sync.dma_start(wq[:], wq_d.ap()[:])
        wk = ptile("wk", [64, 64], f32)
        nc.sync.dma_start(wk[:], wk_d.ap()[:])
        wv7 = ptile("wv7", [64, 7], f32)
        nc.gpsimd.memset(wv7[:], 0.0)
        nc.sync.dma_start(wv7[:, 0:6], wv_d.ap()[:])
        b1r = ptile("b1r", [128, 64], f32)
        nc.sync.dma_start(b1r[:], b1_d.ap()[:])
        l1g = ptile("l1g", [128, 64], f32)
        nc.sync.dma_start(l1g[:], l1g_d.ap()[:])
        l1b = ptile("l1b", [128, 64], f32)
        nc.sync.dma_start(l1b[:], l1b_d.ap()[:])
        l2g = ptile("l2g", [128, 6], f32)
        nc.sync.dma_start(l2g[:], l2g_d.ap()[:])
        l2b = ptile("l2b", [128, 6], f32)
        nc.sync.dma_start(l2b[:], l2b_d.ap()[:])
        b2r = ptile("b2r", [128, 6], f32)
        nc.sync.dma_start(b2r[:], b2_d.ap()[:])

        epsc = ptile("epsc", [128, 1], f32)
        nc.gpsimd.memset(epsc[:], LN_EPS)
        sel = ptile("sel", [128, nblk * GROUP], bf16)
        hlnT = ptile("hlnT", [64, N], f32)        # full h_ln^T after AG
        hT_loc_sb = ptile("hT_loc_sb", [64, P], f32)
        gat1 = ptile("gat1", [128, 8 * 72], f32)
        gat2 = ptile("gat2", [128, 8 * 56], f32)
        hln_rows = ptile("hln_rows", [128, 8 * 64], f32)
        htln = ptile("htln", [128, 8 * 6], f32)

        def rows_to_dram(dram, sb_view, ncols, col0=0, cast=None):
            """sb_view [128, 8, w] -> dram rows [(t*128+p), col0:col0+w]."""
            dv = dram.ap()
            if cast is not None:
                dv = dv.bitcast(cast)
            dv = dv.rearrange("(t p) d -> p t d", p=128)
            nc.sync.dma_start(dv[:, :, col0:col0 + ncols], sb_view)

        # ================= P0: x -> T1 =================
        with ExitStack() as ctx:
            pool = ctx.enter_context(tc.tile_pool(name="p0", bufs=3))
            ps = ctx.enter_context(tc.tile_pool(name="p0ps", bufs=3, space="PSUM"))
            h1a = pool.tile([128, 8 * 80], f32, tag="h1a")
            t1h = pool.tile([128, 8 * 64], bf16, tag="t1h")
            for m in range(8):
                xt = pool.tile([128, 256], f32, tag="xt")
                nc.sync.dma_start(xt[:], xin.ap()[m * 128:(m + 1) * 128, :])
                pm = ps.tile([128, 80], f32, tag="pm")
                for k in range(2):
                    pt = ps.tile([128, 128], f32, tag="pt")
                    nc.tensor.transpose(pt[:], xt[:, k * 128:(k + 1) * 128], eye[:])
                    xTk = pool.tile([128, 128], f32, tag="xTk")
                    nc.vector.tensor_copy(xTk[:], pt[:])
                    nc.tensor.matmul(
                        pm[:], xTk[:],
                        w1aug[:].rearrange("p (k d) -> p k d", k=2)[:, k, :],
                        start=(k == 0), stop=(k == 1))
                h1a3 = h1a[:].rearrange("p (t d) -> p t d", t=8)
                nc.scalar.activation(h1a3[:, m, :], pm[:], AF.Copy)
                nc.vector.tensor_copy(
                    t1h[:].rearrange("p (t d) -> p t d", t=8)[:, m, :],
                    pm[:, 0:64])
            h1a3 = h1a[:].rearrange("p (t d) -> p t d", t=8)
            t1h3 = t1h[:].rearrange("p (t d) -> p t d", t=8)
            rows_to_dram(t1_loc, t1h3[:, :, :], 64, col0=0, cast=bf16)
            rows_to_dram(t1_loc, h1a3[:, :, 64:72], 8, col0=32)
            rows_to_dram(t1_loc, h1a3[:, :, 72:80], 8, col0=40)
            if debug:
                rows_to_dram(dbg["dbg_h1a"], h1a3[:, :, :], 80)
            nc.gpsimd.collective_compute(
                "AllGather", OP.bypass, replica_groups=[list(range(NCORES))],
                ins=[t1_loc.ap()[:]], outs=[t1_full.ap()[:]])

        # ============ edge phase (shared by both layers) ============
        def edge_phase(table, it_src, it_dst, hcols, acol, dcol, gatacc, wmsg,
                       build_sel):
            # hcols: # bf16 feature cols; acol/dcol: f32 col of asrc/adst
            with ExitStack() as ctx:
                gp = ctx.enter_context(tc.tile_pool(name="gp", bufs=2))
                sp = ctx.enter_context(tc.tile_pool(name="sp", bufs=2))
                pg = ctx.enter_context(tc.tile_pool(name="pg", bufs=4,
                                                    space="PSUM"))
                for t in range(8):
                    j0 = t * CB
                    gs = gp.tile([128, CB * 64], f32, tag="gs")
                    nc.gpsimd.dma_gather(
                        gs[:].rearrange("p (j e) -> p j e", e=64),
                        table.ap()[:], it_src[:, j0 * 8:(j0 + CB) * 8],
                        num_idxs=CB * 128, num_idxs_reg=CB * 128, elem_size=64,
                        single_packet=False)
                    gd = gp.tile([128, CB * 64], f32, tag="gd")
                    nc.gpsimd.dma_gather(
                        gd[:].rearrange("p (j e) -> p j e", e=64),
                        table.ap()[:], it_dst[:, j0 * 8:(j0 + CB) * 8],
                        num_idxs=CB * 128, num_idxs_reg=CB * 128, elem_size=64,
                        single_packet=False)
                    gs3 = gs[:].rearrange("p (j e) -> p j e", e=64)
                    gd3 = gd[:].rearrange("p (j e) -> p j e", e=64)
                    z = sp.tile([128, CB * 8], f32, tag="z")
                    z3 = z[:].rearrange("p (j e) -> p j e", e=8)
                    nc.vector.tensor_tensor(z3, gs3[:, :, acol:acol + 8],
                                            gd3[:, :, dcol:dcol + 8], OP.add)
                    u = sp.tile([128, CB * 8], f32, tag="u")
                    nc.vector.tensor_scalar_mul(u[:], z[:], 0.2)
                    nc.vector.tensor_max(z[:], z[:], u[:])
                    exf = sp.tile([128, CB * 8], f32, tag="exf")
                    nc.scalar.activation(exf[:], z[:], AF.Exp)
                    exb = sp.tile([128, CB * 8], bf16, tag="exb")
                    nc.vector.tensor_copy(exb[:], exf[:])
                    exb3 = exb[:].rearrange("p (j e) -> p j e", e=8)
                    W = hcols + 8
                    msgs = sp.tile([128, CB * W], bf16, tag="msgs")
                    m3 = msgs[:].rearrange("p (j e) -> p j e", e=W)
                    hb = gs3.bitcast(bf16)  # [128, CB, 128] bf16
                    exb4 = exb3.rearrange("p j (h a) -> p j h a", a=1)
                    nc.vector.tensor_tensor(
                        m3[:, :, 0:hcols].rearrange("p j (h c) -> p j h c", h=8),
                        hb[:, :, 0:hcols].rearrange("p j (h c) -> p j h c", h=8),
                        bc(exb4, (128, CB, 8, hcols // 8)), OP.mult)
                    nc.vector.tensor_copy(m3[:, :, hcols:W], exb3)
                    if build_sel:
                        sel3 = sel[:].rearrange("p (j e) -> p j e", e=GROUP)
                        io_b = bc(iotaf[:].rearrange("p (a e) -> p a e", a=1),
                                  (128, CB, GROUP))
                        do_b = bc(dst_off[:, j0:j0 + CB]
                                  .rearrange("p (j a) -> p j a", a=1),
                                  (128, CB, GROUP))
                        nc.vector.tensor_tensor(sel3[:, j0:j0 + CB, :], io_b,
                                                do_b, OP.is_equal)
                    sel3 = sel[:].rearrange("p (j e) -> p j e", e=GROUP)
                    ga = gatacc[:].rearrange("p (t d) -> p t d", d=W)
                    for g in (0, 1):
                        pgt = pg.tile([64, W], f32, tag="pgt")
                        for b in range(nbt):
                            jj = (2 * t + g) * nbt + b
                            nc.tensor.matmul(
                                pgt[:], sel3[:, jj, :], m3[:, jj - j0, :],
                                start=(b == 0), stop=(b == nbt - 1))
                        nc.scalar.activation(
                            ga[64 * g:64 * (g + 1), t, :], pgt[:], AF.Copy)

        if phases >= 1:
            edge_phase(t1_full, src_it[:], dst_it[:], 64, 32, 40, gat1, 72,
                       build_sel=True)
        if debug and phases >= 1:
            rows_to_dram(dbg["dbg_gat1"],
                         gat1[:].rearrange("p (t d) -> p t d", d=72)[:, :, :], 72)

        # ============ P2: GAT1 -> h_ln ============
        if phases < 2:
            raise StopPhases()
        with ExitStack() as ctx:
            sp = ctx.enter_context(tc.tile_pool(name="p2", bufs=2))
            g3 = gat1[:].rearrange("p (t d) -> p t d", d=72)
            rec = sp.tile([128, 8 * 8], f32, tag="rec")
            nc.vector.reciprocal(rec[:].rearrange("p (t h) -> p t h", h=8),
                                 g3[:, :, 64:72])
            h1 = hln_rows[:].rearrange("p (t d) -> p t d", d=64)
            rec4 = rec[:].rearrange("p (t h a) -> p t h a", t=8, h=8)
            nc.vector.tensor_tensor(
                h1.rearrange("p t (h c) -> p t h c", h=8),
                g3[:, :, 0:64].rearrange("p t (h c) -> p t h c", h=8),
                bc(rec4, (128, 8, 8, 8)), OP.mult)
            b1b = bc(b1r[:].rearrange("(p) (a d) -> p a d", a=1), (128, 8, 64))
            nc.vector.tensor_tensor(h1, h1, b1b, OP.add)
            # layernorm over 64
            rs_ = sp.tile([128, 8], f32, tag="rs_")
            nc.vector.tensor_reduce(rs_[:], h1, mybir.AxisListType.X, OP.add)
            mean = sp.tile([128, 8], f32, tag="mean")
            nc.scalar.mul(mean[:], rs_[:], 1.0 / 64)
            nc.vector.tensor_tensor(
                h1, h1, bc(mean[:].rearrange("p (t a) -> p t a", a=1),
                           (128, 8, 64)), OP.subtract)
            sq = sp.tile([128, 8 * 64], f32, tag="sq")
            ssum = sp.tile([128, 8], f32, tag="ssum")
            sq3 = sq[:].rearrange("p (t d) -> p t d", d=64)
            nc.scalar.activation(sq3, h1, AF.Square)
            nc.vector.tensor_reduce(ssum[:], sq3, mybir.AxisListType.X, OP.add)
            std_ = sp.tile([128, 8], f32, tag="std_")
            nc.scalar.activation(std_[:], ssum[:], AF.Sqrt, bias=epsc[:],
                                 scale=1.0 / 64)
            rstd = sp.tile([128, 8], f32, tag="rstd")
            nc.vector.reciprocal(rstd[:], std_[:])
            nc.vector.tensor_tensor(
                h1, h1, bc(rstd[:].rearrange("p (t a) -> p t a", a=1),
                           (128, 8, 64)), OP.mult)
            nc.vector.tensor_tensor(
                h1, h1, bc(l1g[:].rearrange("p (a d) -> p a d", a=1),
                           (128, 8, 64)), OP.mult)
            nc.vector.tensor_tensor(
                h1, h1, bc(l1b[:].rearrange("p (a d) -> p a d", a=1),
                           (128, 8, 64)), OP.add)
            # elu
            mn = sp.tile([128, 8 * 64], f32, tag="mn")
            nc.vector.tensor_scalar_min(mn[:], hln_rows[:], 0.0)
            ee = sp.tile([128, 8 * 64], f32, tag="ee")
            nc.scalar.activation(ee[:], mn[:], AF.Exp)
            nc.vector.tensor_scalar_max(hln_rows[:], hln_rows[:], 0.0)
            nc.vector.tensor_add(hln_rows[:], hln_rows[:], ee[:])
            nc.vector.tensor_scalar_add(hln_rows[:], hln_rows[:], -1.0)
            if debug:
                rows_to_dram(dbg["dbg_hln"],
                             hln_rows[:].rearrange("p (t d) -> p t d", d=64)
                             [:, :, :], 64)

        # ============ P3: transpose + AG h_ln^T ============
        if phases < 3:
            raise StopPhases()
        with ExitStack() as ctx:
            ps = ctx.enter_context(tc.tile_pool(name="p3ps", bufs=3,
                                                space="PSUM"))
            hr = hln_rows[:].rearrange("p (t d) -> p t d", d=64)
            for m in range(8):
                pt = ps.tile([64, 128], f32, tag="pt")
                nc.tensor.transpose(pt[:], hr[:, m, :], eye[:])
                nc.vector.tensor_copy(hT_loc_sb[:, m * 128:(m + 1) * 128], pt[:])
            nc.sync.dma_start(ht_loc.ap()[:], hT_loc_sb[:])
            nc.gpsimd.collective_compute(
                "AllGather", OP.bypass, replica_groups=[list(range(NCORES))],
                ins=[ht_loc.ap()[:]], outs=[ht_ag.ap()[:]])
            for c in range(NCORES):
                nc.sync.dma_start(hlnT[:, c * P:(c + 1) * P],
                                  ht_ag.ap()[c * 64:(c + 1) * 64, :])

        # ============ P4: attention ============
        if phases < 4:
            raise StopPhases()
        with ExitStack() as ctx:
            pool = ctx.enter_context(tc.tile_pool(name="p4", bufs=2))
            ps = ctx.enter_context(tc.tile_pool(name="p4ps", bufs=2,
                                                space="PSUM"))
            pvps = ctx.enter_context(tc.tile_pool(name="pvps", bufs=1,
                                                  space="PSUM"))
            kT = pers.tile([64, N], f32r, name="kT", tag="kT")
            qT = pers.tile([64, P], f32r, name="qT", tag="qT")
            vaug = pers.tile([128, 64 * 7], bf16, name="vaug", tag="vaug")
            for j in range(16):
                pk = ps.tile([64, 512], f32, tag="pss")
                nc.tensor.matmul(pk[:], wk[:], hlnT[:, j * 512:(j + 1) * 512],
                                 start=True, stop=True)
                nc.vector.tensor_copy(kT[:, j * 512:(j + 1) * 512], pk[:])
            for j in range(2):
                pq = ps.tile([64, 512], f32, tag="pss")
                nc.tensor.matmul(pq[:], wq[:],
                                 hT_loc_sb[:, j * 512:(j + 1) * 512],
                                 start=True, stop=True)
                nc.vector.tensor_copy(qT[:, j * 512:(j + 1) * 512], pq[:])
            va3 = vaug[:].rearrange("p (n d) -> p n d", d=7)
            for nt in range(64):
                pv = ps.tile([128, 7], f32, tag="pss")
                nc.tensor.matmul(pv[:], hlnT[:, nt * 128:(nt + 1) * 128],
                                 wv7[:], start=True, stop=True)
                nc.vector.tensor_copy(va3[:, nt, :], pv[:])
            nc.gpsimd.memset(va3[:, :, 6:7], 1.0)

            NTB = 3  # n-tiles per psum batch (3 banks)
            att = pool.tile([128, 8 * 7], f32, tag="att")
            at3 = att[:].rearrange("p (t d) -> p t d", d=7)
            for mc in range(2):
                po = pvps.tile([7, 512], f32, tag="po")
                nb_list = [(s, min(s + NTB, 64)) for s in range(0, 64, NTB)]
                for (s0, s1) in nb_list:
                    w = (s1 - s0) * 512
                    pss = ps.tile([128, NTB * 512], f32, tag="pss")
                    for i, nt in enumerate(range(s0, s1)):
                        nc.tensor.matmul(
                            pss[:, i * 512:(i + 1) * 512],
                            kT[:, nt * 128:(nt + 1) * 128],
                            qT[:, mc * 512:(mc + 1) * 512],
                            start=True, stop=True)
                    pT = pool.tile([128, NTB * 512], bf16, tag="pT")
                    nc.scalar.activation(pT[:, 0:w], pss[:, 0:w], AF.Exp,
                                         scale=0.125)
                    for i, nt in enumerate(range(s0, s1)):
                        nc.tensor.matmul(
                            po[:], va3[:, nt, :].bitcast(bf16),
                            pT[:, i * 512:(i + 1) * 512],
                            start=(nt == 0), stop=(nt == 63),
                            skip_group_check=True)
                spo = pool.tile([7, 512], f32, tag="spo")
                nc.vector.tensor_copy(spo[:], po[:])
                for i in range(4):
                    ptr = ps.tile([128, 7], f32, tag="pss")
                    nc.tensor.transpose(ptr[:], spo[:, i * 128:(i + 1) * 128],
                                        eye[0:7, 0:7])
                    nc.vector.tensor_copy(at3[:, mc * 4 + i, :], ptr[:])
            # normalize + LN over 6
            rec = pool.tile([128, 8], f32, tag="reca")
            nc.vector.reciprocal(rec[:].rearrange("p (t a) -> p t a", a=1),
                                 at3[:, :, 6:7])
            ht3 = htln[:].rearrange("p (t d) -> p t d", d=6)
            nc.vector.tensor_tensor(
                ht3, at3[:, :, 0:6],
                bc(rec[:].rearrange("p (t a) -> p t a", a=1), (128, 8, 6)),
                OP.mult)
            rs_ = pool.tile([128, 8], f32, tag="rsb")
            nc.vector.tensor_reduce(rs_[:], ht3, mybir.AxisListType.X, OP.add)
            mean = pool.tile([128, 8], f32, tag="meanb")
            nc.scalar.mul(mean[:], rs_[:], 1.0 / 6)
            nc.vector.tensor_tensor(
                ht3, ht3, bc(mean[:].rearrange("p (t a) -> p t a", a=1),
                             (128, 8, 6)), OP.subtract)
            sq = pool.tile([128, 8 * 6], f32, tag="sqb")
            ssum = pool.tile([128, 8], f32, tag="ssumb")
            sq3b = sq[:].rearrange("p (t d) -> p t d", d=6)
            nc.scalar.activation(sq3b, ht3, AF.Square)
            nc.vector.tensor_reduce(ssum[:], sq3b, mybir.AxisListType.X, OP.add)
            stdb = pool.tile([128, 8], f32, tag="stdb")
            nc.scalar.activation(stdb[:], ssum[:], AF.Sqrt, bias=epsc[:],
                                 scale=1.0 / 6)
            rstd = pool.tile([128, 8], f32, tag="rstdb")
            nc.vector.reciprocal(rstd[:], stdb[:])
            nc.vector.tensor_tensor(
                ht3, ht3, bc(rstd[:].rearrange("p (t a) -> p t a", a=1),
                             (128, 8, 6)), OP.mult)
            nc.vector.tensor_tensor(
                ht3, ht3, bc(l2g[:].rearrange("p (a d) -> p a d", a=1),
                             (128, 8, 6)), OP.mult)
            nc.vector.tensor_tensor(
                ht3, ht3, bc(l2b[:].rearrange("p (a d) -> p a d", a=1),
                             (128, 8, 6)), OP.add)
            if debug:
                rows_to_dram(dbg["dbg_ht"], ht3[:, :, :], 6)

        # ============ P5: T2 build + AG ============
        if phases < 5:
            raise StopPhases()
        with ExitStack() as ctx:
            pool = ctx.enter_context(tc.tile_pool(name="p5", bufs=3))
            ps = ctx.enter_context(tc.tile_pool(name="p5ps", bufs=3,
                                                space="PSUM"))
            htT = pool.tile([6, P], f32, tag="htT")
            ht3 = htln[:].rearrange("p (t d) -> p t d", d=6)
            for m in range(8):
                pt = ps.tile([6, 128], f32, tag="pt2")
                nc.tensor.transpose(pt[:], ht3[:, m, :], eye[:])
                nc.vector.tensor_copy(htT[:, m * 128:(m + 1) * 128], pt[:])
            h2a = pool.tile([128, 8 * 64], f32, tag="h2a")
            h2b = pool.tile([128, 8 * 48], bf16, tag="h2b")
            h2a3 = h2a[:].rearrange("p (t d) -> p t d", d=64)
            h2b3 = h2b[:].rearrange("p (t d) -> p t d", d=48)
            for m in range(8):
                pm = ps.tile([128, 64], f32, tag="pm2")
                nc.tensor.matmul(pm[:], hT_loc_sb[:, m * 128:(m + 1) * 128],
                                 w2top[:], start=True, stop=False)
                nc.tensor.matmul(pm[:], htT[:, m * 128:(m + 1) * 128],
                                 w2bot[:], start=False, stop=True)
                nc.scalar.activation(h2a3[:, m, :], pm[:], AF.Copy)
                nc.vector.tensor_copy(h2b3[:, m, :], pm[:, 0:48])
            rows_to_dram(t2_loc, h2b3[:, :, :], 48, col0=0, cast=bf16)
            rows_to_dram(t2_loc, h2a3[:, :, 48:56], 8, col0=24)
            rows_to_dram(t2_loc, h2a3[:, :, 56:64], 8, col0=32)
            if debug:
                rows_to_dram(dbg["dbg_t2"], h2a3[:, :, :], 64)
            nc.gpsimd.collective_compute(
                "AllGather", OP.bypass, replica_groups=[list(range(NCORES))],
                ins=[t2_loc.ap()[:]], outs=[t2_full.ap()[:]])

        # ============ P6: GAT2 edge phase ============
        if phases < 6:
            raise StopPhases()
        edge_phase(t2_full, src_it[:], dst_it[:], 48, 24, 32, gat2, 56,
                   build_sel=False)

        # ============ P7: finale ============
        if phases < 7:
            raise StopPhases()
        with ExitStack() as ctx:
            sp = ctx.enter_context(tc.tile_pool(name="p7", bufs=2))
            g3 = gat2[:].rearrange("p (t d) -> p t d", d=56)
            d8 = sp.tile([128, 8 * 8], f32, tag="d8")
            nc.vector.tensor_scalar_mul(d8[:].rearrange("p (t h) -> p t h", h=8),
                                        g3[:, :, 48:56], 8.0)
            rec = sp.tile([128, 8 * 8], f32, tag="rec2")
            nc.vector.reciprocal(rec[:], d8[:])
            avg = sp.tile([128, 8 * 48], f32, tag="avg")
            a4 = avg[:].rearrange("p (t h c) -> p t h c", t=8, h=8)
            rec4 = rec[:].rearrange("p (t h a) -> p t h a", t=8, h=8)
            nc.vector.tensor_tensor(
                a4, g3[:, :, 0:48].rearrange("p t (h c) -> p t h c", h=8),
                bc(rec4, (128, 8, 8, 6)), OP.mult)
            swp = sp.tile([128, 8 * 48], f32, tag="swp")
            s4 = swp[:].rearrange("p (t c h) -> p t c h", t=8, c=6)
            nc.vector.tensor_copy(
                s4, avg[:].rearrange("p (t h c) -> p t h c", t=8, h=8)
                .rearrange("p t h c -> p t c h"))
            out2 = sp.tile([128, 8 * 6], f32, tag="out2")
            o3 = out2[:].rearrange("p (t d) -> p t d", d=6)
            nc.vector.tensor_reduce(o3, s4, mybir.AxisListType.X, OP.add)
            nc.vector.tensor_tensor(
                o3, o3, bc(b2r[:].rearrange("p (a d) -> p a d", a=1),
                           (128, 8, 6)), OP.add)
            ex = sp.tile([128, 8 * 6], f32, tag="exo")
            es = sp.tile([128, 8], f32, tag="eso")
            ex3 = ex[:].rearrange("p (t d) -> p t d", d=6)
            nc.scalar.activation(ex3, o3, AF.Exp)
            nc.vector.tensor_reduce(es[:], ex3, mybir.AxisListType.X, OP.add)
            ls = sp.tile([128, 8], f32, tag="lso")
            nc.scalar.activation(ls[:], es[:], AF.Ln)
            nc.vector.tensor_tensor(
                o3, o3, bc(ls[:].rearrange("p (t a) -> p t a", a=1),
                           (128, 8, 6)), OP.subtract)
            rows_to_dram(out_d, o3[:, :, :], 6)

      except StopPhases:
        with tc.tile_pool(name="zop", bufs=1) as zp:
            zo = zp.tile([128, 8 * 6], f32, tag="zo")
            nc.gpsimd.memset(zo[:], 0.0)
            dv = out_d.ap().rearrange("(t p) d -> p t d", p=128)
            nc.sync.dma_start(dv, zo[:].rearrange("p (t d) -> p t d", d=6))
    nc.compile()
    return nc


_CACHE = {}


def kernel(**inputs):
    x = np.ascontiguousarray(np.asarray(inputs["x"], np.float32))
    ep = prep_edges(np.asarray(inputs["edge_index"]))
    wp = prep_weights(
        inputs["W1"], inputs["a_src1"], inputs["a_dst1"], inputs["b1"],
        inputs["ln1_g"], inputs["ln1_b"], inputs["Wq"], inputs["Wk"],
        inputs["Wv"], inputs["ln2_g"], inputs["ln2_b"], inputs["W2"],
        inputs["a_src2"], inputs["a_dst2"], inputs["b2"])
    nbt = max(18, (ep["nbt"] + 1) // 2 * 2)
    ep = prep_edges(np.asarray(inputs["edge_index"]), nbt=nbt)

    if nbt not in _CACHE:
        _CACHE[nbt] = build_kernel(nbt)
    nc = _CACHE[nbt]

    shared = dict(
        W1aug=wp["W1aug"], W2aug=wp["W2aug"], Wq=wp["Wq"], Wk=wp["Wk"],
        Wv=wp["Wv"], b1_rep=wp["b1_rep"], ln1g_rep=wp["ln1g_rep"],
        ln1b_rep=wp["ln1b_rep"], ln2g_rep=wp["ln2g_rep"],
        ln2b_rep=wp["ln2b_rep"], b2_rep=wp["b2_rep"], eye=wp["eye"])
    in_maps = []
    for c in range(NCORES):
        m = dict(shared)
        m["xin"] = x[c * P:(c + 1) * P]
        m["src_it"] = ep["src_w"][c]
        m["dst_it"] = ep["dst_w"][c]
        m["dst_off"] = ep["dst_off"][c]
        in_maps.append(m)

    last_err = None
    for attempt in range(3):
        try:
            res = run_bass_kernel_spmd(nc, in_maps,
                                       core_ids=list(range(NCORES)))
            out = np.concatenate(
                [res.results[c]["out"] for c in range(NCORES)], axis=0)
            if np.isfinite(out).all():
                return out
            last_err = RuntimeError("non-finite output")
        except Exception as e:  # transient NRT/axon failures
            last_err = e
            import time as _time
            _time.sleep(15)
    raise last_err



# revision 4
# speedup vs baseline: 6.1670x; 6.1670x over previous
"""Self-contained Trainium2 Bass kernel for nn_GAT_transformer.

kernel(**inputs) -> np.ndarray [8192, 6] (log_softmax output).

Strategy: 8-core SPMD, nodes (and incident edges grouped by dst) sharded
across cores. GAT message passing uses dma_gather from an AllGathered
per-node table, segment softmax without max subtraction, and per-128-edge
one-hot selector matmuls with a fused denominator column. The dense NxN
attention runs keys-on-partitions with row-sharded Q and AllGathered K/V.

The dispatch path is latency/bandwidth-bound over the remote PJRT tunnel,
so the host->device contract is minimized: x ships pre-transposed in
bf16, the gather index tables ship in their compact 16-row wrap form and
are replicated across gpsimd stripes on device, dst offsets ship as
uint8, and the small weights ship sharded (1/8 per core) and are
AllGathered + unpacked on device. The jitted executable is cached across
calls.
"""
import time
from contextlib import ExitStack
from functools import partial

import numpy as np
import ml_dtypes

import jax
import jax.numpy as jnp
from jax.sharding import Mesh, PartitionSpec, NamedSharding
from jax.experimental.shard_map import shard_map

import concourse.bacc as bacc
import concourse.tile as tile
from concourse import mybir, bass2jax

N = 8192
E = 262144
D_IN = 256
HEADS = 8
HID = 8
D_OUT = 6
S_MAX = 64
NEG_SLOPE = 0.2
LN_EPS = 1e-5
NCORES = 8
P = N // NCORES            # 1024 nodes per core
GROUP = 64                 # dsts per segment-matmul group
NGROUP = P // GROUP        # 16 groups per core

# weight blob layout (f32 elements, flat)
O_W1 = 0                   # W1aug [256, 80]
O_W2T = 20480              # W2aug rows 0:64  [64, 64]
O_W2B = 24576              # W2aug rows 64:70 [6, 64]
O_WQ = 24960               # Wq [64, 64]
O_WK = 29056               # Wk [64, 64]
O_WV = 33152               # Wv [64, 6]
O_VEC = 33536              # b1(64) ln1g(64) ln1b(64) ln2g(6) ln2b(6) b2(6)
W_TOT = 33746
WSH = 4224                 # per-core blob width (8 * 4224 = 33792 >= W_TOT)

BF16 = ml_dtypes.bfloat16


def wrap_idx(idx):
    """int array [n] (n % 16 == 0) -> int16 [16, n//16] wrap (compact form;
    replicated to the 8 gpsimd stripes on device)."""
    idx = np.asarray(idx, np.int16)
    return idx.reshape(-1, 16).T.copy()


def prep_edges(edge_index):
    """Shard + sort + block the edge list.

    Returns (idx [NCORES, 32, nblk*8] i16   (rows 0:16 src, 16:32 dst),
             off [NCORES, 128, nblk] u8, nbt)."""
    src = np.asarray(edge_index[0], np.int64)
    dst = np.asarray(edge_index[1], np.int64)
    loops = np.arange(N, dtype=np.int64)
    src = np.concatenate([src, loops])
    dst = np.concatenate([dst, loops])

    per_core = []
    max_blocks = 0
    for c in range(NCORES):
        m = (dst // P) == c
        s, d = src[m], dst[m] - c * P
        order = np.argsort(d, kind="stable")
        s, d = s[order], d[order]
        g = d // GROUP
        cnt = np.bincount(g, minlength=NGROUP)
        nb = (cnt + 127) // 128
        max_blocks = max(max_blocks, int(nb.max()))
        per_core.append((s, d, cnt))

    nbt = max(18, (max_blocks + 1) // 2 * 2)
    nblk = nbt * NGROUP

    src_idx = np.zeros((NCORES, nblk * 128), np.int64)
    dst_idx = np.zeros((NCORES, nblk * 128), np.int64)
    dst_off = np.full((NCORES, nblk * 128), GROUP, np.uint8)
    for c in range(NCORES):
        s, d, cnt = per_core[c]
        pos = 0
        for grp in range(NGROUP):
            n = int(cnt[grp])
            base = grp * nbt * 128
            src_idx[c, base:base + n] = s[pos:pos + n]
            dst_idx[c, base:base + n] = d[pos:pos + n] + c * P
            dst_off[c, base:base + n] = (d[pos:pos + n] - grp * GROUP)
            pos += n
    idx = np.stack([
        np.concatenate([wrap_idx(src_idx[c]), wrap_idx(dst_idx[c])], axis=0)
        for c in range(NCORES)])
    off = dst_off.reshape(NCORES, nblk, 128).transpose(0, 2, 1).copy()
    return idx, off, nbt


def expand_att(a):
    """a [HEADS, C] -> block matrix [HEADS*C, HEADS] so that
    (h @ A)[n, head] = sum_c h[n, head, c] * a[head, c]."""
    hh, cc = a.shape
    A = np.zeros((hh * cc, hh), np.float32)
    for h in range(hh):
        A[h * cc:(h + 1) * cc, h] = a[h]
    return A


def prep_weights(W1, a_src1, a_dst1, b1, ln1_g, ln1_b, Wq, Wk, Wv,
                 ln2_g, ln2_b, W2, a_src2, a_dst2, b2):
    """Constant-fold the tiny weights into one flat blob [NCORES, WSH]."""
    W1 = np.asarray(W1, np.float32)
    W2 = np.asarray(W2, np.float32)
    W1aug = np.concatenate(
        [W1, W1 @ expand_att(np.asarray(a_src1, np.float32)),
         W1 @ expand_att(np.asarray(a_dst1, np.float32))], axis=1)  # [256, 80]
    W2aug = np.concatenate(
        [W2, W2 @ expand_att(np.asarray(a_src2, np.float32)),
         W2 @ expand_att(np.asarray(a_dst2, np.float32))], axis=1)  # [70, 64]
    blob = np.zeros(NCORES * WSH, np.float32)
    # W1aug stored so that flat[(k*128+p)*80+d] = W1aug[k*128+p, d]
    blob[O_W1:O_W1 + 20480] = W1aug.reshape(-1)
    blob[O_W2T:O_W2T + 4096] = W2aug[0:64].reshape(-1)
    blob[O_W2B:O_W2B + 384] = W2aug[64:70].reshape(-1)
    blob[O_WQ:O_WQ + 4096] = np.asarray(Wq, np.float32).reshape(-1)
    blob[O_WK:O_WK + 4096] = np.asarray(Wk, np.float32).reshape(-1)
    blob[O_WV:O_WV + 384] = np.asarray(Wv, np.float32).reshape(-1)
    vec = np.concatenate([
        np.asarray(b1, np.float32).reshape(-1),
        np.asarray(ln1_g, np.float32).reshape(-1),
        np.asarray(ln1_b, np.float32).reshape(-1),
        np.asarray(ln2_g, np.float32).reshape(-1),
        np.asarray(ln2_b, np.float32).reshape(-1),
        np.asarray(b2, np.float32).reshape(-1)])
    blob[O_VEC:O_VEC + 210] = vec
    return blob.reshape(NCORES, WSH)


f32 = mybir.dt.float32
f32r = mybir.dt.float32r
bf16 = mybir.dt.bfloat16
i16 = mybir.dt.int16
i32 = mybir.dt.int32
u8 = mybir.dt.uint8
AF = mybir.ActivationFunctionType
OP = mybir.AluOpType


def bc(ap, shape):
    return ap.broadcast_to(tuple(shape))


def build_kernel(nbt):
    nblk = nbt * NGROUP
    CB = 2 * nbt              # blocks per dst-tile chunk (2 groups)
    nc = bacc.Bacc("TRN2", target_bir_lowering=False, debug=False,
                   num_devices=NCORES)

    # ---------------- DRAM I/O ----------------
    xin = nc.dram_tensor("xin", [256, P], bf16, kind="ExternalInput")
    idx_d = nc.dram_tensor("idx", [32, nblk * 8], i16, kind="ExternalInput")
    off_d = nc.dram_tensor("offu8", [128, nblk], u8, kind="ExternalInput")
    wblob_d = nc.dram_tensor("wblob", [1, WSH], f32, kind="ExternalInput")

    out_d = nc.dram_tensor("out", [P, 6], f32, kind="ExternalOutput")

    wfull_d = nc.dram_tensor("wfull", [NCORES, WSH], f32, addr_space="Shared")
    t1_loc = nc.dram_tensor("t1_loc", [P, 64], f32)
    t1_full = nc.dram_tensor("t1_full", [N, 64], f32, addr_space="Shared")
    t2_loc = nc.dram_tensor("t2_loc", [P, 64], f32)
    t2_full = nc.dram_tensor("t2_full", [N, 64], f32, addr_space="Shared")
    ht_loc = nc.dram_tensor("ht_loc", [64, P], f32)
    ht_ag = nc.dram_tensor("ht_ag", [64 * NCORES, P], f32, addr_space="Shared")

    with tile.TileContext(nc) as tc, ExitStack() as top:
        # ---------------- persistent SBUF ----------------
        pers = top.enter_context(tc.tile_pool(name="pers", bufs=1))

        def ptile(name, shape, dtype):
            return pers.tile(shape, dtype, name=name, tag=name)

        # weights: AllGather the sharded blob, then unpack.
        # (collectives cannot read IO tensors -> bounce via internal DRAM)
        wstage_d = nc.dram_tensor("wstage", [1, WSH], f32)
        nc.sync.dma_start(wstage_d.ap()[:], wblob_d.ap()[:])
        nc.gpsimd.collective_compute(
            "AllGather", OP.bypass, replica_groups=[list(range(NCORES))],
            ins=[wstage_d.ap()[:]], outs=[wfull_d.ap()[:]])
        flat = wfull_d.ap().rearrange("a b -> (a b)")

        w1f = ptile("w1f", [128, 160], f32)
        nc.sync.dma_start(
            w1f[:].rearrange("p (k d) -> p k d", k=2),
            flat[O_W1:O_W1 + 20480].rearrange("(k p d) -> p k d", p=128, d=80))
        w1aug = ptile("w1aug", [128, 2 * 80], bf16)
        nc.vector.tensor_copy(w1aug[:], w1f[:])
        w2top = ptile("w2top", [64, 64], f32)
        nc.sync.dma_start(w2top[:],
                          flat[O_W2T:O_W2T + 4096].rearrange("(p d) -> p d", d=64))
        w2bot = ptile("w2bot", [6, 64], f32)
        nc.sync.dma_start(w2bot[:],
                          flat[O_W2B:O_W2B + 384].rearrange("(p d) -> p d", d=64))
        wq = ptile("wq", [64, 64], f32)
        nc.sync.dma_start(wq[:],
                          flat[O_WQ:O_WQ + 4096].rearrange("(p d) -> p d", d=64))
        wk = ptile("wk", [64, 64], f32)
        nc.sync.dma_start(wk[:],
                          flat[O_WK:O_WK + 4096].rearrange("(p d) -> p d", d=64))
        wv7 = ptile("wv7", [64, 7], f32)
        nc.gpsimd.memset(wv7[:], 0.0)
        nc.sync.dma_start(wv7[:, 0:6],
                          flat[O_WV:O_WV + 384].rearrange("(p d) -> p d", d=6))
        vec210 = ptile("vec210", [1, 210], f32)
        nc.sync.dma_start(vec210[:],
                          flat[O_VEC:O_VEC + 210].rearrange("(a b) -> a b", a=1))
        ones1 = ptile("ones1", [1, 128], f32)
        nc.gpsimd.memset(ones1[:], 1.0)
        rep210 = ptile("rep210", [128, 210], f32)
        with tc.tile_pool(name="sps", bufs=1, space="PSUM") as sps:
            rp = sps.tile([128, 210], f32, tag="rp")
            nc.tensor.matmul(rp[:], ones1[:], vec210[:], start=True, stop=True)
            nc.scalar.activation(rep210[:], rp[:], AF.Copy)
        b1r = rep210[:][:, 0:64]
        l1g = rep210[:][:, 64:128]
        l1b = rep210[:][:, 128:192]
        l2g = rep210[:][:, 192:198]
        l2b = rep210[:][:, 198:204]
        b2r = rep210[:][:, 204:210]

        # identity matrix via iota + is_equal
        coli = ptile("coli", [128, 128], i32)
        nc.gpsimd.iota(coli[:], pattern=[[1, 128]], base=0, channel_multiplier=0)
        rowi = ptile("rowi", [128, 1], i32)
        nc.gpsimd.iota(rowi[:], pattern=[[1, 1]], base=0, channel_multiplier=1)
        colf = ptile("colf", [128, 128], f32)
        nc.vector.tensor_copy(colf[:], coli[:])
        rowf = ptile("rowf", [128, 1], f32)
        nc.vector.tensor_copy(rowf[:], rowi[:])
        eye = ptile("eye", [128, 128], f32)
        nc.vector.tensor_tensor(eye[:], colf[:], bc(rowf[:], (128, 128)),
                                OP.is_equal)

        iotaf = ptile("iotaf", [128, GROUP], f32)
        ioi = ptile("ioi", [128, GROUP], i32)
        nc.gpsimd.iota(ioi[:], pattern=[[1, GROUP]], base=0, channel_multiplier=0)
        nc.vector.tensor_copy(iotaf[:], ioi[:])

        # gather index tables: replicate compact 16-row wraps to 8 stripes
        src_it = ptile("src_it", [128, nblk * 8], i16)
        dst_it = ptile("dst_it", [128, nblk * 8], i16)
        for g in range(8):
            nc.sync.dma_start(src_it[:][16 * g:16 * (g + 1), :],
                              idx_d.ap()[0:16, :])
            nc.sync.dma_start(dst_it[:][16 * g:16 * (g + 1), :],
                              idx_d.ap()[16:32, :])
        offu = ptile("offu", [128, nblk], u8)
        nc.sync.dma_start(offu[:], off_d.ap()[:])
        dst_off = ptile("dst_off", [128, nblk], f32)
        nc.vector.tensor_copy(dst_off[:], offu[:])

        epsc = ptile("epsc", [128, 1], f32)
        nc.gpsimd.memset(epsc[:], LN_EPS)
        sel = ptile("sel", [128, nblk * GROUP], bf16)
        hlnT = ptile("hlnT", [64, N], f32)        # full h_ln^T after AG
        hT_loc_sb = ptile("hT_loc_sb", [64, P], f32)
        gat1 = ptile("gat1", [128, 8 * 72], f32)
        gat2 = ptile("gat2", [128, 8 * 56], f32)
        hln_rows = ptile("hln_rows", [128, 8 * 64], f32)
        htln = ptile("htln", [128, 8 * 6], f32)

        def rows_to_dram(dram, sb_view, ncols, col0=0, cast=None):
            """sb_view [128, 8, w] -> dram rows [(t*128+p), col0:col0+w]."""
            dv = dram.ap()
            if cast is not None:
                dv = dv.bitcast(cast)
            dv = dv.rearrange("(t p) d -> p t d", p=128)
            nc.sync.dma_start(dv[:, :, col0:col0 + ncols], sb_view)

        # ================= P0: x -> T1 =================
        with ExitStack() as ctx:
            pool = ctx.enter_context(tc.tile_pool(name="p0", bufs=3))
            ps = ctx.enter_context(tc.tile_pool(name="p0ps", bufs=3, space="PSUM"))
            xsb = pool.tile([128, 2 * P], bf16, tag="xsb")
            nc.sync.dma_start(xsb[:].rearrange("p (k n) -> p k n", k=2),
                              xin.ap().rearrange("(k p) n -> p k n", p=128))
            xv = xsb[:].rearrange("p (k n) -> p k n", k=2)
            wv_ = w1aug[:].rearrange("p (k d) -> p k d", k=2)
            h1a = pool.tile([128, 8 * 80], f32, tag="h1a")
            t1h = pool.tile([128, 8 * 64], bf16, tag="t1h")
            for m in range(8):
                pm = ps.tile([128, 80], f32, tag="pm")
                for k in range(2):
                    nc.tensor.matmul(
                        pm[:], xv[:, k, m * 128:(m + 1) * 128], wv_[:, k, :],
                        start=(k == 0), stop=(k == 1))
                h1a3 = h1a[:].rearrange("p (t d) -> p t d", t=8)
                nc.scalar.activation(h1a3[:, m, :], pm[:], AF.Copy)
                nc.vector.tensor_copy(
                    t1h[:].rearrange("p (t d) -> p t d", t=8)[:, m, :],
                    pm[:, 0:64])
            h1a3 = h1a[:].rearrange("p (t d) -> p t d", t=8)
            t1h3 = t1h[:].rearrange("p (t d) -> p t d", t=8)
            rows_to_dram(t1_loc, t1h3[:, :, :], 64, col0=0, cast=bf16)
            rows_to_dram(t1_loc, h1a3[:, :, 64:72], 8, col0=32)
            rows_to_dram(t1_loc, h1a3[:, :, 72:80], 8, col0=40)
            nc.gpsimd.collective_compute(
                "AllGather", OP.bypass, replica_groups=[list(range(NCORES))],
                ins=[t1_loc.ap()[:]], outs=[t1_full.ap()[:]])

        # ============ edge phase (shared by both layers) ============
        def edge_phase(table, it_src, it_dst, hcols, acol, dcol, gatacc,
                       build_sel):
            # hcols: # bf16 feature cols; acol/dcol: f32 col of asrc/adst
            with ExitStack() as ctx:
                gp = ctx.enter_context(tc.tile_pool(name="gp", bufs=2))
                sp = ctx.enter_context(tc.tile_pool(name="sp", bufs=2))
                pg = ctx.enter_context(tc.tile_pool(name="pg", bufs=4,
                                                    space="PSUM"))
                for t in range(8):
                    j0 = t * CB
                    gs = gp.tile([128, CB * 64], f32, tag="gs")
                    nc.gpsimd.dma_gather(
                        gs[:].rearrange("p (j e) -> p j e", e=64),
                        table.ap()[:], it_src[:, j0 * 8:(j0 + CB) * 8],
                        num_idxs=CB * 128, num_idxs_reg=CB * 128, elem_size=64,
                        single_packet=False)
                    gd = gp.tile([128, CB * 64], f32, tag="gd")
                    nc.gpsimd.dma_gather(
                        gd[:].rearrange("p (j e) -> p j e", e=64),
                        table.ap()[:], it_dst[:, j0 * 8:(j0 + CB) * 8],
                        num_idxs=CB * 128, num_idxs_reg=CB * 128, elem_size=64,
                        single_packet=False)
                    gs3 = gs[:].rearrange("p (j e) -> p j e", e=64)
                    gd3 = gd[:].rearrange("p (j e) -> p j e", e=64)
                    z = sp.tile([128, CB * 8], f32, tag="z")
                    z3 = z[:].rearrange("p (j e) -> p j e", e=8)
                    nc.vector.tensor_tensor(z3, gs3[:, :, acol:acol + 8],
                                            gd3[:, :, dcol:dcol + 8], OP.add)
                    u = sp.tile([128, CB * 8], f32, tag="u")
                    nc.vector.tensor_scalar_mul(u[:], z[:], 0.2)
                    nc.vector.tensor_max(z[:], z[:], u[:])
                    exf = sp.tile([128, CB * 8], f32, tag="exf")
                    nc.scalar.activation(exf[:], z[:], AF.Exp)
                    exb = sp.tile([128, CB * 8], bf16, tag="exb")
                    nc.vector.tensor_copy(exb[:], exf[:])
                    exb3 = exb[:].rearrange("p (j e) -> p j e", e=8)
                    W = hcols + 8
                    msgs = sp.tile([128, CB * W], bf16, tag="msgs")
                    m3 = msgs[:].rearrange("p (j e) -> p j e", e=W)
                    hb = gs3.bitcast(bf16)  # [128, CB, 128] bf16
                    exb4 = exb3.rearrange("p j (h a) -> p j h a", a=1)
                    nc.vector.tensor_tensor(
                        m3[:, :, 0:hcols].rearrange("p j (h c) -> p j h c", h=8),
                        hb[:, :, 0:hcols].rearrange("p j (h c) -> p j h c", h=8),
                        bc(exb4, (128, CB, 8, hcols // 8)), OP.mult)
                    nc.vector.tensor_copy(m3[:, :, hcols:W], exb3)
                    if build_sel:
                        sel3 = sel[:].rearrange("p (j e) -> p j e", e=GROUP)
                        io_b = bc(iotaf[:].rearrange("p (a e) -> p a e", a=1),
                                  (128, CB, GROUP))
                        do_b = bc(dst_off[:, j0:j0 + CB]
                                  .rearrange("p (j a) -> p j a", a=1),
                                  (128, CB, GROUP))
                        nc.vector.tensor_tensor(sel3[:, j0:j0 + CB, :], io_b,
                                                do_b, OP.is_equal)
                    sel3 = sel[:].rearrange("p (j e) -> p j e", e=GROUP)
                    ga = gatacc[:].rearrange("p (t d) -> p t d", d=W)
                    for g in (0, 1):
                        pgt = pg.tile([64, W], f32, tag="pgt")
                        for b in range(nbt):
                            jj = (2 * t + g) * nbt + b
                            nc.tensor.matmul(
                                pgt[:], sel3[:, jj, :], m3[:, jj - j0, :],
                                start=(b == 0), stop=(b == nbt - 1))
                        nc.scalar.activation(
                            ga[64 * g:64 * (g + 1), t, :], pgt[:], AF.Copy)

        edge_phase(t1_full, src_it[:], dst_it[:], 64, 32, 40, gat1,
                   build_sel=True)

        # ============ P2: GAT1 -> h_ln ============
        with ExitStack() as ctx:
            sp = ctx.enter_context(tc.tile_pool(name="p2", bufs=2))
            g3 = gat1[:].rearrange("p (t d) -> p t d", d=72)
            rec = sp.tile([128, 8 * 8], f32, tag="rec")
            nc.vector.reciprocal(rec[:].rearrange("p (t h) -> p t h", h=8),
                                 g3[:, :, 64:72])
            h1 = hln_rows[:].rearrange("p (t d) -> p t d", d=64)
            rec4 = rec[:].rearrange("p (t h a) -> p t h a", t=8, h=8)
            nc.vector.tensor_tensor(
                h1.rearrange("p t (h c) -> p t h c", h=8),
                g3[:, :, 0:64].rearrange("p t (h c) -> p t h c", h=8),
                bc(rec4, (128, 8, 8, 8)), OP.mult)
            b1b = bc(b1r.rearrange("p (a d) -> p a d", a=1), (128, 8, 64))
            nc.vector.tensor_tensor(h1, h1, b1b, OP.add)
            # layernorm over 64
            rs_ = sp.tile([128, 8], f32, tag="rs_")
            nc.vector.tensor_reduce(rs_[:], h1, mybir.AxisListType.X, OP.add)
            mean = sp.tile([128, 8], f32, tag="mean")
            nc.scalar.mul(mean[:], rs_[:], 1.0 / 64)
            nc.vector.tensor_tensor(
                h1, h1, bc(mean[:].rearrange("p (t a) -> p t a", a=1),
                           (128, 8, 64)), OP.subtract)
            sq = sp.tile([128, 8 * 64], f32, tag="sq")
            ssum = sp.tile([128, 8], f32, tag="ssum")
            sq3 = sq[:].rearrange("p (t d) -> p t d", d=64)
            nc.scalar.activation(sq3, h1, AF.Square)
            nc.vector.tensor_reduce(ssum[:], sq3, mybir.AxisListType.X, OP.add)
            std_ = sp.tile([128, 8], f32, tag="std_")
            nc.scalar.activation(std_[:], ssum[:], AF.Sqrt, bias=epsc[:],
                                 scale=1.0 / 64)
            rstd = sp.tile([128, 8], f32, tag="rstd")
            nc.vector.reciprocal(rstd[:], std_[:])
            nc.vector.tensor_tensor(
                h1, h1, bc(rstd[:].rearrange("p (t a) -> p t a", a=1),
                           (128, 8, 64)), OP.mult)
            nc.vector.tensor_tensor(
                h1, h1, bc(l1g.rearrange("p (a d) -> p a d", a=1),
                           (128, 8, 64)), OP.mult)
            nc.vector.tensor_tensor(
                h1, h1, bc(l1b.rearrange("p (a d) -> p a d", a=1),
                           (128, 8, 64)), OP.add)
            # elu
            mn = sp.tile([128, 8 * 64], f32, tag="mn")
            nc.vector.tensor_scalar_min(mn[:], hln_rows[:], 0.0)
            ee = sp.tile([128, 8 * 64], f32, tag="ee")
            nc.scalar.activation(ee[:], mn[:], AF.Exp)
            nc.vector.tensor_scalar_max(hln_rows[:], hln_rows[:], 0.0)
            nc.vector.tensor_add(hln_rows[:], hln_rows[:], ee[:])
            nc.vector.tensor_scalar_add(hln_rows[:], hln_rows[:], -1.0)

        # ============ P3: transpose + AG h_ln^T ============
        with ExitStack() as ctx:
            ps = ctx.enter_context(tc.tile_pool(name="p3ps", bufs=3,
                                                space="PSUM"))
            hr = hln_rows[:].rearrange("p (t d) -> p t d", d=64)
            for m in range(8):
                pt = ps.tile([64, 128], f32, tag="pt")
                nc.tensor.transpose(pt[:], hr[:, m, :], eye[:])
                nc.vector.tensor_copy(hT_loc_sb[:, m * 128:(m + 1) * 128], pt[:])
            nc.sync.dma_start(ht_loc.ap()[:], hT_loc_sb[:])
            nc.gpsimd.collective_compute(
                "AllGather", OP.bypass, replica_groups=[list(range(NCORES))],
                ins=[ht_loc.ap()[:]], outs=[ht_ag.ap()[:]])
            for c in range(NCORES):
                nc.sync.dma_start(hlnT[:, c * P:(c + 1) * P],
                                  ht_ag.ap()[c * 64:(c + 1) * 64, :])

        # ============ P4: attention ============
        with ExitStack() as ctx:
            pool = ctx.enter_context(tc.tile_pool(name="p4", bufs=2))
            ps = ctx.enter_context(tc.tile_pool(name="p4ps", bufs=2,
                                                space="PSUM"))
            pvps = ctx.enter_context(tc.tile_pool(name="pvps", bufs=1,
                                                  space="PSUM"))
            kT = pers.tile([64, N], f32r, name="kT", tag="kT")
            qT = pers.tile([64, P], f32r, name="qT", tag="qT")
            vaug = pers.tile([128, 64 * 7], bf16, name="vaug", tag="vaug")
            for j in range(16):
                pk = ps.tile([64, 512], f32, tag="pss")
                nc.tensor.matmul(pk[:], wk[:], hlnT[:, j * 512:(j + 1) * 512],
                                 start=True, stop=True)
                nc.vector.tensor_copy(kT[:, j * 512:(j + 1) * 512], pk[:])
            for j in range(2):
                pq = ps.tile([64, 512], f32, tag="pss")
                nc.tensor.matmul(pq[:], wq[:],
                                 hT_loc_sb[:, j * 512:(j + 1) * 512],
                                 start=True, stop=True)
                nc.vector.tensor_copy(qT[:, j * 512:(j + 1) * 512], pq[:])
            va3 = vaug[:].rearrange("p (n d) -> p n d", d=7)
            for nt in range(64):
                pv = ps.tile([128, 7], f32, tag="pss")
                nc.tensor.matmul(pv[:], hlnT[:, nt * 128:(nt + 1) * 128],
                                 wv7[:], start=True, stop=True)
                nc.vector.tensor_copy(va3[:, nt, :], pv[:])
            nc.gpsimd.memset(va3[:, :, 6:7], 1.0)

            NTB = 3  # n-tiles per psum batch (3 banks)
            att = pool.tile([128, 8 * 7], f32, tag="att")
            at3 = att[:].rearrange("p (t d) -> p t d", d=7)
            for mc in range(2):
                po = pvps.tile([7, 512], f32, tag="po")
                nb_list = [(s, min(s + NTB, 64)) for s in range(0, 64, NTB)]
                for (s0, s1) in nb_list:
                    w = (s1 - s0) * 512
                    pss = ps.tile([128, NTB * 512], f32, tag="pss")
                    for i, nt in enumerate(range(s0, s1)):
                        nc.tensor.matmul(
                            pss[:, i * 512:(i + 1) * 512],
                            kT[:, nt * 128:(nt + 1) * 128],
                            qT[:, mc * 512:(mc + 1) * 512],
                            start=True, stop=True)
                    pT = pool.tile([128, NTB * 512], bf16, tag="pT")
                    nc.scalar.activation(pT[:, 0:w], pss[:, 0:w], AF.Exp,
                                         scale=0.125)
                    for i, nt in enumerate(range(s0, s1)):
                        nc.tensor.matmul(
                            po[:], va3[:, nt, :].bitcast(bf16),
                            pT[:, i * 512:(i + 1) * 512],
                            start=(nt == 0), stop=(nt == 63),
                            skip_group_check=True)
                spo = pool.tile([7, 512], f32, tag="spo")
                nc.vector.tensor_copy(spo[:], po[:])
                for i in range(4):
                    ptr = ps.tile([128, 7], f32, tag="pss")
                    nc.tensor.transpose(ptr[:], spo[:, i * 128:(i + 1) * 128],
                                        eye[0:7, 0:7])
                    nc.vector.tensor_copy(at3[:, mc * 4 + i, :], ptr[:])
            # normalize + LN over 6
            rec = pool.tile([128, 8], f32, tag="reca")
            nc.vector.reciprocal(rec[:].rearrange("p (t a) -> p t a", a=1),
                                 at3[:, :, 6:7])
            ht3 = htln[:].rearrange("p (t d) -> p t d", d=6)
            nc.vector.tensor_tensor(
                ht3, at3[:, :, 0:6],
                bc(rec[:].rearrange("p (t a) -> p t a", a=1), (128, 8, 6)),
                OP.mult)
            rs_ = pool.tile([128, 8], f32, tag="rsb")
            nc.vector.tensor_reduce(rs_[:], ht3, mybir.AxisListType.X, OP.add)
            mean = pool.tile([128, 8], f32, tag="meanb")
            nc.scalar.mul(mean[:], rs_[:], 1.0 / 6)
            nc.vector.tensor_tensor(
                ht3, ht3, bc(mean[:].rearrange("p (t a) -> p t a", a=1),
                             (128, 8, 6)), OP.subtract)
            sq = pool.tile([128, 8 * 6], f32, tag="sqb")
            ssum = pool.tile([128, 8], f32, tag="ssumb")
            sq3b = sq[:].rearrange("p (t d) -> p t d", d=6)
            nc.scalar.activation(sq3b, ht3, AF.Square)
            nc.vector.tensor_reduce(ssum[:], sq3b, mybir.AxisListType.X, OP.add)
            stdb = pool.tile([128, 8], f32, tag="stdb")
            nc.scalar.activation(stdb[:], ssum[:], AF.Sqrt, bias=epsc[:],
                                 scale=1.0 / 6)
            rstd = pool.tile([128, 8], f32, tag="rstdb")
            nc.vector.reciprocal(rstd[:], stdb[:])
            nc.vector.tensor_tensor(
                ht3, ht3, bc(rstd[:].rearrange("p (t a) -> p t a", a=1),
                             (128, 8, 6)), OP.mult)
            nc.vector.tensor_tensor(
                ht3, ht3, bc(l2g.rearrange("p (a d) -> p a d", a=1),
                             (128, 8, 6)), OP.mult)
            nc.vector.tensor_tensor(
                ht3, ht3, bc(l2b.rearrange("p (a d) -> p a d", a=1),
                             (128, 8, 6)), OP.add)

        # ============ P5: T2 build + AG ============
        with ExitStack() as ctx:
            pool = ctx.enter_context(tc.tile_pool(name="p5", bufs=3))
            ps = ctx.enter_context(tc.tile_pool(name="p5ps", bufs=3,
                                                space="PSUM"))
            htT = pool.tile([6, P], f32, tag="htT")
            ht3 = htln[:].rearrange("p (t d) -> p t d", d=6)
            for m in range(8):
                pt = ps.tile([6, 128], f32, tag="pt2")
                nc.tensor.transpose(pt[:], ht3[:, m, :], eye[:])
                nc.vector.tensor_copy(htT[:, m * 128:(m + 1) * 128], pt[:])
            h2a = pool.tile([128, 8 * 64], f32, tag="h2a")
            h2b = pool.tile([128, 8 * 48], bf16, tag="h2b")
            h2a3 = h2a[:].rearrange("p (t d) -> p t d", d=64)
            h2b3 = h2b[:].rearrange("p (t d) -> p t d", d=48)
            for m in range(8):
                pm = ps.tile([128, 64], f32, tag="pm2")
                nc.tensor.matmul(pm[:], hT_loc_sb[:, m * 128:(m + 1) * 128],
                                 w2top[:], start=True, stop=False)
                nc.tensor.matmul(pm[:], htT[:, m * 128:(m + 1) * 128],
                                 w2bot[:], start=False, stop=True)
                nc.scalar.activation(h2a3[:, m, :], pm[:], AF.Copy)
                nc.vector.tensor_copy(h2b3[:, m, :], pm[:, 0:48])
            rows_to_dram(t2_loc, h2b3[:, :, :], 48, col0=0, cast=bf16)
            rows_to_dram(t2_loc, h2a3[:, :, 48:56], 8, col0=24)
            rows_to_dram(t2_loc, h2a3[:, :, 56:64], 8, col0=32)
            nc.gpsimd.collective_compute(
                "AllGather", OP.bypass, replica_groups=[list(range(NCORES))],
                ins=[t2_loc.ap()[:]], outs=[t2_full.ap()[:]])

        # ============ P6: GAT2 edge phase ============
        edge_phase(t2_full, src_it[:], dst_it[:], 48, 24, 32, gat2,
                   build_sel=False)

        # ============ P7: finale ============
        with ExitStack() as ctx:
            sp = ctx.enter_context(tc.tile_pool(name="p7", bufs=2))
            g3 = gat2[:].rearrange("p (t d) -> p t d", d=56)
            d8 = sp.tile([128, 8 * 8], f32, tag="d8")
            nc.vector.tensor_scalar_mul(d8[:].rearrange("p (t h) -> p t h", h=8),
                                        g3[:, :, 48:56], 8.0)
            rec = sp.tile([128, 8 * 8], f32, tag="rec2")
            nc.vector.reciprocal(rec[:], d8[:])
            avg = sp.tile([128, 8 * 48], f32, tag="avg")
            a4 = avg[:].rearrange("p (t h c) -> p t h c", t=8, h=8)
            rec4 = rec[:].rearrange("p (t h a) -> p t h a", t=8, h=8)
            nc.vector.tensor_tensor(
                a4, g3[:, :, 0:48].rearrange("p t (h c) -> p t h c", h=8),
                bc(rec4, (128, 8, 8, 6)), OP.mult)
            swp = sp.tile([128, 8 * 48], f32, tag="swp")
            s4 = swp[:].rearrange("p (t c h) -> p t c h", t=8, c=6)
            nc.vector.tensor_copy(
                s4, avg[:].rearrange("p (t h c) -> p t h c", t=8, h=8)
                .rearrange("p t h c -> p t c h"))
            out2 = sp.tile([128, 8 * 6], f32, tag="out2")
            o3 = out2[:].rearrange("p (t d) -> p t d", d=6)
            nc.vector.tensor_reduce(o3, s4, mybir.AxisListType.X, OP.add)
            nc.vector.tensor_tensor(
                o3, o3, bc(b2r.rearrange("p (a d) -> p a d", a=1),
                           (128, 8, 6)), OP.add)
            ex = sp.tile([128, 8 * 6], f32, tag="exo")
            es = sp.tile([128, 8], f32, tag="eso")
            ex3 = ex[:].rearrange("p (t d) -> p t d", d=6)
            nc.scalar.activation(ex3, o3, AF.Exp)
            nc.vector.tensor_reduce(es[:], ex3, mybir.AxisListType.X, OP.add)
            ls = sp.tile([128, 8], f32, tag="lso")
            nc.scalar.activation(ls[:], es[:], AF.Ln)
            nc.vector.tensor_tensor(
                o3, o3, bc(ls[:].rearrange("p (t a) -> p t a", a=1),
                           (128, 8, 6)), OP.subtract)
            rows_to_dram(out_d, o3[:, :, :], 6)

    nc.compile()
    return nc


# ---------------- dispatch layer (cached jit over PJRT) ----------------

_SESS = {}


def _get_session(nbt):
    if nbt in _SESS:
        return _SESS[nbt]
    nc = build_kernel(nbt)
    bass2jax.install_neuronx_cc_hook()
    partition_name = (nc.partition_id_tensor.name
                      if nc.partition_id_tensor else None)
    in_names, out_names, out_avals = [], [], []
    for alloc in nc.m.functions[0].allocations:
        if not isinstance(alloc, mybir.MemoryLocationSet):
            continue
        name = alloc.memorylocations[0].name
        if alloc.kind == "ExternalInput":
            if name != partition_name:
                in_names.append(name)
        elif alloc.kind == "ExternalOutput":
            out_names.append(name)
            out_avals.append(jax.core.ShapedArray(
                tuple(alloc.tensor_shape), mybir.dt.np(alloc.dtype)))
    n_params = len(in_names)
    n_outs = len(out_avals)
    all_names = list(in_names) + list(out_names)
    if partition_name is not None:
        all_names.append(partition_name)

    def _body(*args):
        operands = list(args)
        if partition_name is not None:
            operands.append(bass2jax.partition_id_tensor())
        outs = bass2jax._bass_exec_p.bind(
            *operands,
            out_avals=tuple(out_avals),
            in_names=tuple(all_names),
            out_names=tuple(out_names),
            lowering_input_output_aliases=(),
            sim_require_finite=True,
            sim_require_nnan=True,
            nc=nc,
        )
        return tuple(outs)

    devices = jax.devices()[:NCORES]
    mesh = Mesh(np.asarray(devices), ("core",))
    sharding = NamedSharding(mesh, PartitionSpec("core"))
    sharded = jax.jit(
        shard_map(_body, mesh=mesh,
                  in_specs=(PartitionSpec("core"),) * (n_params + n_outs),
                  out_specs=(PartitionSpec("core"),) * n_outs,
                  check_rep=False),
        donate_argnums=tuple(range(n_params, n_params + n_outs)),
        keep_unused=True)
    zeros_fns = [
        jax.jit(partial(jnp.zeros,
                        (NCORES * a.shape[0], *a.shape[1:]), a.dtype),
                out_shardings=sharding)
        for a in out_avals]
    sess = dict(sharded=sharded, in_names=in_names, out_names=out_names,
                zeros_fns=zeros_fns)
    _SESS[nbt] = sess
    return sess


def _prepare_arrays(inputs):
    """Host prep: full np inputs -> (concat input arrays by name, nbt)."""
    x = np.asarray(inputs["x"], np.float32)
    xT = np.ascontiguousarray(x.T)                       # [256, 8192]
    xTg = np.ascontiguousarray(
        xT.reshape(256, NCORES, P).transpose(1, 0, 2)
    ).reshape(NCORES * 256, P).astype(BF16)
    idx, off, nbt = prep_edges(np.asarray(inputs["edge_index"]))
    wblob = prep_weights(
        inputs["W1"], inputs["a_src1"], inputs["a_dst1"], inputs["b1"],
        inputs["ln1_g"], inputs["ln1_b"], inputs["Wq"], inputs["Wk"],
        inputs["Wv"], inputs["ln2_g"], inputs["ln2_b"], inputs["W2"],
        inputs["a_src2"], inputs["a_dst2"], inputs["b2"])
    arrs = {
        "xin": xTg,
        "idx": np.ascontiguousarray(idx.reshape(NCORES * 32, -1)),
        "offu8": np.ascontiguousarray(off.reshape(NCORES * 128, -1)),
        "wblob": wblob,
    }
    return arrs, nbt


def _run(sess, arrs):
    """Dispatch: host np inputs -> host np output [N, 6]."""
    ins = [arrs[nm] for nm in sess["in_names"]]
    zs = [zf() for zf in sess["zeros_fns"]]
    outs = sess["sharded"](*ins, *zs)
    out = np.asarray(outs[sess["out_names"].index("out")])
    return out.reshape(N, 6)


def kernel(**inputs):
    arrs, nbt = _prepare_arrays(inputs)
    last_err = None
    for attempt in range(4):
        try:
            sess = _get_session(nbt)
            out = _run(sess, arrs)
            if np.isfinite(out).all():
                return out
            last_err = RuntimeError("non-finite output")
        except Exception as e:  # transient NRT/axon failures
            last_err = e
            _SESS.pop(nbt, None)
            time.sleep(10)
    raise last_err


# revision 13
# speedup vs baseline: 9.1521x; 1.4840x over previous
"""Self-contained Trainium2 Bass kernel for nn_GAT_transformer.

kernel(**inputs) -> np.ndarray [8192, 6] (log_softmax output).

Strategy: 8-core SPMD, nodes (and incident edges grouped by dst) sharded
across cores. GAT message passing uses dma_gather from an AllGathered
per-node table, segment softmax without max subtraction, and per-128-edge
one-hot selector matmuls with a fused denominator column. The dense NxN
attention runs keys-on-partitions with row-sharded Q and AllGathered K/V.

The dispatch path is latency/bandwidth-bound over the remote PJRT tunnel,
so the host->device contract is minimized: x ships pre-transposed in
bf16, the gather index tables ship in their compact 16-row wrap form and
are replicated across gpsimd stripes on device, dst offsets ship as
uint8, and the small weights ship sharded (1/8 per core) and are
AllGathered + unpacked on device. The jitted executable is cached across
calls.
"""
import time
from contextlib import ExitStack
from functools import partial

import numpy as np
import ml_dtypes

import jax
import jax.numpy as jnp
from jax.sharding import Mesh, PartitionSpec, NamedSharding
from jax.experimental.shard_map import shard_map

import concourse.bacc as bacc
import concourse.tile as tile
from concourse import mybir, bass2jax

N = 8192
E = 262144
D_IN = 256
HEADS = 8
HID = 8
D_OUT = 6
S_MAX = 64
NEG_SLOPE = 0.2
LN_EPS = 1e-5
NCORES = 8
P = N // NCORES            # 1024 nodes per core
GROUP = 64                 # dsts per segment-matmul group
NGROUP = P // GROUP        # 16 groups per core

# weight blob layout (f32 elements, flat; W1aug folded into host-side h1a)
O_W2T = 0                  # W2aug rows 0:64  [64, 64]
O_W2B = 4096               # W2aug rows 64:70 [6, 64]
O_WQ = 4480                # Wq [64, 64]
O_WK = 8576                # Wk [64, 64]
O_WV = 12672               # Wv [64, 6]
O_VEC = 13056              # b1(64) ln1g(64) ln1b(64) ln2g(6) ln2b(6) b2(6)
W_TOT = 13266
WSH = 1664                 # per-core blob width (8 * 1664 = 13312 >= W_TOT)

BF16 = ml_dtypes.bfloat16


def blob_layout(nbt):
    """Byte offsets of the per-core packed input blob:
    h bf16 [P,64] | a f16 [P,16] | idx i16 [32, nblk*8] | off u8 [128, nblk]
    | wshard f32 [WSH]."""
    nblk = nbt * NGROUP
    o_a = P * 64 * 2
    o_idx = o_a + P * 16 * 2
    o_off = o_idx + 32 * (nblk * 8) * 2
    o_w = (o_off + 128 * nblk + 3) // 4 * 4
    total = o_w + WSH * 4
    return o_a, o_idx, o_off, o_w, total


def wrap_idx(idx):
    """int array [n] (n % 16 == 0) -> int16 [16, n//16] wrap (compact form;
    replicated to the 8 gpsimd stripes on device)."""
    idx = np.asarray(idx, np.int16)
    return idx.reshape(-1, 16).T.copy()


def prep_edges(edge_index):
    """Shard + sort + block the edge list.

    Returns (idx [NCORES, 32, nblk*8] i16   (rows 0:16 src, 16:32 dst),
             off [NCORES, 128, nblk] u8, nbt)."""
    src = np.asarray(edge_index[0], np.int64)
    dst = np.asarray(edge_index[1], np.int64)
    loops = np.arange(N, dtype=np.int64)
    src = np.concatenate([src, loops])
    dst = np.concatenate([dst, loops])

    per_core = []
    max_blocks = 0
    for c in range(NCORES):
        m = (dst // P) == c
        s, d = src[m], dst[m] - c * P
        order = np.argsort(d, kind="stable")
        s, d = s[order], d[order]
        g = d // GROUP
        cnt = np.bincount(g, minlength=NGROUP)
        nb = (cnt + 127) // 128
        max_blocks = max(max_blocks, int(nb.max()))
        per_core.append((s, d, cnt))

    nbt = max(18, (max_blocks + 1) // 2 * 2)
    nblk = nbt * NGROUP

    src_idx = np.zeros((NCORES, nblk * 128), np.int64)
    dst_idx = np.zeros((NCORES, nblk * 128), np.int64)
    dst_off = np.full((NCORES, nblk * 128), GROUP, np.uint8)
    for c in range(NCORES):
        s, d, cnt = per_core[c]
        pos = 0
        for grp in range(NGROUP):
            n = int(cnt[grp])
            base = grp * nbt * 128
            src_idx[c, base:base + n] = s[pos:pos + n]
            dst_idx[c, base:base + n] = d[pos:pos + n] + c * P
            dst_off[c, base:base + n] = (d[pos:pos + n] - grp * GROUP)
            pos += n
    idx = np.stack([
        np.concatenate([wrap_idx(src_idx[c]), wrap_idx(dst_idx[c])], axis=0)
        for c in range(NCORES)])
    off = dst_off.reshape(NCORES, nblk, 128).transpose(0, 2, 1).copy()
    return idx, off, nbt


def expand_att(a):
    """a [HEADS, C] -> block matrix [HEADS*C, HEADS] so that
    (h @ A)[n, head] = sum_c h[n, head, c] * a[head, c]."""
    hh, cc = a.shape
    A = np.zeros((hh * cc, hh), np.float32)
    for h in range(hh):
        A[h * cc:(h + 1) * cc, h] = a[h]
    return A


def prep_weights(a_src2, a_dst2, b1, ln1_g, ln1_b, Wq, Wk, Wv,
                 ln2_g, ln2_b, W2, b2):
    """Constant-fold the tiny weights into one flat blob [NCORES, WSH]."""
    W2 = np.asarray(W2, np.float32)
    W2aug = np.concatenate(
        [W2, W2 @ expand_att(np.asarray(a_src2, np.float32)),
         W2 @ expand_att(np.asarray(a_dst2, np.float32))], axis=1)  # [70, 64]
    blob = np.zeros(NCORES * WSH, np.float32)
    blob[O_W2T:O_W2T + 4096] = W2aug[0:64].reshape(-1)
    blob[O_W2B:O_W2B + 384] = W2aug[64:70].reshape(-1)
    blob[O_WQ:O_WQ + 4096] = np.asarray(Wq, np.float32).reshape(-1)
    blob[O_WK:O_WK + 4096] = np.asarray(Wk, np.float32).reshape(-1)
    blob[O_WV:O_WV + 384] = np.asarray(Wv, np.float32).reshape(-1)
    vec = np.concatenate([
        np.asarray(b1, np.float32).reshape(-1),
        np.asarray(ln1_g, np.float32).reshape(-1),
        np.asarray(ln1_b, np.float32).reshape(-1),
        np.asarray(ln2_g, np.float32).reshape(-1),
        np.asarray(ln2_b, np.float32).reshape(-1),
        np.asarray(b2, np.float32).reshape(-1)])
    blob[O_VEC:O_VEC + 210] = vec
    return blob.reshape(NCORES, WSH)


f32 = mybir.dt.float32
f32r = mybir.dt.float32r
bf16 = mybir.dt.bfloat16
f16 = mybir.dt.float16
i16 = mybir.dt.int16
i32 = mybir.dt.int32
u8 = mybir.dt.uint8
AF = mybir.ActivationFunctionType
OP = mybir.AluOpType


def bc(ap, shape):
    return ap.broadcast_to(tuple(shape))


def build_kernel(nbt):
    nblk = nbt * NGROUP
    CB = 2 * nbt              # blocks per dst-tile chunk (2 groups)
    nc = bacc.Bacc("TRN2", target_bir_lowering=False, debug=False,
                   num_devices=NCORES)

    # ---------------- DRAM I/O ----------------
    o_a, o_idx, o_off, o_w, b_bytes = blob_layout(nbt)
    blob_d = nc.dram_tensor("blob", [1, b_bytes], u8, kind="ExternalInput")
    bv_bf = blob_d.ap().bitcast(bf16).rearrange("a b -> (a b)")
    bv_f16 = blob_d.ap().bitcast(f16).rearrange("a b -> (a b)")
    bv_i16 = blob_d.ap().bitcast(i16).rearrange("a b -> (a b)")
    bv_u8 = blob_d.ap().rearrange("a b -> (a b)")
    bv_f32 = blob_d.ap().bitcast(f32).rearrange("a b -> (a b)")

    out_d = nc.dram_tensor("out", [P, 6], f32, kind="ExternalOutput")

    wfull_d = nc.dram_tensor("wfull", [NCORES, WSH], f32, addr_space="Shared")
    t1_loc = nc.dram_tensor("t1_loc", [P, 64], f32)
    t1_full = nc.dram_tensor("t1_full", [N, 64], f32, addr_space="Shared")
    t2_loc = nc.dram_tensor("t2_loc", [P, 64], f32)
    t2_full = nc.dram_tensor("t2_full", [N, 64], f32, addr_space="Shared")
    ht_loc = nc.dram_tensor("ht_loc", [64, P], f32)
    ht_ag = nc.dram_tensor("ht_ag", [64 * NCORES, P], f32, addr_space="Shared")

    with tile.TileContext(nc) as tc, ExitStack() as top:
        # ---------------- persistent SBUF ----------------
        pers = top.enter_context(tc.tile_pool(name="pers", bufs=1))

        def ptile(name, shape, dtype):
            return pers.tile(shape, dtype, name=name, tag=name)

        # weights: AllGather the sharded blob, then unpack.
        # (collectives cannot read IO tensors -> bounce via internal DRAM)
        wstage_d = nc.dram_tensor("wstage", [1, WSH], f32)
        nc.sync.dma_start(
            wstage_d.ap()[:],
            bv_f32[o_w // 4:o_w // 4 + WSH].rearrange("(a b) -> a b", a=1))
        nc.gpsimd.collective_compute(
            "AllGather", OP.bypass, replica_groups=[list(range(NCORES))],
            ins=[wstage_d.ap()[:]], outs=[wfull_d.ap()[:]])
        flat = wfull_d.ap().rearrange("a b -> (a b)")

        w2top = ptile("w2top", [64, 64], f32)
        nc.sync.dma_start(w2top[:],
                          flat[O_W2T:O_W2T + 4096].rearrange("(p d) -> p d", d=64))
        w2bot = ptile("w2bot", [6, 64], f32)
        nc.sync.dma_start(w2bot[:],
                          flat[O_W2B:O_W2B + 384].rearrange("(p d) -> p d", d=64))
        wq = ptile("wq", [64, 64], f32)
        nc.sync.dma_start(wq[:],
                          flat[O_WQ:O_WQ + 4096].rearrange("(p d) -> p d", d=64))
        wk = ptile("wk", [64, 64], f32)
        nc.sync.dma_start(wk[:],
                          flat[O_WK:O_WK + 4096].rearrange("(p d) -> p d", d=64))
        wv7 = ptile("wv7", [64, 7], f32)
        nc.gpsimd.memset(wv7[:], 0.0)
        nc.sync.dma_start(wv7[:, 0:6],
                          flat[O_WV:O_WV + 384].rearrange("(p d) -> p d", d=6))
        vec210 = ptile("vec210", [1, 210], f32)
        nc.sync.dma_start(vec210[:],
                          flat[O_VEC:O_VEC + 210].rearrange("(a b) -> a b", a=1))
        ones1 = ptile("ones1", [1, 128], f32)
        nc.gpsimd.memset(ones1[:], 1.0)
        rep210 = ptile("rep210", [128, 210], f32)
        with tc.tile_pool(name="sps", bufs=1, space="PSUM") as sps:
            rp = sps.tile([128, 210], f32, tag="rp")
            nc.tensor.matmul(rp[:], ones1[:], vec210[:], start=True, stop=True)
            nc.scalar.activation(rep210[:], rp[:], AF.Copy)
        b1r = rep210[:][:, 0:64]
        l1g = rep210[:][:, 64:128]
        l1b = rep210[:][:, 128:192]
        l2g = rep210[:][:, 192:198]
        l2b = rep210[:][:, 198:204]
        b2r = rep210[:][:, 204:210]

        # identity matrix via iota + is_equal
        coli = ptile("coli", [128, 128], i32)
        nc.gpsimd.iota(coli[:], pattern=[[1, 128]], base=0, channel_multiplier=0)
        rowi = ptile("rowi", [128, 1], i32)
        nc.gpsimd.iota(rowi[:], pattern=[[1, 1]], base=0, channel_multiplier=1)
        colf = ptile("colf", [128, 128], f32)
        nc.vector.tensor_copy(colf[:], coli[:])
        rowf = ptile("rowf", [128, 1], f32)
        nc.vector.tensor_copy(rowf[:], rowi[:])
        eye = ptile("eye", [128, 128], f32)
        nc.vector.tensor_tensor(eye[:], colf[:], bc(rowf[:], (128, 128)),
                                OP.is_equal)

        iotaf = ptile("iotaf", [128, GROUP], f32)
        ioi = ptile("ioi", [128, GROUP], i32)
        nc.gpsimd.iota(ioi[:], pattern=[[1, GROUP]], base=0, channel_multiplier=0)
        nc.vector.tensor_copy(iotaf[:], ioi[:])

        # gather index tables: replicate compact 16-row wraps to 8 stripes
        idxv = bv_i16[o_idx // 2:o_idx // 2 + 32 * nblk * 8].rearrange(
            "(r c) -> r c", r=32)
        src_it = ptile("src_it", [128, nblk * 8], i16)
        dst_it = ptile("dst_it", [128, nblk * 8], i16)
        for g in range(8):
            nc.sync.dma_start(src_it[:][16 * g:16 * (g + 1), :], idxv[0:16, :])
            nc.sync.dma_start(dst_it[:][16 * g:16 * (g + 1), :], idxv[16:32, :])
        offu = ptile("offu", [128, nblk], u8)
        nc.sync.dma_start(offu[:],
                          bv_u8[o_off:o_off + 128 * nblk].rearrange(
                              "(p c) -> p c", p=128))
        dst_off = ptile("dst_off", [128, nblk], f32)
        nc.vector.tensor_copy(dst_off[:], offu[:])

        epsc = ptile("epsc", [128, 1], f32)
        nc.gpsimd.memset(epsc[:], LN_EPS)
        sel = ptile("sel", [128, nblk * GROUP], bf16)
        hlnT = ptile("hlnT", [64, N], f32)        # full h_ln^T after AG
        hT_loc_sb = ptile("hT_loc_sb", [64, P], f32)
        gat1 = ptile("gat1", [128, 8 * 72], f32)
        gat2 = ptile("gat2", [128, 8 * 56], f32)
        hln_rows = ptile("hln_rows", [128, 8 * 64], f32)
        htln = ptile("htln", [128, 8 * 6], f32)

        def rows_to_dram(dram, sb_view, ncols, col0=0, cast=None):
            """sb_view [128, 8, w] -> dram rows [(t*128+p), col0:col0+w]."""
            dv = dram.ap()
            if cast is not None:
                dv = dv.bitcast(cast)
            dv = dv.rearrange("(t p) d -> p t d", p=128)
            nc.sync.dma_start(dv[:, :, col0:col0 + ncols], sb_view)

        # ================= P0: host-projected h1a -> T1 =================
        with ExitStack() as ctx:
            pool = ctx.enter_context(tc.tile_pool(name="p0", bufs=2))
            hrows = pool.tile([128, 8 * 64], bf16, tag="hrows")
            nc.sync.dma_start(
                hrows[:].rearrange("p (t d) -> p t d", t=8),
                bv_bf[0:P * 64].rearrange("(t p d) -> p t d", t=8, p=128, d=64))
            a16 = pool.tile([128, 8 * 16], f16, tag="a16")
            nc.sync.dma_start(
                a16[:].rearrange("p (t d) -> p t d", t=8),
                bv_f16[o_a // 2:o_a // 2 + P * 16].rearrange(
                    "(t p d) -> p t d", t=8, p=128, d=16))
            arows = pool.tile([128, 8 * 16], f32, tag="arows")
            nc.vector.tensor_copy(arows[:], a16[:])
            rows_to_dram(t1_loc,
                         hrows[:].rearrange("p (t d) -> p t d", t=8)[:, :, :],
                         64, col0=0, cast=bf16)
            rows_to_dram(t1_loc,
                         arows[:].rearrange("p (t d) -> p t d", t=8)[:, :, :],
                         16, col0=32)
            nc.gpsimd.collective_compute(
                "AllGather", OP.bypass, replica_groups=[list(range(NCORES))],
                ins=[t1_loc.ap()[:]], outs=[t1_full.ap()[:]])

        # ============ edge phase (shared by both layers) ============
        def edge_phase(table, it_src, it_dst, hcols, acol, dcol, gatacc,
                       build_sel):
            # hcols: # bf16 feature cols; acol/dcol: f32 col of asrc/adst
            with ExitStack() as ctx:
                gp = ctx.enter_context(tc.tile_pool(name="gp", bufs=2))
                sp = ctx.enter_context(tc.tile_pool(name="sp", bufs=2))
                pg = ctx.enter_context(tc.tile_pool(name="pg", bufs=4,
                                                    space="PSUM"))
                for t in range(8):
                    j0 = t * CB
                    gs = gp.tile([128, CB * 64], f32, tag="gs")
                    nc.gpsimd.dma_gather(
                        gs[:].rearrange("p (j e) -> p j e", e=64),
                        table.ap()[:], it_src[:, j0 * 8:(j0 + CB) * 8],
                        num_idxs=CB * 128, num_idxs_reg=CB * 128, elem_size=64,
                        single_packet=False)
                    gd = gp.tile([128, CB * 64], f32, tag="gd")
                    nc.gpsimd.dma_gather(
                        gd[:].rearrange("p (j e) -> p j e", e=64),
                        table.ap()[:], it_dst[:, j0 * 8:(j0 + CB) * 8],
                        num_idxs=CB * 128, num_idxs_reg=CB * 128, elem_size=64,
                        single_packet=False)
                    gs3 = gs[:].rearrange("p (j e) -> p j e", e=64)
                    gd3 = gd[:].rearrange("p (j e) -> p j e", e=64)
                    z = sp.tile([128, CB * 8], f32, tag="z")
                    z3 = z[:].rearrange("p (j e) -> p j e", e=8)
                    nc.vector.tensor_tensor(z3, gs3[:, :, acol:acol + 8],
                                            gd3[:, :, dcol:dcol + 8], OP.add)
                    u = sp.tile([128, CB * 8], f32, tag="u")
                    nc.vector.tensor_scalar_mul(u[:], z[:], 0.2)
                    nc.vector.tensor_max(z[:], z[:], u[:])
                    exf = sp.tile([128, CB * 8], f32, tag="exf")
                    nc.scalar.activation(exf[:], z[:], AF.Exp)
                    exb = sp.tile([128, CB * 8], bf16, tag="exb")
                    nc.vector.tensor_copy(exb[:], exf[:])
                    exb3 = exb[:].rearrange("p (j e) -> p j e", e=8)
                    W = hcols + 8
                    msgs = sp.tile([128, CB * W], bf16, tag="msgs")
                    m3 = msgs[:].rearrange("p (j e) -> p j e", e=W)
                    hb = gs3.bitcast(bf16)  # [128, CB, 128] bf16
                    exb4 = exb3.rearrange("p j (h a) -> p j h a", a=1)
                    nc.vector.tensor_tensor(
                        m3[:, :, 0:hcols].rearrange("p j (h c) -> p j h c", h=8),
                        hb[:, :, 0:hcols].rearrange("p j (h c) -> p j h c", h=8),
                        bc(exb4, (128, CB, 8, hcols // 8)), OP.mult)
                    nc.vector.tensor_copy(m3[:, :, hcols:W], exb3)
                    if build_sel:
                        sel3 = sel[:].rearrange("p (j e) -> p j e", e=GROUP)
                        io_b = bc(iotaf[:].rearrange("p (a e) -> p a e", a=1),
                                  (128, CB, GROUP))
                        do_b = bc(dst_off[:, j0:j0 + CB]
                                  .rearrange("p (j a) -> p j a", a=1),
                                  (128, CB, GROUP))
                        nc.vector.tensor_tensor(sel3[:, j0:j0 + CB, :], io_b,
                                                do_b, OP.is_equal)
                    sel3 = sel[:].rearrange("p (j e) -> p j e", e=GROUP)
                    ga = gatacc[:].rearrange("p (t d) -> p t d", d=W)
                    for g in (0, 1):
                        pgt = pg.tile([64, W], f32, tag="pgt")
                        for b in range(nbt):
                            jj = (2 * t + g) * nbt + b
                            nc.tensor.matmul(
                                pgt[:], sel3[:, jj, :], m3[:, jj - j0, :],
                                start=(b == 0), stop=(b == nbt - 1))
                        nc.scalar.activation(
                            ga[64 * g:64 * (g + 1), t, :], pgt[:], AF.Copy)

        edge_phase(t1_full, src_it[:], dst_it[:], 64, 32, 40, gat1,
                   build_sel=True)

        # ============ P2: GAT1 -> h_ln ============
        with ExitStack() as ctx:
            sp = ctx.enter_context(tc.tile_pool(name="p2", bufs=2))
            g3 = gat1[:].rearrange("p (t d) -> p t d", d=72)
            rec = sp.tile([128, 8 * 8], f32, tag="rec")
            nc.vector.reciprocal(rec[:].rearrange("p (t h) -> p t h", h=8),
                                 g3[:, :, 64:72])
            h1 = hln_rows[:].rearrange("p (t d) -> p t d", d=64)
            rec4 = rec[:].rearrange("p (t h a) -> p t h a", t=8, h=8)
            nc.vector.tensor_tensor(
                h1.rearrange("p t (h c) -> p t h c", h=8),
                g3[:, :, 0:64].rearrange("p t (h c) -> p t h c", h=8),
                bc(rec4, (128, 8, 8, 8)), OP.mult)
            b1b = bc(b1r.rearrange("p (a d) -> p a d", a=1), (128, 8, 64))
            nc.vector.tensor_tensor(h1, h1, b1b, OP.add)
            # layernorm over 64
            rs_ = sp.tile([128, 8], f32, tag="rs_")
            nc.vector.tensor_reduce(rs_[:], h1, mybir.AxisListType.X, OP.add)
            mean = sp.tile([128, 8], f32, tag="mean")
            nc.scalar.mul(mean[:], rs_[:], 1.0 / 64)
            nc.vector.tensor_tensor(
                h1, h1, bc(mean[:].rearrange("p (t a) -> p t a", a=1),
                           (128, 8, 64)), OP.subtract)
            sq = sp.tile([128, 8 * 64], f32, tag="sq")
            ssum = sp.tile([128, 8], f32, tag="ssum")
            sq3 = sq[:].rearrange("p (t d) -> p t d", d=64)
            nc.scalar.activation(sq3, h1, AF.Square)
            nc.vector.tensor_reduce(ssum[:], sq3, mybir.AxisListType.X, OP.add)
            std_ = sp.tile([128, 8], f32, tag="std_")
            nc.scalar.activation(std_[:], ssum[:], AF.Sqrt, bias=epsc[:],
                                 scale=1.0 / 64)
            rstd = sp.tile([128, 8], f32, tag="rstd")
            nc.vector.reciprocal(rstd[:], std_[:])
            nc.vector.tensor_tensor(
                h1, h1, bc(rstd[:].rearrange("p (t a) -> p t a", a=1),
                           (128, 8, 64)), OP.mult)
            nc.vector.tensor_tensor(
                h1, h1, bc(l1g.rearrange("p (a d) -> p a d", a=1),
                           (128, 8, 64)), OP.mult)
            nc.vector.tensor_tensor(
                h1, h1, bc(l1b.rearrange("p (a d) -> p a d", a=1),
                           (128, 8, 64)), OP.add)
            # elu
            mn = sp.tile([128, 8 * 64], f32, tag="mn")
            nc.vector.tensor_scalar_min(mn[:], hln_rows[:], 0.0)
            ee = sp.tile([128, 8 * 64], f32, tag="ee")
            nc.scalar.activation(ee[:], mn[:], AF.Exp)
            nc.vector.tensor_scalar_max(hln_rows[:], hln_rows[:], 0.0)
            nc.vector.tensor_add(hln_rows[:], hln_rows[:], ee[:])
            nc.vector.tensor_scalar_add(hln_rows[:], hln_rows[:], -1.0)

        # ============ P3: transpose + AG h_ln^T ============
        with ExitStack() as ctx:
            ps = ctx.enter_context(tc.tile_pool(name="p3ps", bufs=3,
                                                space="PSUM"))
            hr = hln_rows[:].rearrange("p (t d) -> p t d", d=64)
            for m in range(8):
                pt = ps.tile([64, 128], f32, tag="pt")
                nc.tensor.transpose(pt[:], hr[:, m, :], eye[:])
                nc.vector.tensor_copy(hT_loc_sb[:, m * 128:(m + 1) * 128], pt[:])
            nc.sync.dma_start(ht_loc.ap()[:], hT_loc_sb[:])
            nc.gpsimd.collective_compute(
                "AllGather", OP.bypass, replica_groups=[list(range(NCORES))],
                ins=[ht_loc.ap()[:]], outs=[ht_ag.ap()[:]])
            for c in range(NCORES):
                nc.sync.dma_start(hlnT[:, c * P:(c + 1) * P],
                                  ht_ag.ap()[c * 64:(c + 1) * 64, :])

        # ============ P4: attention ============
        with ExitStack() as ctx:
            pool = ctx.enter_context(tc.tile_pool(name="p4", bufs=2))
            ps = ctx.enter_context(tc.tile_pool(name="p4ps", bufs=2,
                                                space="PSUM"))
            pvps = ctx.enter_context(tc.tile_pool(name="pvps", bufs=1,
                                                  space="PSUM"))
            kT = pers.tile([64, N], f32r, name="kT", tag="kT")
            qT = pers.tile([64, P], f32r, name="qT", tag="qT")
            vaug = pers.tile([128, 64 * 7], bf16, name="vaug", tag="vaug")
            for j in range(16):
                pk = ps.tile([64, 512], f32, tag="pss")
                nc.tensor.matmul(pk[:], wk[:], hlnT[:, j * 512:(j + 1) * 512],
                                 start=True, stop=True)
                nc.vector.tensor_copy(kT[:, j * 512:(j + 1) * 512], pk[:])
            for j in range(2):
                pq = ps.tile([64, 512], f32, tag="pss")
                nc.tensor.matmul(pq[:], wq[:],
                                 hT_loc_sb[:, j * 512:(j + 1) * 512],
                                 start=True, stop=True)
                nc.vector.tensor_copy(qT[:, j * 512:(j + 1) * 512], pq[:])
            va3 = vaug[:].rearrange("p (n d) -> p n d", d=7)
            for nt in range(64):
                pv = ps.tile([128, 7], f32, tag="pss")
                nc.tensor.matmul(pv[:], hlnT[:, nt * 128:(nt + 1) * 128],
                                 wv7[:], start=True, stop=True)
                nc.vector.tensor_copy(va3[:, nt, :], pv[:])
            nc.gpsimd.memset(va3[:, :, 6:7], 1.0)

            NTB = 3  # n-tiles per psum batch (3 banks)
            att = pool.tile([128, 8 * 7], f32, tag="att")
            at3 = att[:].rearrange("p (t d) -> p t d", d=7)
            for mc in range(2):
                po = pvps.tile([7, 512], f32, tag="po")
                nb_list = [(s, min(s + NTB, 64)) for s in range(0, 64, NTB)]
                for (s0, s1) in nb_list:
                    w = (s1 - s0) * 512
                    pss = ps.tile([128, NTB * 512], f32, tag="pss")
                    for i, nt in enumerate(range(s0, s1)):
                        nc.tensor.matmul(
                            pss[:, i * 512:(i + 1) * 512],
                            kT[:, nt * 128:(nt + 1) * 128],
                            qT[:, mc * 512:(mc + 1) * 512],
                            start=True, stop=True)
                    pT = pool.tile([128, NTB * 512], bf16, tag="pT")
                    nc.scalar.activation(pT[:, 0:w], pss[:, 0:w], AF.Exp,
                                         scale=0.125)
                    for i, nt in enumerate(range(s0, s1)):
                        nc.tensor.matmul(
                            po[:], va3[:, nt, :].bitcast(bf16),
                            pT[:, i * 512:(i + 1) * 512],
                            start=(nt == 0), stop=(nt == 63),
                            skip_group_check=True)
                spo = pool.tile([7, 512], f32, tag="spo")
                nc.vector.tensor_copy(spo[:], po[:])
                for i in range(4):
                    ptr = ps.tile([128, 7], f32, tag="pss")
                    nc.tensor.transpose(ptr[:], spo[:, i * 128:(i + 1) * 128],
                                        eye[0:7, 0:7])
                    nc.vector.tensor_copy(at3[:, mc * 4 + i, :], ptr[:])
            # normalize + LN over 6
            rec = pool.tile([128, 8], f32, tag="reca")
            nc.vector.reciprocal(rec[:].rearrange("p (t a) -> p t a", a=1),
                                 at3[:, :, 6:7])
            ht3 = htln[:].rearrange("p (t d) -> p t d", d=6)
            nc.vector.tensor_tensor(
                ht3, at3[:, :, 0:6],
                bc(rec[:].rearrange("p (t a) -> p t a", a=1), (128, 8, 6)),
                OP.mult)
            rs_ = pool.tile([128, 8], f32, tag="rsb")
            nc.vector.tensor_reduce(rs_[:], ht3, mybir.AxisListType.X, OP.add)
            mean = pool.tile([128, 8], f32, tag="meanb")
            nc.scalar.mul(mean[:], rs_[:], 1.0 / 6)
            nc.vector.tensor_tensor(
                ht3, ht3, bc(mean[:].rearrange("p (t a) -> p t a", a=1),
                             (128, 8, 6)), OP.subtract)
            sq = pool.tile([128, 8 * 6], f32, tag="sqb")
            ssum = pool.tile([128, 8], f32, tag="ssumb")
            sq3b = sq[:].rearrange("p (t d) -> p t d", d=6)
            nc.scalar.activation(sq3b, ht3, AF.Square)
            nc.vector.tensor_reduce(ssum[:], sq3b, mybir.AxisListType.X, OP.add)
            stdb = pool.tile([128, 8], f32, tag="stdb")
            nc.scalar.activation(stdb[:], ssum[:], AF.Sqrt, bias=epsc[:],
                                 scale=1.0 / 6)
            rstd = pool.tile([128, 8], f32, tag="rstdb")
            nc.vector.reciprocal(rstd[:], stdb[:])
            nc.vector.tensor_tensor(
                ht3, ht3, bc(rstd[:].rearrange("p (t a) -> p t a", a=1),
                             (128, 8, 6)), OP.mult)
            nc.vector.tensor_tensor(
                ht3, ht3, bc(l2g.rearrange("p (a d) -> p a d", a=1),
                             (128, 8, 6)), OP.mult)
            nc.vector.tensor_tensor(
                ht3, ht3, bc(l2b.rearrange("p (a d) -> p a d", a=1),
                             (128, 8, 6)), OP.add)

        # ============ P5: T2 build + AG ============
        with ExitStack() as ctx:
            pool = ctx.enter_context(tc.tile_pool(name="p5", bufs=3))
            ps = ctx.enter_context(tc.tile_pool(name="p5ps", bufs=3,
                                                space="PSUM"))
            htT = pool.tile([6, P], f32, tag="htT")
            ht3 = htln[:].rearrange("p (t d) -> p t d", d=6)
            for m in range(8):
                pt = ps.tile([6, 128], f32, tag="pt2")
                nc.tensor.transpose(pt[:], ht3[:, m, :], eye[:])
                nc.vector.tensor_copy(htT[:, m * 128:(m + 1) * 128], pt[:])
            h2a = pool.tile([128, 8 * 64], f32, tag="h2a")
            h2b = pool.tile([128, 8 * 48], bf16, tag="h2b")
            h2a3 = h2a[:].rearrange("p (t d) -> p t d", d=64)
            h2b3 = h2b[:].rearrange("p (t d) -> p t d", d=48)
            for m in range(8):
                pm = ps.tile([128, 64], f32, tag="pm2")
                nc.tensor.matmul(pm[:], hT_loc_sb[:, m * 128:(m + 1) * 128],
                                 w2top[:], start=True, stop=False)
                nc.tensor.matmul(pm[:], htT[:, m * 128:(m + 1) * 128],
                                 w2bot[:], start=False, stop=True)
                nc.scalar.activation(h2a3[:, m, :], pm[:], AF.Copy)
                nc.vector.tensor_copy(h2b3[:, m, :], pm[:, 0:48])
            rows_to_dram(t2_loc, h2b3[:, :, :], 48, col0=0, cast=bf16)
            rows_to_dram(t2_loc, h2a3[:, :, 48:56], 8, col0=24)
            rows_to_dram(t2_loc, h2a3[:, :, 56:64], 8, col0=32)
            nc.gpsimd.collective_compute(
                "AllGather", OP.bypass, replica_groups=[list(range(NCORES))],
                ins=[t2_loc.ap()[:]], outs=[t2_full.ap()[:]])

        # ============ P6: GAT2 edge phase ============
        edge_phase(t2_full, src_it[:], dst_it[:], 48, 24, 32, gat2,
                   build_sel=False)

        # ============ P7: finale ============
        with ExitStack() as ctx:
            sp = ctx.enter_context(tc.tile_pool(name="p7", bufs=2))
            g3 = gat2[:].rearrange("p (t d) -> p t d", d=56)
            d8 = sp.tile([128, 8 * 8], f32, tag="d8")
            nc.vector.tensor_scalar_mul(d8[:].rearrange("p (t h) -> p t h", h=8),
                                        g3[:, :, 48:56], 8.0)
            rec = sp.tile([128, 8 * 8], f32, tag="rec2")
            nc.vector.reciprocal(rec[:], d8[:])
            avg = sp.tile([128, 8 * 48], f32, tag="avg")
            a4 = avg[:].rearrange("p (t h c) -> p t h c", t=8, h=8)
            rec4 = rec[:].rearrange("p (t h a) -> p t h a", t=8, h=8)
            nc.vector.tensor_tensor(
                a4, g3[:, :, 0:48].rearrange("p t (h c) -> p t h c", h=8),
                bc(rec4, (128, 8, 8, 6)), OP.mult)
            swp = sp.tile([128, 8 * 48], f32, tag="swp")
            s4 = swp[:].rearrange("p (t c h) -> p t c h", t=8, c=6)
            nc.vector.tensor_copy(
                s4, avg[:].rearrange("p (t h c) -> p t h c", t=8, h=8)
                .rearrange("p t h c -> p t c h"))
            out2 = sp.tile([128, 8 * 6], f32, tag="out2")
            o3 = out2[:].rearrange("p (t d) -> p t d", d=6)
            nc.vector.tensor_reduce(o3, s4, mybir.AxisListType.X, OP.add)
            nc.vector.tensor_tensor(
                o3, o3, bc(b2r.rearrange("p (a d) -> p a d", a=1),
                           (128, 8, 6)), OP.add)
            ex = sp.tile([128, 8 * 6], f32, tag="exo")
            es = sp.tile([128, 8], f32, tag="eso")
            ex3 = ex[:].rearrange("p (t d) -> p t d", d=6)
            nc.scalar.activation(ex3, o3, AF.Exp)
            nc.vector.tensor_reduce(es[:], ex3, mybir.AxisListType.X, OP.add)
            ls = sp.tile([128, 8], f32, tag="lso")
            nc.scalar.activation(ls[:], es[:], AF.Ln)
            nc.vector.tensor_tensor(
                o3, o3, bc(ls[:].rearrange("p (t a) -> p t a", a=1),
                           (128, 8, 6)), OP.subtract)
            rows_to_dram(out_d, o3[:, :, :], 6)

    nc.compile()
    return nc


# ---------------- dispatch layer (cached jit over PJRT) ----------------

_SESS = {}


def _get_session(nbt):
    if nbt in _SESS:
        return _SESS[nbt]
    nc = build_kernel(nbt)
    bass2jax.install_neuronx_cc_hook()
    partition_name = (nc.partition_id_tensor.name
                      if nc.partition_id_tensor else None)
    in_names, out_names, out_avals = [], [], []
    for alloc in nc.m.functions[0].allocations:
        if not isinstance(alloc, mybir.MemoryLocationSet):
            continue
        name = alloc.memorylocations[0].name
        if alloc.kind == "ExternalInput":
            if name != partition_name:
                in_names.append(name)
        elif alloc.kind == "ExternalOutput":
            out_names.append(name)
            out_avals.append(jax.core.ShapedArray(
                tuple(alloc.tensor_shape), mybir.dt.np(alloc.dtype)))
    n_params = len(in_names)
    n_outs = len(out_avals)
    all_names = list(in_names) + list(out_names)
    if partition_name is not None:
        all_names.append(partition_name)

    def _body(*args):
        operands = list(args)
        if partition_name is not None:
            operands.append(bass2jax.partition_id_tensor())
        outs = bass2jax._bass_exec_p.bind(
            *operands,
            out_avals=tuple(out_avals),
            in_names=tuple(all_names),
            out_names=tuple(out_names),
            lowering_input_output_aliases=(),
            sim_require_finite=True,
            sim_require_nnan=True,
            nc=nc,
        )
        return tuple(outs)

    devices = jax.devices()[:NCORES]
    mesh = Mesh(np.asarray(devices), ("core",))
    sharding = NamedSharding(mesh, PartitionSpec("core"))
    sharded = jax.jit(
        shard_map(_body, mesh=mesh,
                  in_specs=(PartitionSpec("core"),) * (n_params + n_outs),
                  out_specs=(PartitionSpec("core"),) * n_outs,
                  check_rep=False),
        donate_argnums=tuple(range(n_params, n_params + n_outs)),
        keep_unused=True)
    zeros_fns = [
        jax.jit(partial(jnp.zeros,
                        (NCORES * a.shape[0], *a.shape[1:]), a.dtype),
                out_shardings=sharding)
        for a in out_avals]
    sess = dict(sharded=sharded, in_names=in_names, out_names=out_names,
                zeros_fns=zeros_fns)
    _SESS[nbt] = sess
    return sess


def _prepare_arrays(inputs):
    """Host prep: full np inputs -> (concat input arrays by name, nbt)."""
    x = np.asarray(inputs["x"], np.float32)
    W1 = np.asarray(inputs["W1"], np.float32)
    W1aug = np.concatenate(
        [W1, W1 @ expand_att(np.asarray(inputs["a_src1"], np.float32)),
         W1 @ expand_att(np.asarray(inputs["a_dst1"], np.float32))],
        axis=1)                                          # [256, 80]
    h1a = x @ W1aug                                      # [N, 80] f32
    h_b = np.ascontiguousarray(h1a[:, 0:64]).astype(BF16)
    a_h = np.ascontiguousarray(h1a[:, 64:80]).astype(np.float16)
    idx, off, nbt = prep_edges(np.asarray(inputs["edge_index"]))
    wblob = prep_weights(
        inputs["a_src2"], inputs["a_dst2"], inputs["b1"],
        inputs["ln1_g"], inputs["ln1_b"], inputs["Wq"], inputs["Wk"],
        inputs["Wv"], inputs["ln2_g"], inputs["ln2_b"], inputs["W2"],
        inputs["b2"])
    o_a, o_idx, o_off, o_w, b_bytes = blob_layout(nbt)
    blob = np.zeros((NCORES, b_bytes), np.uint8)
    for c in range(NCORES):
        for o, arr in ((0, h_b[c * P:(c + 1) * P]),
                       (o_a, a_h[c * P:(c + 1) * P]),
                       (o_idx, idx[c]), (o_off, off[c]), (o_w, wblob[c])):
            bts = np.frombuffer(arr.tobytes(), np.uint8)
            blob[c, o:o + bts.size] = bts
    return {"blob": blob}, nbt


def _run(sess, arrs):
    """Dispatch: host np inputs -> host np output [N, 6]."""
    ins = [arrs[nm] for nm in sess["in_names"]]
    zs = [zf() for zf in sess["zeros_fns"]]
    outs = sess["sharded"](*ins, *zs)
    out = np.asarray(outs[sess["out_names"].index("out")])
    return out.reshape(N, 6)


def kernel(**inputs):
    arrs, nbt = _prepare_arrays(inputs)
    last_err = None
    for attempt in range(4):
        try:
            sess = _get_session(nbt)
            out = _run(sess, arrs)
            if np.isfinite(out).all():
                return out
            last_err = RuntimeError("non-finite output")
        except Exception as e:  # transient NRT/axon failures
            last_err = e
            _SESS.pop(nbt, None)
            time.sleep(10)
    raise last_err


# revision 31
# speedup vs baseline: 9.9377x; 1.0858x over previous
"""Self-contained Trainium2 Bass kernel for nn_GAT_transformer.

kernel(**inputs) -> np.ndarray [8192, 6] (log_softmax output).

Strategy: 8-core SPMD, nodes (and incident edges grouped by dst) sharded
across cores. GAT message passing uses dma_gather from an AllGathered
per-node table, segment softmax without max subtraction, and per-128-edge
one-hot selector matmuls with a fused denominator column. The dense NxN
attention runs keys-on-partitions with row-sharded Q and AllGathered K/V.

The dispatch path is latency/bandwidth-bound over the remote PJRT tunnel,
so the host->device contract is minimized: x ships pre-transposed in
bf16, the gather index tables ship in their compact 16-row wrap form and
are replicated across gpsimd stripes on device, dst offsets ship as
uint8, and the small weights ship sharded (1/8 per core) and are
AllGathered + unpacked on device. The jitted executable is cached across
calls.
"""
import time
from contextlib import ExitStack
from functools import partial

import numpy as np
import ml_dtypes

import jax
import jax.numpy as jnp
from jax.sharding import Mesh, PartitionSpec, NamedSharding
from jax.experimental.shard_map import shard_map

import concourse.bacc as bacc
import concourse.tile as tile
from concourse import mybir, bass2jax

N = 8192
E = 262144
D_IN = 256
HEADS = 8
HID = 8
D_OUT = 6
S_MAX = 64
NEG_SLOPE = 0.2
LN_EPS = 1e-5
NCORES = 8
P = N // NCORES            # 1024 nodes per core
GROUP = 64                 # dsts per segment-matmul group
NGROUP = P // GROUP        # 16 groups per core

# weight blob layout (f32 elements, flat; W1aug folded into host-side h1a)
O_W2T = 0                  # W2aug rows 0:64  [64, 64]
O_W2B = 4096               # W2aug rows 64:70 [6, 64]
O_WQ = 4480                # Wq [64, 64]
O_WK = 8576                # Wk [64, 64]
O_WV = 12672               # Wv [64, 6]
O_VEC = 13056              # b1(64) ln1g(64) ln1b(64) ln2g(6) ln2b(6) b2(6)
W_TOT = 13266
WSH = 1664                 # per-core blob width (8 * 1664 = 13312 >= W_TOT)

BF16 = ml_dtypes.bfloat16


def blob_layout(nbt):
    """Byte offsets of the per-core packed input blob:
    h bf16 [P,64] | a f16 [P,16] | src idx i16 [16, nblk*8] | off u8
    [128, nblk] | wshard f32 [WSH]. (dst gather indices are derived on
    device from off + the static group structure.)"""
    nblk = nbt * NGROUP
    o_a = P * 64 * 2
    o_idx = o_a + P * 16 * 2
    o_off = o_idx + 16 * (nblk * 8) * 2
    o_w = (o_off + 128 * nblk + 3) // 4 * 4
    total = o_w + WSH * 4
    return o_a, o_idx, o_off, o_w, total


def wrap_idx(idx):
    """int array [n] (n % 16 == 0) -> int16 [16, n//16] wrap (compact form;
    replicated to the 8 gpsimd stripes on device)."""
    idx = np.asarray(idx, np.int16)
    return idx.reshape(-1, 16).T.copy()


def prep_edges(edge_index):
    """Shard + sort + block the edge list.

    Returns (idx [NCORES, 16, nblk*8] i16 (src gather wrap),
             off [NCORES, 128, nblk] u8, nbt)."""
    src = np.asarray(edge_index[0], np.int64)
    dst = np.asarray(edge_index[1], np.int64)
    loops = np.arange(N, dtype=np.int64)
    src = np.concatenate([src, loops])
    dst = np.concatenate([dst, loops])

    per_core = []
    max_blocks = 0
    for c in range(NCORES):
        m = (dst // P) == c
        s, d = src[m], dst[m] - c * P
        order = np.argsort(d, kind="stable")
        s, d = s[order], d[order]
        g = d // GROUP
        cnt = np.bincount(g, minlength=NGROUP)
        nb = (cnt + 127) // 128
        max_blocks = max(max_blocks, int(nb.max()))
        per_core.append((s, d, cnt))

    nbt = max(18, (max_blocks + 1) // 2 * 2)
    nblk = nbt * NGROUP

    src_idx = np.zeros((NCORES, nblk * 128), np.int64)
    dst_off = np.full((NCORES, nblk * 128), GROUP, np.uint8)
    for c in range(NCORES):
        s, d, cnt = per_core[c]
        pos = 0
        for grp in range(NGROUP):
            n = int(cnt[grp])
            base = grp * nbt * 128
            src_idx[c, base:base + n] = s[pos:pos + n]
            dst_off[c, base:base + n] = (d[pos:pos + n] - grp * GROUP)
            pos += n
    idx = np.stack([wrap_idx(src_idx[c]) for c in range(NCORES)])
    off = dst_off.reshape(NCORES, nblk, 128).transpose(0, 2, 1).copy()
    return idx, off, nbt


def expand_att(a):
    """a [HEADS, C] -> block matrix [HEADS*C, HEADS] so that
    (h @ A)[n, head] = sum_c h[n, head, c] * a[head, c]."""
    hh, cc = a.shape
    A = np.zeros((hh * cc, hh), np.float32)
    for h in range(hh):
        A[h * cc:(h + 1) * cc, h] = a[h]
    return A


def prep_weights(a_src2, a_dst2, b1, ln1_g, ln1_b, Wq, Wk, Wv,
                 ln2_g, ln2_b, W2, b2):
    """Constant-fold the tiny weights into one flat blob [NCORES, WSH]."""
    W2 = np.asarray(W2, np.float32)
    W2aug = np.concatenate(
        [W2, W2 @ expand_att(np.asarray(a_src2, np.float32)),
         W2 @ expand_att(np.asarray(a_dst2, np.float32))], axis=1)  # [70, 64]
    blob = np.zeros(NCORES * WSH, np.float32)
    blob[O_W2T:O_W2T + 4096] = W2aug[0:64].reshape(-1)
    blob[O_W2B:O_W2B + 384] = W2aug[64:70].reshape(-1)
    blob[O_WQ:O_WQ + 4096] = np.asarray(Wq, np.float32).reshape(-1)
    blob[O_WK:O_WK + 4096] = np.asarray(Wk, np.float32).reshape(-1)
    blob[O_WV:O_WV + 384] = np.asarray(Wv, np.float32).reshape(-1)
    vec = np.concatenate([
        np.asarray(b1, np.float32).reshape(-1),
        np.asarray(ln1_g, np.float32).reshape(-1),
        np.asarray(ln1_b, np.float32).reshape(-1),
        np.asarray(ln2_g, np.float32).reshape(-1),
        np.asarray(ln2_b, np.float32).reshape(-1),
        np.asarray(b2, np.float32).reshape(-1)])
    blob[O_VEC:O_VEC + 210] = vec
    return blob.reshape(NCORES, WSH)


f32 = mybir.dt.float32
f32r = mybir.dt.float32r
bf16 = mybir.dt.bfloat16
f16 = mybir.dt.float16
i16 = mybir.dt.int16
i32 = mybir.dt.int32
u8 = mybir.dt.uint8
AF = mybir.ActivationFunctionType
OP = mybir.AluOpType


def bc(ap, shape):
    return ap.broadcast_to(tuple(shape))


def build_kernel(nbt):
    nblk = nbt * NGROUP
    CB = 2 * nbt              # blocks per dst-tile chunk (2 groups)
    nc = bacc.Bacc("TRN2", target_bir_lowering=False, debug=False,
                   num_devices=NCORES)

    # ---------------- DRAM I/O ----------------
    o_a, o_idx, o_off, o_w, b_bytes = blob_layout(nbt)
    blob_d = nc.dram_tensor("blob", [1, b_bytes], u8, kind="ExternalInput")
    bv_bf = blob_d.ap().bitcast(bf16).rearrange("a b -> (a b)")
    bv_f16 = blob_d.ap().bitcast(f16).rearrange("a b -> (a b)")
    bv_i16 = blob_d.ap().bitcast(i16).rearrange("a b -> (a b)")
    bv_u8 = blob_d.ap().rearrange("a b -> (a b)")
    bv_f32 = blob_d.ap().bitcast(f32).rearrange("a b -> (a b)")

    out_d = nc.dram_tensor("out", [P, 6], f32, kind="ExternalOutput")

    wfull_d = nc.dram_tensor("wfull", [NCORES, WSH], f32, addr_space="Shared")
    t1_loc = nc.dram_tensor("t1_loc", [P, 64], f32)
    t1_full = nc.dram_tensor("t1_full", [N, 64], f32, addr_space="Shared")
    t1a_loc = nc.dram_tensor("t1a_loc", [P + 128, 64], f32)
    t2_loc = nc.dram_tensor("t2_loc", [P, 64], f32)
    t2_full = nc.dram_tensor("t2_full", [N, 64], f32, addr_space="Shared")
    t2a_loc = nc.dram_tensor("t2a_loc", [P + 128, 64], f32)
    ht_loc = nc.dram_tensor("ht_loc", [64, P], f32)
    ht_ag = nc.dram_tensor("ht_ag", [64 * NCORES, P], f32, addr_space="Shared")

    with tile.TileContext(nc) as tc, ExitStack() as top:
        # ---------------- persistent SBUF ----------------
        pers = top.enter_context(tc.tile_pool(name="pers", bufs=1))

        def ptile(name, shape, dtype):
            return pers.tile(shape, dtype, name=name, tag=name)

        # weights: AllGather the sharded blob, then unpack.
        # (collectives cannot read IO tensors -> bounce via internal DRAM)
        wstage_d = nc.dram_tensor("wstage", [1, WSH], f32)
        nc.sync.dma_start(
            wstage_d.ap()[:],
            bv_f32[o_w // 4:o_w // 4 + WSH].rearrange("(a b) -> a b", a=1))
        nc.gpsimd.collective_compute(
            "AllGather", OP.bypass, replica_groups=[list(range(NCORES))],
            ins=[wstage_d.ap()[:]], outs=[wfull_d.ap()[:]])
        flat = wfull_d.ap().rearrange("a b -> (a b)")

        w2top = ptile("w2top", [64, 64], f32)
        nc.sync.dma_start(w2top[:],
                          flat[O_W2T:O_W2T + 4096].rearrange("(p d) -> p d", d=64))
        w2bot = ptile("w2bot", [6, 64], f32)
        nc.sync.dma_start(w2bot[:],
                          flat[O_W2B:O_W2B + 384].rearrange("(p d) -> p d", d=64))
        wq = ptile("wq", [64, 64], f32)
        nc.sync.dma_start(wq[:],
                          flat[O_WQ:O_WQ + 4096].rearrange("(p d) -> p d", d=64))
        wk = ptile("wk", [64, 64], f32)
        nc.sync.dma_start(wk[:],
                          flat[O_WK:O_WK + 4096].rearrange("(p d) -> p d", d=64))
        wv7 = ptile("wv7", [64, 7], f32)
        nc.gpsimd.memset(wv7[:], 0.0)
        nc.sync.dma_start(wv7[:, 0:6],
                          flat[O_WV:O_WV + 384].rearrange("(p d) -> p d", d=6))
        vec210 = ptile("vec210", [1, 210], f32)
        nc.sync.dma_start(vec210[:],
                          flat[O_VEC:O_VEC + 210].rearrange("(a b) -> a b", a=1))
        ones1 = ptile("ones1", [1, 128], f32)
        nc.gpsimd.memset(ones1[:], 1.0)
        rep210 = ptile("rep210", [128, 210], f32)
        with tc.tile_pool(name="sps", bufs=1, space="PSUM") as sps:
            rp = sps.tile([128, 210], f32, tag="rp")
            nc.tensor.matmul(rp[:], ones1[:], vec210[:], start=True, stop=True)
            nc.scalar.activation(rep210[:], rp[:], AF.Copy)
        b1r = rep210[:][:, 0:64]
        l1g = rep210[:][:, 64:128]
        l1b = rep210[:][:, 128:192]
        l2g = rep210[:][:, 192:198]
        l2b = rep210[:][:, 198:204]
        b2r = rep210[:][:, 204:210]

        # identity matrix via iota + is_equal
        coli = ptile("coli", [128, 128], i32)
        nc.gpsimd.iota(coli[:], pattern=[[1, 128]], base=0, channel_multiplier=0)
        rowi = ptile("rowi", [128, 1], i32)
        nc.gpsimd.iota(rowi[:], pattern=[[1, 1]], base=0, channel_multiplier=1)
        colf = ptile("colf", [128, 128], f32)
        nc.vector.tensor_copy(colf[:], coli[:])
        rowf = ptile("rowf", [128, 1], f32)
        nc.vector.tensor_copy(rowf[:], rowi[:])
        eye = ptile("eye", [128, 128], f32)
        nc.vector.tensor_tensor(eye[:], colf[:], bc(rowf[:], (128, 128)),
                                OP.is_equal)

        iotaf = ptile("iotaf", [128, GROUP], f32)
        ioi = ptile("ioi", [128, GROUP], i32)
        nc.gpsimd.iota(ioi[:], pattern=[[1, GROUP]], base=0, channel_multiplier=0)
        nc.vector.tensor_copy(iotaf[:], ioi[:])

        # src gather indices: replicate compact 16-row wrap to 8 stripes
        idxv = bv_i16[o_idx // 2:o_idx // 2 + 16 * nblk * 8].rearrange(
            "(r c) -> r c", r=16)
        src_it = ptile("src_it", [128, nblk * 8], i16)
        for g in range(8):
            nc.sync.dma_start(src_it[:][16 * g:16 * (g + 1), :], idxv[:, :])
        offu = ptile("offu", [128, nblk], u8)
        nc.sync.dma_start(offu[:],
                          bv_u8[o_off:o_off + 128 * nblk].rearrange(
                              "(p c) -> p c", p=128))
        dst_off = ptile("dst_off", [128, nblk], f32)
        nc.vector.tensor_copy(dst_off[:], offu[:])

        # dst gather indices (local node = group*64 + off), derived on
        # device. gpsimd stripe g reads, for block j, only column j*8+g of
        # its 16 partitions; other columns are zeroed (never read).
        base16 = ptile("base16", [128, NGROUP], i32)
        nc.gpsimd.iota(base16[:], pattern=[[GROUP, NGROUP]], base=0,
                       channel_multiplier=0)
        basef = ptile("basef", [128, NGROUP], f32)
        nc.vector.tensor_copy(basef[:], base16[:])
        dstlocf = ptile("dstlocf", [128, nblk], f32)
        nc.vector.tensor_tensor(
            dstlocf[:].rearrange("p (g b) -> p g b", g=NGROUP),
            dst_off[:].rearrange("p (g b) -> p g b", g=NGROUP),
            bc(basef[:].rearrange("p (g a) -> p g a", a=1),
               (128, NGROUP, nbt)), OP.add)
        # wrap layout: slot [p, j*8+k] must hold dstloc[16k + p%16, j] -- a
        # partition interleave. DVE can't shuffle partitions, DMA can:
        # write the 8 bands interleaved to a DRAM row-block, replicate back.
        dstloci = ptile("dstloci", [128, nblk], i16)
        nc.vector.tensor_copy(dstloci[:], dstlocf[:])
        band_d = nc.dram_tensor("band", [16, nblk * 8], i16)
        bvw = band_d.ap().rearrange("r (j e) -> r j e", e=8)
        for k in range(8):
            nc.sync.dma_start(bvw[:, :, k:k + 1],
                              dstloci[:][16 * k:16 * (k + 1), :]
                              .rearrange("p (j a) -> p j a", a=1))
        dst_it = ptile("dst_it", [128, nblk * 8], i16)
        for g in range(8):
            nc.sync.dma_start(dst_it[:][16 * g:16 * (g + 1), :],
                              band_d.ap()[:, :])

        epsc = ptile("epsc", [128, 1], f32)
        nc.gpsimd.memset(epsc[:], LN_EPS)
        sel = ptile("sel", [128, nblk * GROUP], bf16)
        hlnT = ptile("hlnT", [64, N], f32)        # full h_ln^T after AG
        hT_loc_sb = ptile("hT_loc_sb", [64, P], f32)
        gat1 = ptile("gat1", [128, 8 * 72], f32)
        gat2 = ptile("gat2", [128, 8 * 56], f32)
        hln_rows = ptile("hln_rows", [128, 8 * 64], f32)
        htln = ptile("htln", [128, 8 * 6], f32)

        def rows_to_dram(dram, sb_view, ncols, col0=0, cast=None):
            """sb_view [128, 8, w] -> dram rows [(t*128+p), col0:col0+w]."""
            dv = dram.ap()
            if cast is not None:
                dv = dv.bitcast(cast)
            dv = dv.rearrange("(t p) d -> p t d", p=128)
            nc.sync.dma_start(dv[:, :, col0:col0 + ncols], sb_view)

        # ================= P0: host-projected h1a -> T1 =================
        with ExitStack() as ctx:
            pool = ctx.enter_context(tc.tile_pool(name="p0", bufs=2))
            hrows = pool.tile([128, 8 * 64], bf16, tag="hrows")
            nc.sync.dma_start(
                hrows[:].rearrange("p (t d) -> p t d", t=8),
                bv_bf[0:P * 64].rearrange("(t p d) -> p t d", t=8, p=128, d=64))
            a16 = pool.tile([128, 8 * 16], f16, tag="a16")
            nc.sync.dma_start(
                a16[:].rearrange("p (t d) -> p t d", t=8),
                bv_f16[o_a // 2:o_a // 2 + P * 16].rearrange(
                    "(t p d) -> p t d", t=8, p=128, d=16))
            arows = pool.tile([128, 8 * 16], f32, tag="arows")
            nc.vector.tensor_copy(arows[:], a16[:])
            ar3 = arows[:].rearrange("p (t d) -> p t d", t=8)
            rows_to_dram(t1_loc,
                         hrows[:].rearrange("p (t d) -> p t d", t=8)[:, :, :],
                         64, col0=0, cast=bf16)
            rows_to_dram(t1_loc, ar3[:, :, 0:8], 8, col0=32)
            # local adst table for the (small) dst gather; pad rows zeroed
            zero8 = pool.tile([128, 8], f32, tag="zero8")
            nc.gpsimd.memset(zero8[:], 0.0)
            t1av = t1a_loc.ap().rearrange("(t p) d -> p t d", p=128)
            nc.sync.dma_start(t1av[:, 0:8, 0:8], ar3[:, :, 8:16])
            nc.sync.dma_start(t1av[:, 8:9, 0:8],
                              zero8[:].rearrange("p (a d) -> p a d", a=1))
            nc.gpsimd.collective_compute(
                "AllGather", OP.bypass, replica_groups=[list(range(NCORES))],
                ins=[t1_loc.ap()[:]], outs=[t1_full.ap()[:]])

        # ============ edge phase (shared by both layers) ============
        def edge_phase(table, atable, it_src, it_dst, hcols, acol, gatacc,
                       build_sel):
            # hcols: # bf16 feature cols; acol: f32 col of asrc in src rows
            with ExitStack() as ctx:
                gp = ctx.enter_context(tc.tile_pool(name="gp", bufs=2))
                sp = ctx.enter_context(tc.tile_pool(name="sp", bufs=2))
                pg = ctx.enter_context(tc.tile_pool(name="pg", bufs=4,
                                                    space="PSUM"))
                for t in range(8):
                    j0 = t * CB
                    gs = gp.tile([128, CB * 64], f32, tag="gs")
                    nc.gpsimd.dma_gather(
                        gs[:].rearrange("p (j e) -> p j e", e=64),
                        table.ap()[:], it_src[:, j0 * 8:(j0 + CB) * 8],
                        num_idxs=CB * 128, num_idxs_reg=CB * 128, elem_size=64,
                        single_packet=False)
                    gd = gp.tile([128, CB * 64], f32, tag="gd")
                    nc.gpsimd.dma_gather(
                        gd[:].rearrange("p (j e) -> p j e", e=64),
                        atable.ap()[:], it_dst[:, j0 * 8:(j0 + CB) * 8],
                        num_idxs=CB * 128, num_idxs_reg=CB * 128, elem_size=64,
                        single_packet=False)
                    gs3 = gs[:].rearrange("p (j e) -> p j e", e=64)
                    gd3 = gd[:].rearrange("p (j e) -> p j e", e=64)
                    z = sp.tile([128, CB * 8], f32, tag="z")
                    z3 = z[:].rearrange("p (j e) -> p j e", e=8)
                    nc.vector.tensor_tensor(z3, gs3[:, :, acol:acol + 8],
                                            gd3[:, :, 0:8], OP.add)
                    u = sp.tile([128, CB * 8], f32, tag="u")
                    nc.vector.tensor_scalar_mul(u[:], z[:], 0.2)
                    nc.vector.tensor_max(z[:], z[:], u[:])
                    exf = sp.tile([128, CB * 8], f32, tag="exf")
                    nc.scalar.activation(exf[:], z[:], AF.Exp)
                    exb = sp.tile([128, CB * 8], bf16, tag="exb")
                    nc.vector.tensor_copy(exb[:], exf[:])
                    exb3 = exb[:].rearrange("p (j e) -> p j e", e=8)
                    W = hcols + 8
                    msgs = sp.tile([128, CB * W], bf16, tag="msgs")
                    m3 = msgs[:].rearrange("p (j e) -> p j e", e=W)
                    hb = gs3.bitcast(bf16)  # [128, CB, 128] bf16
                    exb4 = exb3.rearrange("p j (h a) -> p j h a", a=1)
                    nc.vector.tensor_tensor(
                        m3[:, :, 0:hcols].rearrange("p j (h c) -> p j h c", h=8),
                        hb[:, :, 0:hcols].rearrange("p j (h c) -> p j h c", h=8),
                        bc(exb4, (128, CB, 8, hcols // 8)), OP.mult)
                    nc.vector.tensor_copy(m3[:, :, hcols:W], exb3)
                    if build_sel:
                        sel3 = sel[:].rearrange("p (j e) -> p j e", e=GROUP)
                        io_b = bc(iotaf[:].rearrange("p (a e) -> p a e", a=1),
                                  (128, CB, GROUP))
                        do_b = bc(dst_off[:, j0:j0 + CB]
                                  .rearrange("p (j a) -> p j a", a=1),
                                  (128, CB, GROUP))
                        nc.vector.tensor_tensor(sel3[:, j0:j0 + CB, :], io_b,
                                                do_b, OP.is_equal)
                    sel3 = sel[:].rearrange("p (j e) -> p j e", e=GROUP)
                    ga = gatacc[:].rearrange("p (t d) -> p t d", d=W)
                    for g in (0, 1):
                        pgt = pg.tile([64, W], f32, tag="pgt")
                        for b in range(nbt):
                            jj = (2 * t + g) * nbt + b
                            nc.tensor.matmul(
                                pgt[:], sel3[:, jj, :], m3[:, jj - j0, :],
                                start=(b == 0), stop=(b == nbt - 1))
                        nc.scalar.activation(
                            ga[64 * g:64 * (g + 1), t, :], pgt[:], AF.Copy)

        edge_phase(t1_full, t1a_loc, src_it[:], dst_it[:], 64, 32, gat1,
                   build_sel=True)

        # ============ P2: GAT1 -> h_ln ============
        with ExitStack() as ctx:
            sp = ctx.enter_context(tc.tile_pool(name="p2", bufs=2))
            g3 = gat1[:].rearrange("p (t d) -> p t d", d=72)
            rec = sp.tile([128, 8 * 8], f32, tag="rec")
            nc.vector.reciprocal(rec[:].rearrange("p (t h) -> p t h", h=8),
                                 g3[:, :, 64:72])
            h1 = hln_rows[:].rearrange("p (t d) -> p t d", d=64)
            rec4 = rec[:].rearrange("p (t h a) -> p t h a", t=8, h=8)
            nc.vector.tensor_tensor(
                h1.rearrange("p t (h c) -> p t h c", h=8),
                g3[:, :, 0:64].rearrange("p t (h c) -> p t h c", h=8),
                bc(rec4, (128, 8, 8, 8)), OP.mult)
            b1b = bc(b1r.rearrange("p (a d) -> p a d", a=1), (128, 8, 64))
            nc.vector.tensor_tensor(h1, h1, b1b, OP.add)
            # layernorm over 64
            rs_ = sp.tile([128, 8], f32, tag="rs_")
            nc.vector.tensor_reduce(rs_[:], h1, mybir.AxisListType.X, OP.add)
            mean = sp.tile([128, 8], f32, tag="mean")
            nc.scalar.mul(mean[:], rs_[:], 1.0 / 64)
            nc.vector.tensor_tensor(
                h1, h1, bc(mean[:].rearrange("p (t a) -> p t a", a=1),
                           (128, 8, 64)), OP.subtract)
            sq = sp.tile([128, 8 * 64], f32, tag="sq")
            ssum = sp.tile([128, 8], f32, tag="ssum")
            sq3 = sq[:].rearrange("p (t d) -> p t d", d=64)
            nc.scalar.activation(sq3, h1, AF.Square)
            nc.vector.tensor_reduce(ssum[:], sq3, mybir.AxisListType.X, OP.add)
            std_ = sp.tile([128, 8], f32, tag="std_")
            nc.scalar.activation(std_[:], ssum[:], AF.Sqrt, bias=epsc[:],
                                 scale=1.0 / 64)
            rstd = sp.tile([128, 8], f32, tag="rstd")
            nc.vector.reciprocal(rstd[:], std_[:])
            nc.vector.tensor_tensor(
                h1, h1, bc(rstd[:].rearrange("p (t a) -> p t a", a=1),
                           (128, 8, 64)), OP.mult)
            nc.vector.tensor_tensor(
                h1, h1, bc(l1g.rearrange("p (a d) -> p a d", a=1),
                           (128, 8, 64)), OP.mult)
            nc.vector.tensor_tensor(
                h1, h1, bc(l1b.rearrange("p (a d) -> p a d", a=1),
                           (128, 8, 64)), OP.add)
            # elu
            mn = sp.tile([128, 8 * 64], f32, tag="mn")
            nc.vector.tensor_scalar_min(mn[:], hln_rows[:], 0.0)
            ee = sp.tile([128, 8 * 64], f32, tag="ee")
            nc.scalar.activation(ee[:], mn[:], AF.Exp)
            nc.vector.tensor_scalar_max(hln_rows[:], hln_rows[:], 0.0)
            nc.vector.tensor_add(hln_rows[:], hln_rows[:], ee[:])
            nc.vector.tensor_scalar_add(hln_rows[:], hln_rows[:], -1.0)

        # ============ P3: transpose + AG h_ln^T ============
        with ExitStack() as ctx:
            ps = ctx.enter_context(tc.tile_pool(name="p3ps", bufs=3,
                                                space="PSUM"))
            hr = hln_rows[:].rearrange("p (t d) -> p t d", d=64)
            for m in range(8):
                pt = ps.tile([64, 128], f32, tag="pt")
                nc.tensor.transpose(pt[:], hr[:, m, :], eye[:])
                nc.vector.tensor_copy(hT_loc_sb[:, m * 128:(m + 1) * 128], pt[:])
            nc.sync.dma_start(ht_loc.ap()[:], hT_loc_sb[:])
            nc.gpsimd.collective_compute(
                "AllGather", OP.bypass, replica_groups=[list(range(NCORES))],
                ins=[ht_loc.ap()[:]], outs=[ht_ag.ap()[:]])
            for c in range(NCORES):
                nc.sync.dma_start(hlnT[:, c * P:(c + 1) * P],
                                  ht_ag.ap()[c * 64:(c + 1) * 64, :])

        # ============ P4: attention ============
        with ExitStack() as ctx:
            pool = ctx.enter_context(tc.tile_pool(name="p4", bufs=2))
            ps = ctx.enter_context(tc.tile_pool(name="p4ps", bufs=2,
                                                space="PSUM"))
            pvps = ctx.enter_context(tc.tile_pool(name="pvps", bufs=1,
                                                  space="PSUM"))
            kT = pers.tile([64, N], f32r, name="kT", tag="kT")
            qT = pers.tile([64, P], f32r, name="qT", tag="qT")
            vaug = pers.tile([128, 64 * 7], bf16, name="vaug", tag="vaug")
            for j in range(16):
                pk = ps.tile([64, 512], f32, tag="pss")
                nc.tensor.matmul(pk[:], wk[:], hlnT[:, j * 512:(j + 1) * 512],
                                 start=True, stop=True)
                nc.vector.tensor_copy(kT[:, j * 512:(j + 1) * 512], pk[:])
            for j in range(2):
                pq = ps.tile([64, 512], f32, tag="pss")
                nc.tensor.matmul(pq[:], wq[:],
                                 hT_loc_sb[:, j * 512:(j + 1) * 512],
                                 start=True, stop=True)
                nc.vector.tensor_copy(qT[:, j * 512:(j + 1) * 512], pq[:])
            va3 = vaug[:].rearrange("p (n d) -> p n d", d=7)
            for nt in range(64):
                pv = ps.tile([128, 7], f32, tag="pss")
                nc.tensor.matmul(pv[:], hlnT[:, nt * 128:(nt + 1) * 128],
                                 wv7[:], start=True, stop=True)
                nc.vector.tensor_copy(va3[:, nt, :], pv[:])
            nc.gpsimd.memset(va3[:, :, 6:7], 1.0)

            NTB = 3  # n-tiles per psum batch (3 banks)
            att = pool.tile([128, 8 * 7], f32, tag="att")
            at3 = att[:].rearrange("p (t d) -> p t d", d=7)
            for mc in range(2):
                po = pvps.tile([7, 512], f32, tag="po")
                nb_list = [(s, min(s + NTB, 64)) for s in range(0, 64, NTB)]
                for (s0, s1) in nb_list:
                    w = (s1 - s0) * 512
                    pss = ps.tile([128, NTB * 512], f32, tag="pss")
                    for i, nt in enumerate(range(s0, s1)):
                        nc.tensor.matmul(
                            pss[:, i * 512:(i + 1) * 512],
                            kT[:, nt * 128:(nt + 1) * 128],
                            qT[:, mc * 512:(mc + 1) * 512],
                            start=True, stop=True)
                    pT = pool.tile([128, NTB * 512], bf16, tag="pT")
                    nc.scalar.activation(pT[:, 0:w], pss[:, 0:w], AF.Exp,
                                         scale=0.125)
                    for i, nt in enumerate(range(s0, s1)):
                        nc.tensor.matmul(
                            po[:], va3[:, nt, :].bitcast(bf16),
                            pT[:, i * 512:(i + 1) * 512],
                            start=(nt == 0), stop=(nt == 63),
                            skip_group_check=True)
                spo = pool.tile([7, 512], f32, tag="spo")
                nc.vector.tensor_copy(spo[:], po[:])
                for i in range(4):
                    ptr = ps.tile([128, 7], f32, tag="pss")
                    nc.tensor.transpose(ptr[:], spo[:, i * 128:(i + 1) * 128],
                                        eye[0:7, 0:7])
                    nc.vector.tensor_copy(at3[:, mc * 4 + i, :], ptr[:])
            # normalize + LN over 6
            rec = pool.tile([128, 8], f32, tag="reca")
            nc.vector.reciprocal(rec[:].rearrange("p (t a) -> p t a", a=1),
                                 at3[:, :, 6:7])
            ht3 = htln[:].rearrange("p (t d) -> p t d", d=6)
            nc.vector.tensor_tensor(
                ht3, at3[:, :, 0:6],
                bc(rec[:].rearrange("p (t a) -> p t a", a=1), (128, 8, 6)),
                OP.mult)
            rs_ = pool.tile([128, 8], f32, tag="rsb")
            nc.vector.tensor_reduce(rs_[:], ht3, mybir.AxisListType.X, OP.add)
            mean = pool.tile([128, 8], f32, tag="meanb")
            nc.scalar.mul(mean[:], rs_[:], 1.0 / 6)
            nc.vector.tensor_tensor(
                ht3, ht3, bc(mean[:].rearrange("p (t a) -> p t a", a=1),
                             (128, 8, 6)), OP.subtract)
            sq = pool.tile([128, 8 * 6], f32, tag="sqb")
            ssum = pool.tile([128, 8], f32, tag="ssumb")
            sq3b = sq[:].rearrange("p (t d) -> p t d", d=6)
            nc.scalar.activation(sq3b, ht3, AF.Square)
            nc.vector.tensor_reduce(ssum[:], sq3b, mybir.AxisListType.X, OP.add)
            stdb = pool.tile([128, 8], f32, tag="stdb")
            nc.scalar.activation(stdb[:], ssum[:], AF.Sqrt, bias=epsc[:],
                                 scale=1.0 / 6)
            rstd = pool.tile([128, 8], f32, tag="rstdb")
            nc.vector.reciprocal(rstd[:], stdb[:])
            nc.vector.tensor_tensor(
                ht3, ht3, bc(rstd[:].rearrange("p (t a) -> p t a", a=1),
                             (128, 8, 6)), OP.mult)
            nc.vector.tensor_tensor(
                ht3, ht3, bc(l2g.rearrange("p (a d) -> p a d", a=1),
                             (128, 8, 6)), OP.mult)
            nc.vector.tensor_tensor(
                ht3, ht3, bc(l2b.rearrange("p (a d) -> p a d", a=1),
                             (128, 8, 6)), OP.add)

        # ============ P5: T2 build + AG ============
        with ExitStack() as ctx:
            pool = ctx.enter_context(tc.tile_pool(name="p5", bufs=3))
            ps = ctx.enter_context(tc.tile_pool(name="p5ps", bufs=3,
                                                space="PSUM"))
            htT = pool.tile([6, P], f32, tag="htT")
            ht3 = htln[:].rearrange("p (t d) -> p t d", d=6)
            for m in range(8):
                pt = ps.tile([6, 128], f32, tag="pt2")
                nc.tensor.transpose(pt[:], ht3[:, m, :], eye[:])
                nc.vector.tensor_copy(htT[:, m * 128:(m + 1) * 128], pt[:])
            h2a = pool.tile([128, 8 * 64], f32, tag="h2a")
            h2b = pool.tile([128, 8 * 48], bf16, tag="h2b")
            h2a3 = h2a[:].rearrange("p (t d) -> p t d", d=64)
            h2b3 = h2b[:].rearrange("p (t d) -> p t d", d=48)
            for m in range(8):
                pm = ps.tile([128, 64], f32, tag="pm2")
                nc.tensor.matmul(pm[:], hT_loc_sb[:, m * 128:(m + 1) * 128],
                                 w2top[:], start=True, stop=False)
                nc.tensor.matmul(pm[:], htT[:, m * 128:(m + 1) * 128],
                                 w2bot[:], start=False, stop=True)
                nc.scalar.activation(h2a3[:, m, :], pm[:], AF.Copy)
                nc.vector.tensor_copy(h2b3[:, m, :], pm[:, 0:48])
            rows_to_dram(t2_loc, h2b3[:, :, :], 48, col0=0, cast=bf16)
            rows_to_dram(t2_loc, h2a3[:, :, 48:56], 8, col0=24)
            zero8b = pool.tile([128, 8], f32, tag="zero8b")
            nc.gpsimd.memset(zero8b[:], 0.0)
            t2av = t2a_loc.ap().rearrange("(t p) d -> p t d", p=128)
            nc.sync.dma_start(t2av[:, 0:8, 0:8], h2a3[:, :, 56:64])
            nc.sync.dma_start(t2av[:, 8:9, 0:8],
                              zero8b[:].rearrange("p (a d) -> p a d", a=1))
            nc.gpsimd.collective_compute(
                "AllGather", OP.bypass, replica_groups=[list(range(NCORES))],
                ins=[t2_loc.ap()[:]], outs=[t2_full.ap()[:]])

        # ============ P6: GAT2 edge phase ============
        edge_phase(t2_full, t2a_loc, src_it[:], dst_it[:], 48, 24, gat2,
                   build_sel=False)

        # ============ P7: finale ============
        with ExitStack() as ctx:
            sp = ctx.enter_context(tc.tile_pool(name="p7", bufs=2))
            g3 = gat2[:].rearrange("p (t d) -> p t d", d=56)
            d8 = sp.tile([128, 8 * 8], f32, tag="d8")
            nc.vector.tensor_scalar_mul(d8[:].rearrange("p (t h) -> p t h", h=8),
                                        g3[:, :, 48:56], 8.0)
            rec = sp.tile([128, 8 * 8], f32, tag="rec2")
            nc.vector.reciprocal(rec[:], d8[:])
            avg = sp.tile([128, 8 * 48], f32, tag="avg")
            a4 = avg[:].rearrange("p (t h c) -> p t h c", t=8, h=8)
            rec4 = rec[:].rearrange("p (t h a) -> p t h a", t=8, h=8)
            nc.vector.tensor_tensor(
                a4, g3[:, :, 0:48].rearrange("p t (h c) -> p t h c", h=8),
                bc(rec4, (128, 8, 8, 6)), OP.mult)
            swp = sp.tile([128, 8 * 48], f32, tag="swp")
            s4 = swp[:].rearrange("p (t c h) -> p t c h", t=8, c=6)
            nc.vector.tensor_copy(
                s4, avg[:].rearrange("p (t h c) -> p t h c", t=8, h=8)
                .rearrange("p t h c -> p t c h"))
            out2 = sp.tile([128, 8 * 6], f32, tag="out2")
            o3 = out2[:].rearrange("p (t d) -> p t d", d=6)
            nc.vector.tensor_reduce(o3, s4, mybir.AxisListType.X, OP.add)
            nc.vector.tensor_tensor(
                o3, o3, bc(b2r.rearrange("p (a d) -> p a d", a=1),
                           (128, 8, 6)), OP.add)
            ex = sp.tile([128, 8 * 6], f32, tag="exo")
            es = sp.tile([128, 8], f32, tag="eso")
            ex3 = ex[:].rearrange("p (t d) -> p t d", d=6)
            nc.scalar.activation(ex3, o3, AF.Exp)
            nc.vector.tensor_reduce(es[:], ex3, mybir.AxisListType.X, OP.add)
            ls = sp.tile([128, 8], f32, tag="lso")
            nc.scalar.activation(ls[:], es[:], AF.Ln)
            nc.vector.tensor_tensor(
                o3, o3, bc(ls[:].rearrange("p (t a) -> p t a", a=1),
                           (128, 8, 6)), OP.subtract)
            rows_to_dram(out_d, o3[:, :, :], 6)

    nc.compile()
    return nc


# ---------------- dispatch layer (cached jit over PJRT) ----------------

_SESS = {}


def _get_session(nbt):
    if nbt in _SESS:
        return _SESS[nbt]
    nc = build_kernel(nbt)
    bass2jax.install_neuronx_cc_hook()
    partition_name = (nc.partition_id_tensor.name
                      if nc.partition_id_tensor else None)
    in_names, out_names, out_avals = [], [], []
    for alloc in nc.m.functions[0].allocations:
        if not isinstance(alloc, mybir.MemoryLocationSet):
            continue
        name = alloc.memorylocations[0].name
        if alloc.kind == "ExternalInput":
            if name != partition_name:
                in_names.append(name)
        elif alloc.kind == "ExternalOutput":
            out_names.append(name)
            out_avals.append(jax.core.ShapedArray(
                tuple(alloc.tensor_shape), mybir.dt.np(alloc.dtype)))
    n_params = len(in_names)
    n_outs = len(out_avals)
    all_names = list(in_names) + list(out_names)
    if partition_name is not None:
        all_names.append(partition_name)

    def _body(*args):
        operands = list(args)
        if partition_name is not None:
            operands.append(bass2jax.partition_id_tensor())
        outs = bass2jax._bass_exec_p.bind(
            *operands,
            out_avals=tuple(out_avals),
            in_names=tuple(all_names),
            out_names=tuple(out_names),
            lowering_input_output_aliases=(),
            sim_require_finite=True,
            sim_require_nnan=True,
            nc=nc,
        )
        return tuple(outs)

    devices = jax.devices()[:NCORES]
    mesh = Mesh(np.asarray(devices), ("core",))
    sharding = NamedSharding(mesh, PartitionSpec("core"))
    sharded = jax.jit(
        shard_map(_body, mesh=mesh,
                  in_specs=(PartitionSpec("core"),) * (n_params + n_outs),
                  out_specs=(PartitionSpec("core"),) * n_outs,
                  check_rep=False),
        donate_argnums=tuple(range(n_params, n_params + n_outs)),
        keep_unused=True)
    zeros_fns = [
        jax.jit(partial(jnp.zeros,
                        (NCORES * a.shape[0], *a.shape[1:]), a.dtype),
                out_shardings=sharding)
        for a in out_avals]
    sess = dict(sharded=sharded, in_names=in_names, out_names=out_names,
                zeros_fns=zeros_fns)
    _SESS[nbt] = sess
    return sess


def _prepare_arrays(inputs):
    """Host prep: full np inputs -> (concat input arrays by name, nbt)."""
    x = np.asarray(inputs["x"], np.float32)
    W1 = np.asarray(inputs["W1"], np.float32)
    W1aug = np.concatenate(
        [W1, W1 @ expand_att(np.asarray(inputs["a_src1"], np.float32)),
         W1 @ expand_att(np.asarray(inputs["a_dst1"], np.float32))],
        axis=1)                                          # [256, 80]
    h1a = x @ W1aug                                      # [N, 80] f32
    h_b = np.ascontiguousarray(h1a[:, 0:64]).astype(BF16)
    a_h = np.ascontiguousarray(h1a[:, 64:80]).astype(np.float16)
    idx, off, nbt = prep_edges(np.asarray(inputs["edge_index"]))
    wblob = prep_weights(
        inputs["a_src2"], inputs["a_dst2"], inputs["b1"],
        inputs["ln1_g"], inputs["ln1_b"], inputs["Wq"], inputs["Wk"],
        inputs["Wv"], inputs["ln2_g"], inputs["ln2_b"], inputs["W2"],
        inputs["b2"])
    o_a, o_idx, o_off, o_w, b_bytes = blob_layout(nbt)
    blob = np.zeros((NCORES, b_bytes), np.uint8)
    for c in range(NCORES):
        for o, arr in ((0, h_b[c * P:(c + 1) * P]),
                       (o_a, a_h[c * P:(c + 1) * P]),
                       (o_idx, idx[c]), (o_off, off[c]), (o_w, wblob[c])):
            bts = np.frombuffer(arr.tobytes(), np.uint8)
            blob[c, o:o + bts.size] = bts
    return {"blob": blob}, nbt


def _run(sess, arrs):
    """Dispatch: host np inputs -> host np output [N, 6]."""
    ins = [arrs[nm] for nm in sess["in_names"]]
    zs = [zf() for zf in sess["zeros_fns"]]
    outs = sess["sharded"](*ins, *zs)
    out = np.asarray(outs[sess["out_names"].index("out")])
    return out.reshape(N, 6)


def kernel(**inputs):
    arrs, nbt = _prepare_arrays(inputs)
    last_err = None
    for attempt in range(4):
        try:
            sess = _get_session(nbt)
            out = _run(sess, arrs)
            if np.isfinite(out).all():
                return out
            last_err = RuntimeError("non-finite output")
        except Exception as e:  # transient NRT/axon failures
            last_err = e
            _SESS.pop(nbt, None)
            time.sleep(10)
    raise last_err


# revision 32
# speedup vs baseline: 9.9613x; 1.0024x over previous
"""Self-contained Trainium2 Bass kernel for nn_GAT_transformer.

kernel(**inputs) -> np.ndarray [8192, 6] (log_softmax output).

Strategy: 8-core SPMD, nodes (and incident edges grouped by dst) sharded
across cores. GAT message passing uses dma_gather from an AllGathered
per-node table, segment softmax without max subtraction, and per-128-edge
one-hot selector matmuls with a fused denominator column. The dense NxN
attention runs keys-on-partitions with row-sharded Q and AllGathered K/V.

The dispatch path is latency/bandwidth-bound over the remote PJRT tunnel,
so the host->device contract is minimized: x ships pre-transposed in
bf16, the gather index tables ship in their compact 16-row wrap form and
are replicated across gpsimd stripes on device, dst offsets ship as
uint8, and the small weights ship sharded (1/8 per core) and are
AllGathered + unpacked on device. The jitted executable is cached across
calls.
"""
import time
from contextlib import ExitStack
from functools import partial

import numpy as np
import ml_dtypes

import jax
import jax.numpy as jnp
from jax.sharding import Mesh, PartitionSpec, NamedSharding
from jax.experimental.shard_map import shard_map

import concourse.bacc as bacc
import concourse.tile as tile
from concourse import mybir, bass2jax

N = 8192
E = 262144
D_IN = 256
HEADS = 8
HID = 8
D_OUT = 6
S_MAX = 64
NEG_SLOPE = 0.2
LN_EPS = 1e-5
NCORES = 8
P = N // NCORES            # 1024 nodes per core
GROUP = 64                 # dsts per segment-matmul group
NGROUP = P // GROUP        # 16 groups per core

# weight blob layout (f32 elements, flat; W1aug folded into host-side h1a)
O_W2T = 0                  # W2aug rows 0:64  [64, 64]
O_W2B = 4096               # W2aug rows 64:70 [6, 64]
O_WQ = 4480                # Wq [64, 64]
O_WK = 8576                # Wk [64, 64]
O_WV = 12672               # Wv [64, 6]
O_VEC = 13056              # b1(64) ln1g(64) ln1b(64) ln2g(6) ln2b(6) b2(6)
W_TOT = 13266
WSH = 1664                 # per-core blob width (8 * 1664 = 13312 >= W_TOT)

BF16 = ml_dtypes.bfloat16


def blob_layout(nbt):
    """Byte offsets of the per-core packed input blob:
    h bf16 [P,64] | a f16 [P,16] | src idx i16 [16, nblk*8] | off u8
    [128, nblk] | wshard f32 [WSH]. (dst gather indices are derived on
    device from off + the static group structure.)"""
    nblk = nbt * NGROUP
    o_a = P * 64 * 2
    o_idx = o_a + P * 16 * 2
    o_off = o_idx + 16 * (nblk * 8) * 2
    o_w = (o_off + 128 * nblk + 3) // 4 * 4
    total = o_w + WSH * 4
    return o_a, o_idx, o_off, o_w, total


def wrap_idx(idx):
    """int array [n] (n % 16 == 0) -> int16 [16, n//16] wrap (compact form;
    replicated to the 8 gpsimd stripes on device)."""
    idx = np.asarray(idx, np.int16)
    return idx.reshape(-1, 16).T.copy()


def prep_edges(edge_index):
    """Shard + sort + block the edge list.

    Returns (idx [NCORES, 16, nblk*8] i16 (src gather wrap),
             off [NCORES, 128, nblk] u8, nbt)."""
    src = np.asarray(edge_index[0], np.int64)
    dst = np.asarray(edge_index[1], np.int64)
    loops = np.arange(N, dtype=np.int64)
    src = np.concatenate([src, loops])
    dst = np.concatenate([dst, loops])

    per_core = []
    max_blocks = 0
    for c in range(NCORES):
        m = (dst // P) == c
        s, d = src[m], dst[m] - c * P
        order = np.argsort(d, kind="stable")
        s, d = s[order], d[order]
        g = d // GROUP
        cnt = np.bincount(g, minlength=NGROUP)
        nb = (cnt + 127) // 128
        max_blocks = max(max_blocks, int(nb.max()))
        per_core.append((s, d, cnt))

    nbt = max(18, (max_blocks + 1) // 2 * 2)
    nblk = nbt * NGROUP

    src_idx = np.zeros((NCORES, nblk * 128), np.int64)
    dst_off = np.full((NCORES, nblk * 128), GROUP, np.uint8)
    for c in range(NCORES):
        s, d, cnt = per_core[c]
        pos = 0
        for grp in range(NGROUP):
            n = int(cnt[grp])
            base = grp * nbt * 128
            src_idx[c, base:base + n] = s[pos:pos + n]
            dst_off[c, base:base + n] = (d[pos:pos + n] - grp * GROUP)
            pos += n
    idx = np.stack([wrap_idx(src_idx[c]) for c in range(NCORES)])
    off = dst_off.reshape(NCORES, nblk, 128).transpose(0, 2, 1).copy()
    return idx, off, nbt


def expand_att(a):
    """a [HEADS, C] -> block matrix [HEADS*C, HEADS] so that
    (h @ A)[n, head] = sum_c h[n, head, c] * a[head, c]."""
    hh, cc = a.shape
    A = np.zeros((hh * cc, hh), np.float32)
    for h in range(hh):
        A[h * cc:(h + 1) * cc, h] = a[h]
    return A


def prep_weights(a_src2, a_dst2, b1, ln1_g, ln1_b, Wq, Wk, Wv,
                 ln2_g, ln2_b, W2, b2):
    """Constant-fold the tiny weights into one flat blob [NCORES, WSH]."""
    W2 = np.asarray(W2, np.float32)
    W2aug = np.concatenate(
        [W2, W2 @ expand_att(np.asarray(a_src2, np.float32)),
         W2 @ expand_att(np.asarray(a_dst2, np.float32))], axis=1)  # [70, 64]
    blob = np.zeros(NCORES * WSH, np.float32)
    blob[O_W2T:O_W2T + 4096] = W2aug[0:64].reshape(-1)
    blob[O_W2B:O_W2B + 384] = W2aug[64:70].reshape(-1)
    blob[O_WQ:O_WQ + 4096] = np.asarray(Wq, np.float32).reshape(-1)
    blob[O_WK:O_WK + 4096] = np.asarray(Wk, np.float32).reshape(-1)
    blob[O_WV:O_WV + 384] = np.asarray(Wv, np.float32).reshape(-1)
    vec = np.concatenate([
        np.asarray(b1, np.float32).reshape(-1),
        np.asarray(ln1_g, np.float32).reshape(-1),
        np.asarray(ln1_b, np.float32).reshape(-1),
        np.asarray(ln2_g, np.float32).reshape(-1),
        np.asarray(ln2_b, np.float32).reshape(-1),
        np.asarray(b2, np.float32).reshape(-1)])
    blob[O_VEC:O_VEC + 210] = vec
    return blob.reshape(NCORES, WSH)


f32 = mybir.dt.float32
f32r = mybir.dt.float32r
bf16 = mybir.dt.bfloat16
f16 = mybir.dt.float16
i16 = mybir.dt.int16
i32 = mybir.dt.int32
u8 = mybir.dt.uint8
AF = mybir.ActivationFunctionType
OP = mybir.AluOpType


def bc(ap, shape):
    return ap.broadcast_to(tuple(shape))


def build_kernel(nbt):
    nblk = nbt * NGROUP
    CB = 2 * nbt              # blocks per dst-tile chunk (2 groups)
    nc = bacc.Bacc("TRN2", target_bir_lowering=False, debug=False,
                   num_devices=NCORES)

    # ---------------- DRAM I/O ----------------
    o_a, o_idx, o_off, o_w, b_bytes = blob_layout(nbt)
    blob_d = nc.dram_tensor("blob", [1, b_bytes], u8, kind="ExternalInput")
    bv_bf = blob_d.ap().bitcast(bf16).rearrange("a b -> (a b)")
    bv_f16 = blob_d.ap().bitcast(f16).rearrange("a b -> (a b)")
    bv_i16 = blob_d.ap().bitcast(i16).rearrange("a b -> (a b)")
    bv_u8 = blob_d.ap().rearrange("a b -> (a b)")
    bv_f32 = blob_d.ap().bitcast(f32).rearrange("a b -> (a b)")

    out_d = nc.dram_tensor("out", [P, 6], f32, kind="ExternalOutput")

    wfull_d = nc.dram_tensor("wfull", [NCORES, WSH], f32, addr_space="Shared")
    t1_loc = nc.dram_tensor("t1_loc", [P, 64], f32)
    t1_full = nc.dram_tensor("t1_full", [N, 64], f32, addr_space="Shared")
    t1a_loc = nc.dram_tensor("t1a_loc", [P + 128, 64], f32)
    t2_loc = nc.dram_tensor("t2_loc", [P, 64], f32)
    t2_full = nc.dram_tensor("t2_full", [N, 64], f32, addr_space="Shared")
    t2a_loc = nc.dram_tensor("t2a_loc", [P + 128, 64], f32)
    ht_loc = nc.dram_tensor("ht_loc", [64, P], f32)
    ht_ag = nc.dram_tensor("ht_ag", [64 * NCORES, P], f32, addr_space="Shared")

    with tile.TileContext(nc) as tc, ExitStack() as top:
        # ---------------- persistent SBUF ----------------
        pers = top.enter_context(tc.tile_pool(name="pers", bufs=1))

        def ptile(name, shape, dtype):
            return pers.tile(shape, dtype, name=name, tag=name)

        # weights: AllGather the sharded blob, then unpack.
        # (collectives cannot read IO tensors -> bounce via internal DRAM)
        wstage_d = nc.dram_tensor("wstage", [1, WSH], f32)
        nc.sync.dma_start(
            wstage_d.ap()[:],
            bv_f32[o_w // 4:o_w // 4 + WSH].rearrange("(a b) -> a b", a=1))
        nc.gpsimd.collective_compute(
            "AllGather", OP.bypass, replica_groups=[list(range(NCORES))],
            ins=[wstage_d.ap()[:]], outs=[wfull_d.ap()[:]])
        flat = wfull_d.ap().rearrange("a b -> (a b)")

        w2top = ptile("w2top", [64, 64], f32)
        nc.sync.dma_start(w2top[:],
                          flat[O_W2T:O_W2T + 4096].rearrange("(p d) -> p d", d=64))
        w2bot = ptile("w2bot", [6, 64], f32)
        nc.sync.dma_start(w2bot[:],
                          flat[O_W2B:O_W2B + 384].rearrange("(p d) -> p d", d=64))
        wq = ptile("wq", [64, 64], f32)
        nc.sync.dma_start(wq[:],
                          flat[O_WQ:O_WQ + 4096].rearrange("(p d) -> p d", d=64))
        wk = ptile("wk", [64, 64], f32)
        nc.sync.dma_start(wk[:],
                          flat[O_WK:O_WK + 4096].rearrange("(p d) -> p d", d=64))
        wv7 = ptile("wv7", [64, 7], f32)
        nc.gpsimd.memset(wv7[:], 0.0)
        nc.sync.dma_start(wv7[:, 0:6],
                          flat[O_WV:O_WV + 384].rearrange("(p d) -> p d", d=6))
        vec210 = ptile("vec210", [1, 210], f32)
        nc.sync.dma_start(vec210[:],
                          flat[O_VEC:O_VEC + 210].rearrange("(a b) -> a b", a=1))
        ones1 = ptile("ones1", [1, 128], f32)
        nc.gpsimd.memset(ones1[:], 1.0)
        rep210 = ptile("rep210", [128, 210], f32)
        with tc.tile_pool(name="sps", bufs=1, space="PSUM") as sps:
            rp = sps.tile([128, 210], f32, tag="rp")
            nc.tensor.matmul(rp[:], ones1[:], vec210[:], start=True, stop=True)
            nc.scalar.activation(rep210[:], rp[:], AF.Copy)
        b1r = rep210[:][:, 0:64]
        l1g = rep210[:][:, 64:128]
        l1b = rep210[:][:, 128:192]
        l2g = rep210[:][:, 192:198]
        l2b = rep210[:][:, 198:204]
        b2r = rep210[:][:, 204:210]

        # identity matrix via iota + is_equal
        coli = ptile("coli", [128, 128], i32)
        nc.gpsimd.iota(coli[:], pattern=[[1, 128]], base=0, channel_multiplier=0)
        rowi = ptile("rowi", [128, 1], i32)
        nc.gpsimd.iota(rowi[:], pattern=[[1, 1]], base=0, channel_multiplier=1)
        colf = ptile("colf", [128, 128], f32)
        nc.vector.tensor_copy(colf[:], coli[:])
        rowf = ptile("rowf", [128, 1], f32)
        nc.vector.tensor_copy(rowf[:], rowi[:])
        eye = ptile("eye", [128, 128], f32)
        nc.vector.tensor_tensor(eye[:], colf[:], bc(rowf[:], (128, 128)),
                                OP.is_equal)

        iotaf = ptile("iotaf", [128, GROUP], f32)
        ioi = ptile("ioi", [128, GROUP], i32)
        nc.gpsimd.iota(ioi[:], pattern=[[1, GROUP]], base=0, channel_multiplier=0)
        nc.vector.tensor_copy(iotaf[:], ioi[:])

        # src gather indices: replicate compact 16-row wrap to 8 stripes
        idxv = bv_i16[o_idx // 2:o_idx // 2 + 16 * nblk * 8].rearrange(
            "(r c) -> r c", r=16)
        src_it = ptile("src_it", [128, nblk * 8], i16)
        for g in range(8):
            nc.sync.dma_start(src_it[:][16 * g:16 * (g + 1), :], idxv[:, :])
        offu = ptile("offu", [128, nblk], u8)
        nc.sync.dma_start(offu[:],
                          bv_u8[o_off:o_off + 128 * nblk].rearrange(
                              "(p c) -> p c", p=128))
        dst_off = ptile("dst_off", [128, nblk], f32)
        nc.vector.tensor_copy(dst_off[:], offu[:])

        # dst gather indices (local node = group*64 + off), derived on
        # device. gpsimd stripe g reads, for block j, only column j*8+g of
        # its 16 partitions; other columns are zeroed (never read).
        base16 = ptile("base16", [128, NGROUP], i32)
        nc.gpsimd.iota(base16[:], pattern=[[GROUP, NGROUP]], base=0,
                       channel_multiplier=0)
        basef = ptile("basef", [128, NGROUP], f32)
        nc.vector.tensor_copy(basef[:], base16[:])
        dstlocf = ptile("dstlocf", [128, nblk], f32)
        nc.vector.tensor_tensor(
            dstlocf[:].rearrange("p (g b) -> p g b", g=NGROUP),
            dst_off[:].rearrange("p (g b) -> p g b", g=NGROUP),
            bc(basef[:].rearrange("p (g a) -> p g a", a=1),
               (128, NGROUP, nbt)), OP.add)
        # wrap layout: slot [p, j*8+k] must hold dstloc[16k + p%16, j] -- a
        # partition interleave. DVE can't shuffle partitions, DMA can:
        # write the 8 bands interleaved to a DRAM row-block, replicate back.
        dstloci = ptile("dstloci", [128, nblk], i16)
        nc.vector.tensor_copy(dstloci[:], dstlocf[:])
        band_d = nc.dram_tensor("band", [16, nblk * 8], i16)
        bvw = band_d.ap().rearrange("r (j e) -> r j e", e=8)
        for k in range(8):
            nc.sync.dma_start(bvw[:, :, k:k + 1],
                              dstloci[:][16 * k:16 * (k + 1), :]
                              .rearrange("p (j a) -> p j a", a=1))
        dst_it = ptile("dst_it", [128, nblk * 8], i16)
        for g in range(8):
            nc.sync.dma_start(dst_it[:][16 * g:16 * (g + 1), :],
                              band_d.ap()[:, :])

        epsc = ptile("epsc", [128, 1], f32)
        nc.gpsimd.memset(epsc[:], LN_EPS)
        sel = ptile("sel", [128, nblk * GROUP], bf16)
        hlnT = ptile("hlnT", [64, N], f32)        # full h_ln^T after AG
        hT_loc_sb = ptile("hT_loc_sb", [64, P], f32)
        gat1 = ptile("gat1", [128, 8 * 72], f32)
        gat2 = ptile("gat2", [128, 8 * 56], f32)
        hln_rows = ptile("hln_rows", [128, 8 * 64], f32)
        htln = ptile("htln", [128, 8 * 6], f32)

        def rows_to_dram(dram, sb_view, ncols, col0=0, cast=None):
            """sb_view [128, 8, w] -> dram rows [(t*128+p), col0:col0+w]."""
            dv = dram.ap()
            if cast is not None:
                dv = dv.bitcast(cast)
            dv = dv.rearrange("(t p) d -> p t d", p=128)
            nc.sync.dma_start(dv[:, :, col0:col0 + ncols], sb_view)

        # ================= P0: host-projected h1a -> T1 =================
        with ExitStack() as ctx:
            pool = ctx.enter_context(tc.tile_pool(name="p0", bufs=2))
            hrows = pool.tile([128, 8 * 64], bf16, tag="hrows")
            nc.sync.dma_start(
                hrows[:].rearrange("p (t d) -> p t d", t=8),
                bv_bf[0:P * 64].rearrange("(t p d) -> p t d", t=8, p=128, d=64))
            a16 = pool.tile([128, 8 * 16], f16, tag="a16")
            nc.sync.dma_start(
                a16[:].rearrange("p (t d) -> p t d", t=8),
                bv_f16[o_a // 2:o_a // 2 + P * 16].rearrange(
                    "(t p d) -> p t d", t=8, p=128, d=16))
            arows = pool.tile([128, 8 * 16], f32, tag="arows")
            nc.vector.tensor_copy(arows[:], a16[:])
            ar3 = arows[:].rearrange("p (t d) -> p t d", t=8)
            rows_to_dram(t1_loc,
                         hrows[:].rearrange("p (t d) -> p t d", t=8)[:, :, :],
                         64, col0=0, cast=bf16)
            rows_to_dram(t1_loc, ar3[:, :, 0:8], 8, col0=32)
            # local adst table for the (small) dst gather; pad rows zeroed
            zero8 = pool.tile([128, 8], f32, tag="zero8")
            nc.gpsimd.memset(zero8[:], 0.0)
            t1av = t1a_loc.ap().rearrange("(t p) d -> p t d", p=128)
            nc.sync.dma_start(t1av[:, 0:8, 0:8], ar3[:, :, 8:16])
            nc.sync.dma_start(t1av[:, 8:9, 0:8],
                              zero8[:].rearrange("p (a d) -> p a d", a=1))
            nc.gpsimd.collective_compute(
                "AllGather", OP.bypass, replica_groups=[list(range(NCORES))],
                ins=[t1_loc.ap()[:]], outs=[t1_full.ap()[:]])

        # ============ edge phase (shared by both layers) ============
        def edge_phase(table, atable, it_src, it_dst, hcols, acol, gatacc,
                       build_sel):
            # hcols: # bf16 feature cols; acol: f32 col of asrc in src rows
            with ExitStack() as ctx:
                gp = ctx.enter_context(tc.tile_pool(name="gp", bufs=2))
                sp = ctx.enter_context(tc.tile_pool(name="sp", bufs=2))
                pg = ctx.enter_context(tc.tile_pool(name="pg", bufs=4,
                                                    space="PSUM"))
                for t in range(8):
                    j0 = t * CB
                    gs = gp.tile([128, CB * 64], f32, tag="gs")
                    nc.gpsimd.dma_gather(
                        gs[:].rearrange("p (j e) -> p j e", e=64),
                        table.ap()[:], it_src[:, j0 * 8:(j0 + CB) * 8],
                        num_idxs=CB * 128, num_idxs_reg=CB * 128, elem_size=64,
                        single_packet=False)
                    gd = gp.tile([128, CB * 64], f32, tag="gd")
                    nc.gpsimd.dma_gather(
                        gd[:].rearrange("p (j e) -> p j e", e=64),
                        atable.ap()[:], it_dst[:, j0 * 8:(j0 + CB) * 8],
                        num_idxs=CB * 128, num_idxs_reg=CB * 128, elem_size=64,
                        single_packet=False)
                    gs3 = gs[:].rearrange("p (j e) -> p j e", e=64)
                    gd3 = gd[:].rearrange("p (j e) -> p j e", e=64)
                    z = sp.tile([128, CB * 8], f32, tag="z")
                    z3 = z[:].rearrange("p (j e) -> p j e", e=8)
                    nc.vector.tensor_tensor(z3, gs3[:, :, acol:acol + 8],
                                            gd3[:, :, 0:8], OP.add)
                    u = sp.tile([128, CB * 8], f32, tag="u")
                    nc.vector.tensor_scalar_mul(u[:], z[:], 0.2)
                    nc.vector.tensor_max(z[:], z[:], u[:])
                    exf = sp.tile([128, CB * 8], f32, tag="exf")
                    nc.scalar.activation(exf[:], z[:], AF.Exp)
                    exb = sp.tile([128, CB * 8], bf16, tag="exb")
                    nc.vector.tensor_copy(exb[:], exf[:])
                    exb3 = exb[:].rearrange("p (j e) -> p j e", e=8)
                    W = hcols + 8
                    msgs = sp.tile([128, CB * W], bf16, tag="msgs")
                    m3 = msgs[:].rearrange("p (j e) -> p j e", e=W)
                    hb = gs3.bitcast(bf16)  # [128, CB, 128] bf16
                    exb4 = exb3.rearrange("p j (h a) -> p j h a", a=1)
                    nc.vector.tensor_tensor(
                        m3[:, :, 0:hcols].rearrange("p j (h c) -> p j h c", h=8),
                        hb[:, :, 0:hcols].rearrange("p j (h c) -> p j h c", h=8),
                        bc(exb4, (128, CB, 8, hcols // 8)), OP.mult)
                    nc.vector.tensor_copy(m3[:, :, hcols:W], exb3)
                    if build_sel:
                        sel3 = sel[:].rearrange("p (j e) -> p j e", e=GROUP)
                        io_b = bc(iotaf[:].rearrange("p (a e) -> p a e", a=1),
                                  (128, CB, GROUP))
                        do_b = bc(dst_off[:, j0:j0 + CB]
                                  .rearrange("p (j a) -> p j a", a=1),
                                  (128, CB, GROUP))
                        nc.vector.tensor_tensor(sel3[:, j0:j0 + CB, :], io_b,
                                                do_b, OP.is_equal)
                    sel3 = sel[:].rearrange("p (j e) -> p j e", e=GROUP)
                    ga = gatacc[:].rearrange("p (t d) -> p t d", d=W)
                    for g in (0, 1):
                        pgt = pg.tile([64, W], f32, tag="pgt")
                        for b in range(nbt):
                            jj = (2 * t + g) * nbt + b
                            nc.tensor.matmul(
                                pgt[:], sel3[:, jj, :], m3[:, jj - j0, :],
                                start=(b == 0), stop=(b == nbt - 1))
                        nc.scalar.activation(
                            ga[64 * g:64 * (g + 1), t, :], pgt[:], AF.Copy)

        edge_phase(t1_full, t1a_loc, src_it[:], dst_it[:], 64, 32, gat1,
                   build_sel=True)

        # ============ P2: GAT1 -> h_ln ============
        with ExitStack() as ctx:
            sp = ctx.enter_context(tc.tile_pool(name="p2", bufs=2))
            g3 = gat1[:].rearrange("p (t d) -> p t d", d=72)
            rec = sp.tile([128, 8 * 8], f32, tag="rec")
            nc.vector.reciprocal(rec[:].rearrange("p (t h) -> p t h", h=8),
                                 g3[:, :, 64:72])
            h1 = hln_rows[:].rearrange("p (t d) -> p t d", d=64)
            rec4 = rec[:].rearrange("p (t h a) -> p t h a", t=8, h=8)
            nc.vector.tensor_tensor(
                h1.rearrange("p t (h c) -> p t h c", h=8),
                g3[:, :, 0:64].rearrange("p t (h c) -> p t h c", h=8),
                bc(rec4, (128, 8, 8, 8)), OP.mult)
            b1b = bc(b1r.rearrange("p (a d) -> p a d", a=1), (128, 8, 64))
            nc.vector.tensor_tensor(h1, h1, b1b, OP.add)
            # layernorm over 64
            rs_ = sp.tile([128, 8], f32, tag="rs_")
            nc.vector.tensor_reduce(rs_[:], h1, mybir.AxisListType.X, OP.add)
            mean = sp.tile([128, 8], f32, tag="mean")
            nc.scalar.mul(mean[:], rs_[:], 1.0 / 64)
            nc.vector.tensor_tensor(
                h1, h1, bc(mean[:].rearrange("p (t a) -> p t a", a=1),
                           (128, 8, 64)), OP.subtract)
            sq = sp.tile([128, 8 * 64], f32, tag="sq")
            ssum = sp.tile([128, 8], f32, tag="ssum")
            sq3 = sq[:].rearrange("p (t d) -> p t d", d=64)
            nc.scalar.activation(sq3, h1, AF.Square)
            nc.vector.tensor_reduce(ssum[:], sq3, mybir.AxisListType.X, OP.add)
            std_ = sp.tile([128, 8], f32, tag="std_")
            nc.scalar.activation(std_[:], ssum[:], AF.Sqrt, bias=epsc[:],
                                 scale=1.0 / 64)
            rstd = sp.tile([128, 8], f32, tag="rstd")
            nc.vector.reciprocal(rstd[:], std_[:])
            nc.vector.tensor_tensor(
                h1, h1, bc(rstd[:].rearrange("p (t a) -> p t a", a=1),
                           (128, 8, 64)), OP.mult)
            nc.vector.tensor_tensor(
                h1, h1, bc(l1g.rearrange("p (a d) -> p a d", a=1),
                           (128, 8, 64)), OP.mult)
            nc.vector.tensor_tensor(
                h1, h1, bc(l1b.rearrange("p (a d) -> p a d", a=1),
                           (128, 8, 64)), OP.add)
            # elu
            mn = sp.tile([128, 8 * 64], f32, tag="mn")
            nc.vector.tensor_scalar_min(mn[:], hln_rows[:], 0.0)
            ee = sp.tile([128, 8 * 64], f32, tag="ee")
            nc.scalar.activation(ee[:], mn[:], AF.Exp)
            nc.vector.tensor_scalar_max(hln_rows[:], hln_rows[:], 0.0)
            nc.vector.tensor_add(hln_rows[:], hln_rows[:], ee[:])
            nc.vector.tensor_scalar_add(hln_rows[:], hln_rows[:], -1.0)

        # ============ P3: transpose + AG h_ln^T ============
        with ExitStack() as ctx:
            ps = ctx.enter_context(tc.tile_pool(name="p3ps", bufs=3,
                                                space="PSUM"))
            hr = hln_rows[:].rearrange("p (t d) -> p t d", d=64)
            for m in range(8):
                pt = ps.tile([64, 128], f32, tag="pt")
                nc.tensor.transpose(pt[:], hr[:, m, :], eye[:])
                nc.vector.tensor_copy(hT_loc_sb[:, m * 128:(m + 1) * 128], pt[:])
            nc.sync.dma_start(ht_loc.ap()[:], hT_loc_sb[:])
            nc.gpsimd.collective_compute(
                "AllGather", OP.bypass, replica_groups=[list(range(NCORES))],
                ins=[ht_loc.ap()[:]], outs=[ht_ag.ap()[:]])
            for c in range(NCORES):
                nc.sync.dma_start(hlnT[:, c * P:(c + 1) * P],
                                  ht_ag.ap()[c * 64:(c + 1) * 64, :])

        # ============ P4: attention ============
        with ExitStack() as ctx:
            pool = ctx.enter_context(tc.tile_pool(name="p4", bufs=2))
            ps = ctx.enter_context(tc.tile_pool(name="p4ps", bufs=2,
                                                space="PSUM"))
            pvps = ctx.enter_context(tc.tile_pool(name="pvps", bufs=1,
                                                  space="PSUM"))
            kT = pers.tile([64, N], f32r, name="kT", tag="kT")
            qT = pers.tile([64, P], f32r, name="qT", tag="qT")
            vaug = pers.tile([128, 64 * 7], bf16, name="vaug", tag="vaug")
            for j in range(16):
                pk = ps.tile([64, 512], f32, tag="pss")
                nc.tensor.matmul(pk[:], wk[:], hlnT[:, j * 512:(j + 1) * 512],
                                 start=True, stop=True)
                nc.vector.tensor_copy(kT[:, j * 512:(j + 1) * 512], pk[:])
            for j in range(2):
                pq = ps.tile([64, 512], f32, tag="pss")
                nc.tensor.matmul(pq[:], wq[:],
                                 hT_loc_sb[:, j * 512:(j + 1) * 512],
                                 start=True, stop=True)
                nc.vector.tensor_copy(qT[:, j * 512:(j + 1) * 512], pq[:])
            va3 = vaug[:].rearrange("p (n d) -> p n d", d=7)
            for nt in range(64):
                pv = ps.tile([128, 7], f32, tag="pss")
                nc.tensor.matmul(pv[:], hlnT[:, nt * 128:(nt + 1) * 128],
                                 wv7[:], start=True, stop=True)
                nc.vector.tensor_copy(va3[:, nt, :], pv[:])
            nc.gpsimd.memset(va3[:, :, 6:7], 1.0)

            NTB = 3  # n-tiles per psum batch (3 banks)
            att = pool.tile([128, 8 * 7], f32, tag="att")
            at3 = att[:].rearrange("p (t d) -> p t d", d=7)
            for mc in range(2):
                po = pvps.tile([7, 512], f32, tag="po")
                nb_list = [(s, min(s + NTB, 64)) for s in range(0, 64, NTB)]
                for (s0, s1) in nb_list:
                    w = (s1 - s0) * 512
                    pss = ps.tile([128, NTB * 512], f32, tag="pss")
                    for i, nt in enumerate(range(s0, s1)):
                        nc.tensor.matmul(
                            pss[:, i * 512:(i + 1) * 512],
                            kT[:, nt * 128:(nt + 1) * 128],
                            qT[:, mc * 512:(mc + 1) * 512],
                            start=True, stop=True)
                    pT = pool.tile([128, NTB * 512], bf16, tag="pT")
                    nc.scalar.activation(pT[:, 0:w], pss[:, 0:w], AF.Exp,
                                         scale=0.125)
                    for i, nt in enumerate(range(s0, s1)):
                        nc.tensor.matmul(
                            po[:], va3[:, nt, :].bitcast(bf16),
                            pT[:, i * 512:(i + 1) * 512],
                            start=(nt == 0), stop=(nt == 63),
                            skip_group_check=True)
                spo = pool.tile([7, 512], f32, tag="spo")
                nc.vector.tensor_copy(spo[:], po[:])
                for i in range(4):
                    ptr = ps.tile([128, 7], f32, tag="pss")
                    nc.tensor.transpose(ptr[:], spo[:, i * 128:(i + 1) * 128],
                                        eye[0:7, 0:7])
                    nc.vector.tensor_copy(at3[:, mc * 4 + i, :], ptr[:])
            # normalize + LN over 6
            rec = pool.tile([128, 8], f32, tag="reca")
            nc.vector.reciprocal(rec[:].rearrange("p (t a) -> p t a", a=1),
                                 at3[:, :, 6:7])
            ht3 = htln[:].rearrange("p (t d) -> p t d", d=6)
            nc.vector.tensor_tensor(
                ht3, at3[:, :, 0:6],
                bc(rec[:].rearrange("p (t a) -> p t a", a=1), (128, 8, 6)),
                OP.mult)
            rs_ = pool.tile([128, 8], f32, tag="rsb")
            nc.vector.tensor_reduce(rs_[:], ht3, mybir.AxisListType.X, OP.add)
            mean = pool.tile([128, 8], f32, tag="meanb")
            nc.scalar.mul(mean[:], rs_[:], 1.0 / 6)
            nc.vector.tensor_tensor(
                ht3, ht3, bc(mean[:].rearrange("p (t a) -> p t a", a=1),
                             (128, 8, 6)), OP.subtract)
            sq = pool.tile([128, 8 * 6], f32, tag="sqb")
            ssum = pool.tile([128, 8], f32, tag="ssumb")
            sq3b = sq[:].rearrange("p (t d) -> p t d", d=6)
            nc.scalar.activation(sq3b, ht3, AF.Square)
            nc.vector.tensor_reduce(ssum[:], sq3b, mybir.AxisListType.X, OP.add)
            stdb = pool.tile([128, 8], f32, tag="stdb")
            nc.scalar.activation(stdb[:], ssum[:], AF.Sqrt, bias=epsc[:],
                                 scale=1.0 / 6)
            rstd = pool.tile([128, 8], f32, tag="rstdb")
            nc.vector.reciprocal(rstd[:], stdb[:])
            nc.vector.tensor_tensor(
                ht3, ht3, bc(rstd[:].rearrange("p (t a) -> p t a", a=1),
                             (128, 8, 6)), OP.mult)
            nc.vector.tensor_tensor(
                ht3, ht3, bc(l2g.rearrange("p (a d) -> p a d", a=1),
                             (128, 8, 6)), OP.mult)
            nc.vector.tensor_tensor(
                ht3, ht3, bc(l2b.rearrange("p (a d) -> p a d", a=1),
                             (128, 8, 6)), OP.add)

        # ============ P5: T2 build + AG ============
        with ExitStack() as ctx:
            pool = ctx.enter_context(tc.tile_pool(name="p5", bufs=3))
            ps = ctx.enter_context(tc.tile_pool(name="p5ps", bufs=3,
                                                space="PSUM"))
            htT = pool.tile([6, P], f32, tag="htT")
            ht3 = htln[:].rearrange("p (t d) -> p t d", d=6)
            for m in range(8):
                pt = ps.tile([6, 128], f32, tag="pt2")
                nc.tensor.transpose(pt[:], ht3[:, m, :], eye[:])
                nc.vector.tensor_copy(htT[:, m * 128:(m + 1) * 128], pt[:])
            h2a = pool.tile([128, 8 * 64], f32, tag="h2a")
            h2b = pool.tile([128, 8 * 48], bf16, tag="h2b")
            h2a3 = h2a[:].rearrange("p (t d) -> p t d", d=64)
            h2b3 = h2b[:].rearrange("p (t d) -> p t d", d=48)
            for m in range(8):
                pm = ps.tile([128, 64], f32, tag="pm2")
                nc.tensor.matmul(pm[:], hT_loc_sb[:, m * 128:(m + 1) * 128],
                                 w2top[:], start=True, stop=False)
                nc.tensor.matmul(pm[:], htT[:, m * 128:(m + 1) * 128],
                                 w2bot[:], start=False, stop=True)
                nc.scalar.activation(h2a3[:, m, :], pm[:], AF.Copy)
                nc.vector.tensor_copy(h2b3[:, m, :], pm[:, 0:48])
            rows_to_dram(t2_loc, h2b3[:, :, :], 48, col0=0, cast=bf16)
            rows_to_dram(t2_loc, h2a3[:, :, 48:56], 8, col0=24)
            zero8b = pool.tile([128, 8], f32, tag="zero8b")
            nc.gpsimd.memset(zero8b[:], 0.0)
            t2av = t2a_loc.ap().rearrange("(t p) d -> p t d", p=128)
            nc.sync.dma_start(t2av[:, 0:8, 0:8], h2a3[:, :, 56:64])
            nc.sync.dma_start(t2av[:, 8:9, 0:8],
                              zero8b[:].rearrange("p (a d) -> p a d", a=1))
            nc.gpsimd.collective_compute(
                "AllGather", OP.bypass, replica_groups=[list(range(NCORES))],
                ins=[t2_loc.ap()[:]], outs=[t2_full.ap()[:]])

        # ============ P6: GAT2 edge phase ============
        edge_phase(t2_full, t2a_loc, src_it[:], dst_it[:], 48, 24, gat2,
                   build_sel=False)

        # ============ P7: finale ============
        with ExitStack() as ctx:
            sp = ctx.enter_context(tc.tile_pool(name="p7", bufs=2))
            g3 = gat2[:].rearrange("p (t d) -> p t d", d=56)
            d8 = sp.tile([128, 8 * 8], f32, tag="d8")
            nc.vector.tensor_scalar_mul(d8[:].rearrange("p (t h) -> p t h", h=8),
                                        g3[:, :, 48:56], 8.0)
            rec = sp.tile([128, 8 * 8], f32, tag="rec2")
            nc.vector.reciprocal(rec[:], d8[:])
            avg = sp.tile([128, 8 * 48], f32, tag="avg")
            a4 = avg[:].rearrange("p (t h c) -> p t h c", t=8, h=8)
            rec4 = rec[:].rearrange("p (t h a) -> p t h a", t=8, h=8)
            nc.vector.tensor_tensor(
                a4, g3[:, :, 0:48].rearrange("p t (h c) -> p t h c", h=8),
                bc(rec4, (128, 8, 8, 6)), OP.mult)
            swp = sp.tile([128, 8 * 48], f32, tag="swp")
            s4 = swp[:].rearrange("p (t c h) -> p t c h", t=8, c=6)
            nc.vector.tensor_copy(
                s4, avg[:].rearrange("p (t h c) -> p t h c", t=8, h=8)
                .rearrange("p t h c -> p t c h"))
            out2 = sp.tile([128, 8 * 6], f32, tag="out2")
            o3 = out2[:].rearrange("p (t d) -> p t d", d=6)
            nc.vector.tensor_reduce(o3, s4, mybir.AxisListType.X, OP.add)
            nc.vector.tensor_tensor(
                o3, o3, bc(b2r.rearrange("p (a d) -> p a d", a=1),
                           (128, 8, 6)), OP.add)
            ex = sp.tile([128, 8 * 6], f32, tag="exo")
            es = sp.tile([128, 8], f32, tag="eso")
            ex3 = ex[:].rearrange("p (t d) -> p t d", d=6)
            nc.scalar.activation(ex3, o3, AF.Exp)
            nc.vector.tensor_reduce(es[:], ex3, mybir.AxisListType.X, OP.add)
            ls = sp.tile([128, 8], f32, tag="lso")
            nc.scalar.activation(ls[:], es[:], AF.Ln)
            nc.vector.tensor_tensor(
                o3, o3, bc(ls[:].rearrange("p (t a) -> p t a", a=1),
                           (128, 8, 6)), OP.subtract)
            rows_to_dram(out_d, o3[:, :, :], 6)

    nc.compile()
    return nc


# ---------------- dispatch layer (cached jit over PJRT) ----------------

_SESS = {}


def _get_session(nbt):
    if nbt in _SESS:
        return _SESS[nbt]
    nc = build_kernel(nbt)
    bass2jax.install_neuronx_cc_hook()
    partition_name = (nc.partition_id_tensor.name
                      if nc.partition_id_tensor else None)
    in_names, out_names, out_avals = [], [], []
    for alloc in nc.m.functions[0].allocations:
        if not isinstance(alloc, mybir.MemoryLocationSet):
            continue
        name = alloc.memorylocations[0].name
        if alloc.kind == "ExternalInput":
            if name != partition_name:
                in_names.append(name)
        elif alloc.kind == "ExternalOutput":
            out_names.append(name)
            out_avals.append(jax.core.ShapedArray(
                tuple(alloc.tensor_shape), mybir.dt.np(alloc.dtype)))
    n_params = len(in_names)
    n_outs = len(out_avals)
    all_names = list(in_names) + list(out_names)
    if partition_name is not None:
        all_names.append(partition_name)

    def _body(*args):
        operands = list(args)
        if partition_name is not None:
            operands.append(bass2jax.partition_id_tensor())
        outs = bass2jax._bass_exec_p.bind(
            *operands,
            out_avals=tuple(out_avals),
            in_names=tuple(all_names),
            out_names=tuple(out_names),
            lowering_input_output_aliases=(),
            sim_require_finite=True,
            sim_require_nnan=True,
            nc=nc,
        )
        return tuple(outs)

    devices = jax.devices()[:NCORES]
    mesh = Mesh(np.asarray(devices), ("core",))
    sharding = NamedSharding(mesh, PartitionSpec("core"))
    sharded = jax.jit(
        shard_map(_body, mesh=mesh,
                  in_specs=(PartitionSpec("core"),) * (n_params + n_outs),
                  out_specs=(PartitionSpec("core"),) * n_outs,
                  check_rep=False),
        donate_argnums=tuple(range(n_params, n_params + n_outs)),
        keep_unused=True)
    zeros_fns = [
        jax.jit(partial(jnp.zeros,
                        (NCORES * a.shape[0], *a.shape[1:]), a.dtype),
                out_shardings=sharding)
        for a in out_avals]
    sess = dict(sharded=sharded, in_names=in_names, out_names=out_names,
                zeros_fns=zeros_fns)
    _SESS[nbt] = sess
    return sess


def _prepare_arrays(inputs):
    """Host prep: full np inputs -> (concat input arrays by name, nbt)."""
    x = np.asarray(inputs["x"], np.float32)
    W1 = np.asarray(inputs["W1"], np.float32)
    W1aug = np.concatenate(
        [W1, W1 @ expand_att(np.asarray(inputs["a_src1"], np.float32)),
         W1 @ expand_att(np.asarray(inputs["a_dst1"], np.float32))],
        axis=1)                                          # [256, 80]
    h1a = x @ W1aug                                      # [N, 80] f32
    h_b = np.ascontiguousarray(h1a[:, 0:64]).astype(BF16)
    a_h = np.ascontiguousarray(h1a[:, 64:80]).astype(np.float16)
    idx, off, nbt = prep_edges(np.asarray(inputs["edge_index"]))
    wblob = prep_weights(
        inputs["a_src2"], inputs["a_dst2"], inputs["b1"],
        inputs["ln1_g"], inputs["ln1_b"], inputs["Wq"], inputs["Wk"],
        inputs["Wv"], inputs["ln2_g"], inputs["ln2_b"], inputs["W2"],
        inputs["b2"])
    o_a, o_idx, o_off, o_w, b_bytes = blob_layout(nbt)
    blob = np.zeros((NCORES, b_bytes), np.uint8)
    for c in range(NCORES):
        for o, arr in ((0, h_b[c * P:(c + 1) * P]),
                       (o_a, a_h[c * P:(c + 1) * P]),
                       (o_idx, idx[c]), (o_off, off[c]), (o_w, wblob[c])):
            bts = np.frombuffer(arr.tobytes(), np.uint8)
            blob[c, o:o + bts.size] = bts
    return {"blob": blob}, nbt


def _run(sess, arrs):
    """Dispatch: host np inputs -> host np output [N, 6]."""
    ins = [arrs[nm] for nm in sess["in_names"]]
    zs = [zf() for zf in sess["zeros_fns"]]
    outs = sess["sharded"](*ins, *zs)
    out = np.asarray(outs[sess["out_names"].index("out")])
    return out.reshape(N, 6)


def kernel(**inputs):
    arrs, nbt = _prepare_arrays(inputs)
    last_err = None
    for backoff in (10, 15, 30, 45, 60, 0):
        try:
            sess = _get_session(nbt)
            out = _run(sess, arrs)
            if np.isfinite(out).all():
                return out
            last_err = RuntimeError("non-finite output")
        except Exception as e:  # transient NRT/axon failures
            last_err = e
        _SESS.pop(nbt, None)
        time.sleep(backoff)
    raise last_err
